# revision 1
# baseline (speedup 1.0000x reference)
"""Self-contained Trainium2 kernel for the dense transformer block problem.

kernel(**inputs) takes the FULL inputs (as produced by the reference
setup_inputs), shards them across 8 NeuronCores (2 cores per batch element,
causal-balanced parity split of query blocks), runs a Bass/Tile SPMD kernel,
and reassembles the full [B, T, C] output.
"""
"""Transformer block (pre-LN attention + MLP) for trn2, 8-core SPMD.

Sharding: 2 cores per batch element (B=4). Within a pair, query blocks of 128
tokens are split by parity (core parity p owns global blocks {2j+p}), which
balances causal attention work. Each core computes K/V for the full sequence
of its batch element (redundant within the pair) so there are no collectives.

Per-core layouts:
  activations for matmuls flow transposed: [C_chunk x 128 partitions, tokens], f32r
  attention: scoresT [keys, q] (f32r matmul) -> +mask bias -> exp (ACT) -> weiT bf16
  V is bf16, augmented with a ones column; AV accumulates [attn^T ; rowsum] in PSUM
  softmax normalization via PE transpose + per-partition reciprocal
"""
import sys
sys.path.insert(0, '/opt/trn_rl_repo')
import numpy as np
from contextlib import ExitStack

import concourse.bacc as bacc
import concourse.tile as tile
import concourse.mybir as mybir
from concourse.masks import make_identity

F32 = mybir.dt.float32
F32R = mybir.dt.float32r
BF16 = mybir.dt.bfloat16
AF = mybir.ActivationFunctionType
ALU = mybir.AluOpType

B, T, C, H, DH = 4, 2048, 1024, 16, 64
N_CORES = 8
TOK = 1024          # own tokens per core
NB = TOK // 128     # 8 own query blocks
KB = T // 128       # 16 key blocks
CCH = C // 128      # 8 channel chunks
FF = 4 * C          # 4096
FCH = FF // 128     # 32 ff chunks
EPS = 1e-5
NEG = -1e30

IN_NAMES = ["xfull", "xown", "qpos", "Wq", "Wk", "Wv", "Wp", "bp",
            "W1", "b1", "W2", "b2", "qbias", "kbias", "vbias"]


def _score_chunks(nq):
    """split nq (multiple of 128) into pieces, avoiding <256 pieces when possible"""
    out = []
    rem = nq
    while rem > 0:
        if rem == 640:
            take = 384
        elif rem >= 512:
            take = 512
        else:
            take = rem
        out.append(take)
        rem -= take
    return out


def build(nc, reps=1):
    """Trace the SPMD program into nc (a bacc.Bacc). Call nc.compile() after.

    Weight inputs arrive pre-folded on the host:
      Wq/Wk/Wv = diag(g1) @ W (dtype f32r);  qbias/kbias/vbias = be1 @ W
      W1 = diag(g2) @ W1 (f32r);  b1 = b1 + be2 @ W1
      Wp, W2 plain f32r.  g/be tensors are consumed host-side only.
    """
    def din(name, shape, dt=F32):
        return nc.dram_tensor(name, shape, dt, kind="ExternalInput")

    xfull_d = din("xfull", [T, C])
    xown_d = din("xown", [TOK, C])
    qpos_d = din("qpos", [NB, 128])
    Wq_d = din("Wq", [C, C], F32R); Wk_d = din("Wk", [C, C], F32R)
    Wv_d = din("Wv", [C, C], F32R); Wp_d = din("Wp", [C, C], F32R)
    bp_d = din("bp", [1, C]); W1_d = din("W1", [C, FF], F32R); b1_d = din("b1", [1, FF])
    W2_d = din("W2", [FF, C], F32R); b2_d = din("b2", [1, C])
    qb_d = din("qbias", [NB, 128])   # be1 @ Wq, laid out [pair, dh-stacked 128]
    kb_d = din("kbias", [NB, 128])   # be1 @ Wk
    vb_d = din("vbias", [1, C])      # be1 @ Wv
    out_d = nc.dram_tensor("out", [TOK, C], F32, kind="ExternalOutput")
    x2_d = nc.dram_tensor("x2_scratch", [TOK, C], F32)  # internal DRAM scratch
    attnT_d = nc.dram_tensor("attnT_scratch", [C, TOK], F32R)  # [dh-stacked C, own tokens]

    Wqv = Wq_d.ap().rearrange("(o p) m -> o p m", p=128)
    Wkv = Wk_d.ap().rearrange("(o p) m -> o p m", p=128)
    Wvv = Wv_d.ap().rearrange("(o p) m -> o p m", p=128)
    Wpv = Wp_d.ap().rearrange("(o p) m -> o p m", p=128)
    W1v = W1_d.ap().rearrange("(o p) m -> o p m", p=128)
    W2v = W2_d.ap().rearrange("(o p) m -> p o m", p=128)  # [128, 32, 1024]
    xf = xfull_d.ap()
    xo = xown_d.ap()

    for _rep in range(reps):
        _build_one(nc, locals())
    return IN_NAMES


def _build_one(nc, env):
    (xfull_d, xown_d, qpos_d, Wq_d, Wk_d, Wv_d, Wp_d, bp_d, W1_d, b1_d, W2_d,
     b2_d, qb_d, kb_d, vb_d, out_d, x2_d, attnT_d, Wqv, Wkv, Wvv, Wpv, W1v, W2v,
     xf, xo) = (
        env[k] for k in ["xfull_d", "xown_d", "qpos_d", "Wq_d", "Wk_d", "Wv_d",
                         "Wp_d", "bp_d", "W1_d", "b1_d", "W2_d", "b2_d", "qb_d",
                         "kb_d", "vb_d", "out_d", "x2_d", "attnT_d", "Wqv", "Wkv",
                         "Wvv", "Wpv", "W1v", "W2v", "xf", "xo"])
    import concourse.tile as tile
    from contextlib import ExitStack
    with tile.TileContext(nc) as tc, ExitStack() as top:
        const = top.enter_context(tc.tile_pool(name="const", bufs=1))
        ident = const.tile([128, 128], F32)
        make_identity(nc, ident[:])
        eps_t = const.tile([128, 1], F32)
        nc.vector.memset(eps_t[:], EPS)

        def ln_stats(nc, pool, x_ap):
            n = x_ap.shape[-1] // 512
            xg = x_ap.rearrange("p (n f) -> p n f", f=512)
            stats = pool.tile([128, n, 6], F32, tag="ln_stats")
            mv = pool.tile([128, 2], F32, tag="ln_mv")
            for i in range(n):
                nc.vector.bn_stats(stats[:, i], xg[:, i])
            nc.vector.bn_aggr(mv[:], stats[:])
            rstd = pool.tile([128, 1], F32, tag="ln_rstd")
            nc.scalar.activation(rstd[:], mv[:, 1:2], AF.Sqrt, bias=eps_t[:])
            nc.vector.reciprocal(rstd[:], rstd[:])
            return mv[:, 0:1], rstd

        def ln_apply(nc, pool, out_ap, x_ap, mean, rstd):
            # out = (x - mu) * rstd on ACT: Identity(x * rstd + (-mu * rstd))
            nmr = pool.tile([128, 1], F32, tag="ln_nmr")
            nc.vector.tensor_scalar(nmr[:], mean, rstd[:], -1.0,
                                    op0=ALU.mult, op1=ALU.mult)
            nc.scalar.activation(out_ap, x_ap, AF.Identity,
                                 bias=nmr[:], scale=rstd[:])

        # ============ Stage A: LN1 over full T -> hT [128, CCH, T] f32r ============
        es_h = ExitStack()
        hp = es_h.enter_context(tc.tile_pool(name="hT", bufs=1, side="right"))
        hT = hp.tile([128, CCH, T], F32R)
        with tc.tile_pool(name="stA", bufs=3) as stA, \
             tc.tile_pool(name="stA_ps", bufs=3, space="PSUM") as psA:
            for tb in range(T // 128):
                x_t = stA.tile([128, C], F32, tag="x_t")
                nc.sync.dma_start(x_t[:], xf[tb * 128:(tb + 1) * 128, :])
                mean, rstd = ln_stats(nc, stA, x_t[:])
                hrow = stA.tile([128, C], F32, tag="hrow")
                ln_apply(nc, stA, hrow[:], x_t[:], mean, rstd)
                for cc in range(CCH):
                    pt = psA.tile([128, 128], F32, tag="psA_t")
                    nc.tensor.transpose(pt[:], hrow[:, cc * 128:(cc + 1) * 128], ident[:])
                    eng = nc.scalar.copy if cc % 2 == 0 else nc.vector.tensor_copy
                    eng(hT[:, cc, tb * 128:(tb + 1) * 128], pt[:])

        # ============ Stage B1: V (token-major, bf16, ones-augmented) ============
        es_qkv = ExitStack()
        vp = es_qkv.enter_context(tc.tile_pool(name="Vp", bufs=1))
        V_sb = vp.tile([128, KB, H, 65], F32R)
        ones_f = vp.tile([128, 1], F32)
        nc.vector.memset(ones_f[:], 1.0)
        ones_r = vp.tile([128, 1], F32R)
        nc.vector.tensor_copy(ones_r[:], ones_f[:])
        nc.vector.tensor_copy(V_sb[:, :, :, 64:65],
                              ones_r[:, 0:1, None, None].to_broadcast([128, KB, H, 1]))
        with tc.tile_pool(name="stB1a", bufs=2) as stB1a, \
             tc.tile_pool(name="stB1c", bufs=1) as stB1c, \
             tc.tile_pool(name="stB1_ps", bufs=2, space="PSUM") as psB1:
            vb_b = stB1c.tile([128, C], F32)
            nc.sync.dma_start(vb_b[:], vb_d.ap().to_broadcast([128, C]))
            for grp in range(2):
                wv_g = stB1a.tile([128, CCH, 512], F32R, tag="wv_g")
                nc.sync.dma_start(wv_g[:], Wvv.transpose([1, 0, 2])[:, :, grp * 512:(grp + 1) * 512])
                for tb in range(KB):
                    pv = psB1.tile([128, 512], F32, tag="pv")
                    for cc in range(CCH):
                        nc.tensor.matmul(pv[:], hT[:, cc, tb * 128:(tb + 1) * 128],
                                         wv_g[:, cc], start=(cc == 0), stop=(cc == CCH - 1))
                    nc.vector.tensor_tensor(
                        V_sb[:, tb, grp * 8:(grp + 1) * 8, 0:64],
                        pv[:].rearrange("p (h d) -> p h d", d=64),
                        vb_b[:, grp * 512:(grp + 1) * 512].rearrange("p (h d) -> p h d", d=64),
                        ALU.add)

        # ============ Stage B2: KT [128(dh pair-stacked), pair, T] f32r ============
        ktp = es_qkv.enter_context(tc.tile_pool(name="KTp", bufs=1))
        KT = ktp.tile([128, CCH, T], F32R)
        with tc.tile_pool(name="stB2", bufs=2) as stB2, \
             tc.tile_pool(name="stB2c", bufs=1) as stB2c, \
             tc.tile_pool(name="stB2_ps", bufs=3, space="PSUM") as psB2:
            kb_sb = stB2c.tile([128, NB], F32)
            nc.sync.dma_start(kb_sb[:], kb_d.ap().rearrange("o p -> p o"))
            for pair in range(CCH):
                wk_p = stB2.tile([128, CCH, 128], F32R, tag="wk_p")
                nc.sync.dma_start(wk_p[:], Wkv.transpose([1, 0, 2])[:, :, pair * 128:(pair + 1) * 128])
                for nt in range(T // 512):
                    pk = psB2.tile([128, 512], F32, tag="pk")
                    for cc in range(CCH):
                        nc.tensor.matmul(pk[:], wk_p[:, cc],
                                         hT[:, cc, nt * 512:(nt + 1) * 512],
                                         start=(cc == 0), stop=(cc == CCH - 1))
                    nc.vector.tensor_scalar(KT[:, pair, nt * 512:(nt + 1) * 512], pk[:],
                                            kb_sb[:, pair:pair + 1], None, op0=ALU.add)

        # ============ Stage A': LN1 of own rows -> hTown; then B3: QT ============
        es_h.close()  # free hT
        es_ho = ExitStack()
        hop = es_ho.enter_context(tc.tile_pool(name="hTown", bufs=1, side="right"))
        hTown = hop.tile([128, CCH, TOK], F32R)
        with tc.tile_pool(name="stA2", bufs=3) as stA2, \
             tc.tile_pool(name="stA2_ps", bufs=3, space="PSUM") as psA2:
            for tb in range(NB):
                x_t = stA2.tile([128, C], F32, tag="x_t2")
                nc.sync.dma_start(x_t[:], xo[tb * 128:(tb + 1) * 128, :])
                mean, rstd = ln_stats(nc, stA2, x_t[:])
                hrow = stA2.tile([128, C], F32, tag="hrow2")
                ln_apply(nc, stA2, hrow[:], x_t[:], mean, rstd)
                for cc in range(CCH):
                    pt = psA2.tile([128, 128], F32, tag="psA2_t")
                    nc.tensor.transpose(pt[:], hrow[:, cc * 128:(cc + 1) * 128], ident[:])
                    eng = nc.scalar.copy if cc % 2 == 0 else nc.vector.tensor_copy
                    eng(hTown[:, cc, tb * 128:(tb + 1) * 128], pt[:])

        qtp = es_qkv.enter_context(tc.tile_pool(name="QTp", bufs=1))
        QT = qtp.tile([128, CCH, TOK], F32R)
        with tc.tile_pool(name="stB3", bufs=2) as stB3, \
             tc.tile_pool(name="stB3c", bufs=1) as stB3c, \
             tc.tile_pool(name="stB3_ps", bufs=3, space="PSUM") as psB3:
            qb_sb = stB3c.tile([128, NB], F32)
            nc.sync.dma_start(qb_sb[:], qb_d.ap().rearrange("o p -> p o"))
            for pair in range(CCH):
                wq_p = stB3.tile([128, CCH, 128], F32R, tag="wq_p")
                nc.sync.dma_start(wq_p[:], Wqv.transpose([1, 0, 2])[:, :, pair * 128:(pair + 1) * 128])
                for nt in range(TOK // 512):
                    pq = psB3.tile([128, 512], F32, tag="pq")
                    for cc in range(CCH):
                        nc.tensor.matmul(pq[:], wq_p[:, cc],
                                         hTown[:, cc, nt * 512:(nt + 1) * 512],
                                         start=(cc == 0), stop=(cc == CCH - 1))
                    nc.vector.tensor_scalar(QT[:, pair, nt * 512:(nt + 1) * 512], pq[:],
                                            qb_sb[:, pair:pair + 1], None, op0=ALU.add)
        es_ho.close()  # free hTown

        # ---------- mask constants (scoped to attention) ----------
        es_mask = ExitStack()
        maskp = es_mask.enter_context(tc.tile_pool(name="maskp", bufs=1, side="right"))
        kp_i = maskp.tile([128, KB], mybir.dt.int32)
        nc.gpsimd.iota(kp_i[:], pattern=[[128, KB]], base=0, channel_multiplier=1)
        kp_f = maskp.tile([128, KB], F32)
        nc.vector.tensor_copy(kp_f[:], kp_i[:])
        qb = maskp.tile([128, NB, 128], F32)
        for j in range(NB):
            nc.sync.dma_start(qb[:, j], qpos_d.ap()[j:j + 1, :].to_broadcast([128, 128]))
        biasm = maskp.tile([128, NB, 2, 128], F32)
        for j in range(NB):
            for t in range(2):
                # m01[p_key, f_q] = (qpos_j[f] >= keypos(k=2j+t)[p])
                nc.vector.tensor_scalar(
                    biasm[:, j, t], qb[:, j], kp_f[:, 2 * j + t:2 * j + t + 1], None,
                    op0=ALU.is_ge)

        # ============ Stage C: attention ============
        with tc.tile_pool(name="stC", bufs=3) as stC, \
             tc.tile_pool(name="stC_att_ps", bufs=2, space="PSUM") as psCa, \
             tc.tile_pool(name="stC_s_ps", bufs=2, space="PSUM") as psCs, \
             tc.tile_pool(name="stC_t_ps", bufs=2, space="PSUM") as psCt:
            for h in range(H):
                pair, off = h // 2, 64 * (h % 2)
                ps_att = psCa.tile([128, TOK], F32, tag="ps_att")
                for k in range(KB):
                    jmin = k // 2
                    q0 = jmin * 128
                    nq = TOK - q0
                    weiT = stC.tile([128, TOK], F32R, tag="weiT")
                    qa = 0
                    while qa < nq:  # one 1-bank psum tile + one exp per 512 cols
                        qn = min(512, nq - qa)
                        ps_s = psCs.tile([128, 512], F32, tag="ps_s")
                        nc.tensor.matmul(
                            ps_s[:, 0:qn],
                            KT[off:off + 64, pair, k * 128:(k + 1) * 128],
                            QT[off:off + 64, pair, q0 + qa:q0 + qa + qn],
                            start=True, stop=True)
                        nc.scalar.activation(weiT[:, qa:qa + qn], ps_s[:, 0:qn],
                                             AF.Exp, scale=0.125)
                        qa += qn
                    nc.vector.tensor_tensor(weiT[:, 0:128], weiT[:, 0:128],
                                            biasm[:, jmin, k - 2 * jmin], ALU.mult)
                    # AV: one matmul per 512-col PSUM bank (start=True must
                    # clear a whole bank, so groups are bank-aligned)
                    if k <= 7:  # bank 0: q cols [q0, 512)
                        nc.tensor.matmul(
                            ps_att[0:65, q0:512],
                            V_sb[:, k, h, :],
                            weiT[:, 0:512 - q0],
                            start=(k == 0), stop=(k == 7))
                    b1lo = max(512, q0)  # bank 1: q cols [b1lo, 1024)
                    nc.tensor.matmul(
                        ps_att[0:65, b1lo:TOK],
                        V_sb[:, k, h, :],
                        weiT[:, b1lo - q0:TOK - q0],
                        start=(k == 0), stop=(k == KB - 1))
                # normalize + transpose back into attnT
                for j in range(NB):
                    sb_at = stC.tile([128, 128], F32, tag="sb_at")
                    nc.vector.tensor_copy(sb_at[0:65, :], ps_att[0:65, j * 128:(j + 1) * 128])
                    pt1 = psCt.tile([128, 128], F32, tag="ptn")
                    nc.tensor.transpose(pt1[:], sb_at[:], ident[:])
                    recip = stC.tile([128, 1], F32, tag="recip")
                    nc.vector.reciprocal(recip[:], pt1[:, 64:65])
                    attn_j = stC.tile([128, 64], F32, tag="attn_j")
                    nc.vector.tensor_scalar_mul(attn_j[:], pt1[:, 0:64], recip[:])
                    pt2 = psCt.tile([128, 128], F32, tag="ptn")
                    nc.tensor.transpose(pt2[0:64, :], attn_j[:], ident[:])
                    att_st = stC.tile([64, 128], F32R, tag="att_st")
                    nc.vector.tensor_copy(att_st[:], pt2[0:64, :])
                    nc.sync.dma_start(
                        attnT_d.ap()[pair * 128 + off:pair * 128 + off + 64,
                                     j * 128:(j + 1) * 128], att_st[:])
        es_qkv.close()   # free V, KT, QT
        attnTv = attnT_d.ap().rearrange("(o p) t -> o p t", p=128)

        # ============ Stage D: Wp proj + residual + LN2 ============
        es_x2 = ExitStack()
        x2p = es_x2.enter_context(tc.tile_pool(name="x2h2", bufs=1))
        h2T = x2p.tile([128, CCH, TOK], F32R)
        with tc.tile_pool(name="stD", bufs=2) as stD, \
             tc.tile_pool(name="stD_c", bufs=1) as stDc, \
             tc.tile_pool(name="stD_ps", bufs=2, space="PSUM") as psD, \
             tc.tile_pool(name="stD_t_ps", bufs=2, space="PSUM") as psDt:
            bpb = stDc.tile([128, C], F32)
            nc.sync.dma_start(bpb[:], bp_d.ap().to_broadcast([128, C]))
            for nt in range(TOK // 512):
                pT_sb = stD.tile([128, CCH, 512], F32, tag="pT_sb")
                at_nt = stD.tile([128, CCH, 512], F32R, tag="at_nt")
                nc.sync.dma_start(at_nt[:],
                                  attnTv.transpose([1, 0, 2])[:, :, nt * 512:(nt + 1) * 512])
                for co in range(CCH):
                    pp = psD.tile([128, 512], F32, tag="pp")
                    wp_c = stD.tile([128, CCH, 128], F32R, tag="wp_c")
                    nc.sync.dma_start(wp_c[:], Wpv.transpose([1, 0, 2])[:, :, co * 128:(co + 1) * 128])
                    for cc in range(CCH):
                        nc.tensor.matmul(pp[:], wp_c[:, cc],
                                         at_nt[:, cc],
                                         start=(cc == 0), stop=(cc == CCH - 1))
                    nc.scalar.copy(pT_sb[:, co], pp[:])
                for sub in range(4):
                    tb = nt * 4 + sub
                    x2_t = stD.tile([128, C], F32, tag="x2_t")
                    xo_t = stD.tile([128, C], F32, tag="xo_t")
                    nc.sync.dma_start(xo_t[:], xo[tb * 128:(tb + 1) * 128, :])
                    for co in range(CCH):
                        ptd = psDt.tile([128, 128], F32, tag="ptd")
                        nc.tensor.transpose(ptd[:], pT_sb[:, co, sub * 128:(sub + 1) * 128],
                                            ident[:])
                        nc.vector.tensor_tensor(x2_t[:, co * 128:(co + 1) * 128], ptd[:],
                                                xo_t[:, co * 128:(co + 1) * 128], ALU.add)
                    nc.vector.tensor_tensor(x2_t[:], x2_t[:], bpb[:], ALU.add)
                    nc.sync.dma_start(x2_d.ap()[tb * 128:(tb + 1) * 128, :], x2_t[:])
                    # LN2
                    mean, rstd = ln_stats(nc, stD, x2_t[:])
                    h2row = stD.tile([128, C], F32, tag="h2row")
                    ln_apply(nc, stD, h2row[:], x2_t[:], mean, rstd)
                    for cc in range(CCH):
                        pt = psDt.tile([128, 128], F32, tag="ptd2")
                        nc.tensor.transpose(pt[:], h2row[:, cc * 128:(cc + 1) * 128], ident[:])
                        eng = nc.scalar.copy if cc % 2 == 0 else nc.vector.tensor_copy
                        eng(h2T[:, cc, tb * 128:(tb + 1) * 128], pt[:])
        es_mask.close()  # free mask constants

        # ============ Stage E: MLP split by ff-halves (W1/W2 streamed once) ====
        # ff2_sb accumulates the two ff-half partial products in SBUF.
        es_ff2 = ExitStack()
        ff2p = es_ff2.enter_context(tc.tile_pool(name="ff2sb", bufs=1))
        ff2_sb = ff2p.tile([128, CCH, TOK], F32)
        with tc.tile_pool(name="stF_c", bufs=1) as stFc:
            b1p = stFc.tile([128, FCH], F32)
            nc.sync.dma_start(b1p[:], b1_d.ap().rearrange("x (o p) -> p (x o)", p=128))
            b2b = stFc.tile([128, C], F32)
            nc.sync.dma_start(b2b[:], b2_d.ap().to_broadcast([128, C]))
            FH = FCH // 2  # 16 ff chunks per half
            for fh in range(2):
                es_half = ExitStack()
                ffp = es_half.enter_context(tc.tile_pool(name="ff1T", bufs=1))
                ff1T = ffp.tile([128, FH, TOK], F32R)
                with tc.tile_pool(name="stE1", bufs=2) as stE1, \
                     tc.tile_pool(name="stE1_ps", bufs=2, space="PSUM") as psE1:
                    for fog in range(4):
                        w1g = stE1.tile([128, CCH, 512], F32R, tag="w1g")
                        nc.sync.dma_start(
                            w1g[:], W1v.transpose([1, 0, 2])
                            [:, :, fh * 2048 + fog * 512:fh * 2048 + (fog + 1) * 512])
                        for f4 in range(4):
                            fo = fog * 4 + f4          # local ff chunk in this half
                            for nt in range(TOK // 512):
                                pf = psE1.tile([128, 512], F32, tag="pf")
                                for cc in range(CCH):
                                    nc.tensor.matmul(
                                        pf[:], w1g[:, cc, f4 * 128:(f4 + 1) * 128],
                                        h2T[:, cc, nt * 512:(nt + 1) * 512],
                                        start=(cc == 0), stop=(cc == CCH - 1))
                                nc.scalar.activation(
                                    ff1T[:, fo, nt * 512:(nt + 1) * 512], pf[:], AF.Relu,
                                    bias=b1p[:, fh * FH + fo:fh * FH + fo + 1])
                with tc.tile_pool(name="stE2", bufs=2) as stE2, \
                     tc.tile_pool(name="stE2_ps", bufs=2, space="PSUM") as psE2:
                    for co in range(CCH):
                        w2c = stE2.tile([128, FH, 128], F32R, tag="w2c")
                        nc.sync.dma_start(
                            w2c[:], W2v[:, fh * FH:(fh + 1) * FH, co * 128:(co + 1) * 128])
                        for nt in range(TOK // 512):
                            p2 = psE2.tile([128, 512], F32, tag="p2")
                            for fo in range(FH):
                                nc.tensor.matmul(p2[:], w2c[:, fo],
                                                 ff1T[:, fo, nt * 512:(nt + 1) * 512],
                                                 start=(fo == 0), stop=(fo == FH - 1))
                            dst = ff2_sb[:, co, nt * 512:(nt + 1) * 512]
                            if fh == 0:
                                nc.scalar.copy(dst, p2[:])
                            else:
                                nc.vector.tensor_tensor(dst, dst, p2[:], ALU.add)
                es_half.close()
            # ============ Stage F: transpose + residual + output ============
            with tc.tile_pool(name="stF", bufs=2) as stF, \
                 tc.tile_pool(name="stF_ps", bufs=2, space="PSUM") as psF:
                for tb in range(NB):
                    out_t = stF.tile([128, C], F32, tag="out_t")
                    x2_t = stF.tile([128, C], F32, tag="x2r_t")
                    nc.sync.dma_start(x2_t[:], x2_d.ap()[tb * 128:(tb + 1) * 128, :])
                    sub = tb % 4
                    for co in range(CCH):
                        ptf = psF.tile([128, 128], F32, tag="ptf")
                        nc.tensor.transpose(
                            ptf[:], ff2_sb[:, co, tb * 128:(tb + 1) * 128], ident[:])
                        nc.vector.tensor_tensor(out_t[:, co * 128:(co + 1) * 128], ptf[:],
                                                x2_t[:, co * 128:(co + 1) * 128], ALU.add)
                    nc.vector.tensor_tensor(out_t[:], out_t[:], b2b[:], ALU.add)
                    nc.sync.dma_start(out_d.ap()[tb * 128:(tb + 1) * 128, :], out_t[:])
        es_ff2.close()
        es_x2.close()


def make_nc():
    nc = bacc.Bacc("TRN2", target_bir_lowering=False, debug=False,
                   num_devices=N_CORES)
    build(nc)
    nc.compile()
    return nc


def shard_inputs(inputs):
    """Full inputs dict -> list of 8 per-core in_maps.

    Folds LN1 gain/bias into Wq/Wk/Wv (weights scaled by g1 per input channel,
    be1 contribution becomes an additive bias on q/k/v) and LN2's into W1/b1.
    """
    x = np.asarray(inputs["x"], np.float32)
    assert x.shape == (B, T, C)
    f64 = np.float64
    Wq = np.asarray(inputs["Wq"], f64); Wk = np.asarray(inputs["Wk"], f64)
    Wv = np.asarray(inputs["Wv"], f64); Wp = np.asarray(inputs["Wp"], np.float32)
    W1 = np.asarray(inputs["W1"], f64); W2 = np.asarray(inputs["W2"], np.float32)
    g1 = np.asarray(inputs["g1"], f64); be1 = np.asarray(inputs["be1"], f64)
    g2 = np.asarray(inputs["g2"], f64); be2 = np.asarray(inputs["be2"], f64)
    b1 = np.asarray(inputs["b1"], f64)
    shared = {
        "Wq": (g1[:, None] * Wq).astype(np.float32),
        "Wk": (g1[:, None] * Wk).astype(np.float32),
        "Wv": (g1[:, None] * Wv).astype(np.float32),
        "Wp": Wp, "W2": W2,
        "W1": (g2[:, None] * W1).astype(np.float32),
        "qbias": (be1 @ Wq).astype(np.float32).reshape(NB, 128),
        "kbias": (be1 @ Wk).astype(np.float32).reshape(NB, 128),
        "vbias": (be1 @ Wv).astype(np.float32).reshape(1, C),
        "b1": (b1 + be2 @ W1).astype(np.float32).reshape(1, FF),
        "bp": np.asarray(inputs["bp"], np.float32).reshape(1, C),
        "b2": np.asarray(inputs["b2"], np.float32).reshape(1, C),
    }
    in_maps = []
    for c in range(N_CORES):
        b, par = c // 2, c % 2
        gblocks = [2 * j + par for j in range(NB)]
        rows = np.concatenate([x[b, g * 128:(g + 1) * 128, :] for g in gblocks], 0)
        qpos = np.stack([np.arange(g * 128, (g + 1) * 128, dtype=np.float32)
                         for g in gblocks], 0)
        m = {"xfull": np.ascontiguousarray(x[b]),
             "xown": np.ascontiguousarray(rows), "qpos": qpos}
        m.update(shared)
        in_maps.append(m)
    return in_maps


def unshard_outputs(results):
    """list of per-core {'out': [TOK, C]} -> [B, T, C]"""
    out = np.zeros((B, T, C), np.float32)
    for c in range(N_CORES):
        b, par = c // 2, c % 2
        r = np.asarray(results[c]["out"])
        for j in range(NB):
            g = 2 * j + par
            out[b, g * 128:(g + 1) * 128, :] = r[j * 128:(j + 1) * 128, :]
    return out


_NC_CACHE = {}

def _get_nc():
    if "nc" not in _NC_CACHE:
        nc = bacc.Bacc("TRN2", target_bir_lowering=False, debug=False,
                       num_devices=N_CORES)
        build(nc, reps=1)
        nc.compile()
        _NC_CACHE["nc"] = nc
    return _NC_CACHE["nc"]


def kernel(**inputs):
    from concourse.bass_utils import run_bass_kernel_spmd
    nc = _get_nc()
    in_maps = shard_inputs(inputs)
    res = run_bass_kernel_spmd(nc, in_maps, core_ids=list(range(N_CORES)))
    return unshard_outputs(res.results)



# revision 4
# speedup vs baseline: 32.8001x; 32.8001x over previous
"""Self-contained Trainium2 kernel for the dense transformer block problem.

kernel(**inputs) takes the FULL inputs (as produced by the reference
setup_inputs), shards them across 8 NeuronCores (2 cores per batch element,
causal-balanced parity split of query blocks), runs a Bass/Tile SPMD kernel,
and reassembles the full [B, T, C] output.
"""
"""Transformer block (pre-LN attention + MLP) for trn2, 8-core SPMD.

Sharding: 2 cores per batch element (B=4). Within a pair, query blocks of 128
tokens are split by parity (core parity p owns global blocks {2j+p}), which
balances causal attention work. Each core computes K/V for the full sequence
of its batch element (redundant within the pair) so there are no collectives.

Per-core layouts:
  activations for matmuls flow transposed: [C_chunk x 128 partitions, tokens], f32r
  attention: scoresT [keys, q] (f32r matmul) -> +mask bias -> exp (ACT) -> weiT bf16
  V is bf16, augmented with a ones column; AV accumulates [attn^T ; rowsum] in PSUM
  softmax normalization via PE transpose + per-partition reciprocal
"""
import sys
sys.path.insert(0, '/opt/trn_rl_repo')
import numpy as np
from contextlib import ExitStack

import concourse.bacc as bacc
import concourse.tile as tile
import concourse.mybir as mybir
from concourse.masks import make_identity

F32 = mybir.dt.float32
F32R = mybir.dt.float32r
BF16 = mybir.dt.bfloat16
AF = mybir.ActivationFunctionType
ALU = mybir.AluOpType

B, T, C, H, DH = 4, 2048, 1024, 16, 64
N_CORES = 8
TOK = 1024          # own tokens per core
NB = TOK // 128     # 8 own query blocks
KB = T // 128       # 16 key blocks
CCH = C // 128      # 8 channel chunks
FF = 4 * C          # 4096
FCH = FF // 128     # 32 ff chunks
EPS = 1e-5
NEG = -1e30

IN_NAMES = ["xfull", "xown", "qpos", "Wq", "Wk", "Wv", "Wp", "bp",
            "W1", "b1", "W2", "b2", "qbias", "kbias", "vbias"]


def _score_chunks(nq):
    """split nq (multiple of 128) into pieces, avoiding <256 pieces when possible"""
    out = []
    rem = nq
    while rem > 0:
        if rem == 640:
            take = 384
        elif rem >= 512:
            take = 512
        else:
            take = rem
        out.append(take)
        rem -= take
    return out


def build(nc, reps=1, loop_reps=None):
    """Trace the SPMD program into nc (a bacc.Bacc). Call nc.compile() after.

    Weight inputs arrive pre-folded on the host:
      Wq/Wk/Wv = diag(g1) @ W (dtype f32r);  qbias/kbias/vbias = be1 @ W
      W1 = diag(g2) @ W1 (f32r);  b1 = b1 + be2 @ W1
      Wp, W2 plain f32r.  g/be tensors are consumed host-side only.
    """
    def din(name, shape, dt=F32):
        return nc.dram_tensor(name, shape, dt, kind="ExternalInput")

    xfull_d = din("xfull", [T, C])
    xown_d = din("xown", [TOK, C])
    qpos_d = din("qpos", [NB, 128])
    Wq_d = din("Wq", [C, C], F32R); Wk_d = din("Wk", [C, C], F32R)
    Wv_d = din("Wv", [C, C], F32R); Wp_d = din("Wp", [C, C], F32R)
    bp_d = din("bp", [1, C]); W1_d = din("W1", [C, FF], F32R); b1_d = din("b1", [1, FF])
    W2_d = din("W2", [FF, C], F32R); b2_d = din("b2", [1, C])
    qb_d = din("qbias", [NB, 128])   # be1 @ Wq, laid out [pair, dh-stacked 128]
    kb_d = din("kbias", [NB, 128])   # be1 @ Wk
    vb_d = din("vbias", [1, C])      # be1 @ Wv
    out_d = nc.dram_tensor("out", [TOK, C], F32, kind="ExternalOutput")
    x2_d = nc.dram_tensor("x2_scratch", [TOK, C], F32)  # internal DRAM scratch
    attnT_d = nc.dram_tensor("attnT_scratch", [C, TOK], F32R)  # [dh-stacked C, own tokens]

    Wqv = Wq_d.ap().rearrange("(o p) m -> o p m", p=128)
    Wkv = Wk_d.ap().rearrange("(o p) m -> o p m", p=128)
    Wvv = Wv_d.ap().rearrange("(o p) m -> o p m", p=128)
    Wpv = Wp_d.ap().rearrange("(o p) m -> o p m", p=128)
    W1v = W1_d.ap().rearrange("(o p) m -> o p m", p=128)
    W2v = W2_d.ap().rearrange("(o p) m -> p o m", p=128)  # [128, 32, 1024]
    xf = xfull_d.ap()
    xo = xown_d.ap()

    env = locals()
    if loop_reps is not None:
        import concourse.tile as tile
        from contextlib import ExitStack
        with tile.TileContext(nc) as tc:
            with tc.For_i(0, loop_reps, 1):
                with ExitStack() as top:
                    _build_body(nc, tc, top, env)
    else:
        for _rep in range(reps):
            _build_one(nc, env)
    return IN_NAMES


def _build_one(nc, env):
    import concourse.tile as tile
    from contextlib import ExitStack
    with tile.TileContext(nc) as tc, ExitStack() as top:
        _build_body(nc, tc, top, env)


def _build_body(nc, tc, top, env):
    (xfull_d, xown_d, qpos_d, Wq_d, Wk_d, Wv_d, Wp_d, bp_d, W1_d, b1_d, W2_d,
     b2_d, qb_d, kb_d, vb_d, out_d, x2_d, attnT_d, Wqv, Wkv, Wvv, Wpv, W1v, W2v,
     xf, xo) = (
        env[k] for k in ["xfull_d", "xown_d", "qpos_d", "Wq_d", "Wk_d", "Wv_d",
                         "Wp_d", "bp_d", "W1_d", "b1_d", "W2_d", "b2_d", "qb_d",
                         "kb_d", "vb_d", "out_d", "x2_d", "attnT_d", "Wqv", "Wkv",
                         "Wvv", "Wpv", "W1v", "W2v", "xf", "xo"])
    from contextlib import ExitStack
    if True:
        const = top.enter_context(tc.tile_pool(name="const", bufs=1))
        ident = const.tile([128, 128], F32)
        make_identity(nc, ident[:])
        eps_t = const.tile([128, 1], F32)
        nc.vector.memset(eps_t[:], EPS)

        def ln_stats(nc, pool, x_ap):
            n = x_ap.shape[-1] // 512
            xg = x_ap.rearrange("p (n f) -> p n f", f=512)
            stats = pool.tile([128, n, 6], F32, tag="ln_stats")
            mv = pool.tile([128, 2], F32, tag="ln_mv")
            for i in range(n):
                nc.vector.bn_stats(stats[:, i], xg[:, i])
            nc.vector.bn_aggr(mv[:], stats[:])
            rstd = pool.tile([128, 1], F32, tag="ln_rstd")
            nc.scalar.activation(rstd[:], mv[:, 1:2], AF.Sqrt, bias=eps_t[:])
            nc.vector.reciprocal(rstd[:], rstd[:])
            return mv[:, 0:1], rstd

        def ln_apply(nc, pool, out_ap, x_ap, mean, rstd):
            # out = (x - mu) * rstd on ACT: Identity(x * rstd + (-mu * rstd))
            nmr = pool.tile([128, 1], F32, tag="ln_nmr")
            nc.vector.tensor_scalar(nmr[:], mean, rstd[:], -1.0,
                                    op0=ALU.mult, op1=ALU.mult)
            nc.scalar.activation(out_ap, x_ap, AF.Identity,
                                 bias=nmr[:], scale=rstd[:])

        # ============ Stage A: LN1 over full T -> hT [128, CCH, T] f32r ============
        es_h = ExitStack()
        hp = es_h.enter_context(tc.tile_pool(name="hT", bufs=1, side="right"))
        hT = hp.tile([128, CCH, T], F32R)
        with tc.tile_pool(name="stA", bufs=3) as stA, \
             tc.tile_pool(name="stA_ps", bufs=3, space="PSUM") as psA:
            for tb in range(T // 128):
                x_t = stA.tile([128, C], F32, tag="x_t")
                nc.sync.dma_start(x_t[:], xf[tb * 128:(tb + 1) * 128, :])
                mean, rstd = ln_stats(nc, stA, x_t[:])
                hrow = stA.tile([128, C], F32, tag="hrow")
                ln_apply(nc, stA, hrow[:], x_t[:], mean, rstd)
                for cc in range(CCH):
                    pt = psA.tile([128, 128], F32, tag="psA_t")
                    nc.tensor.transpose(pt[:], hrow[:, cc * 128:(cc + 1) * 128], ident[:])
                    eng = nc.scalar.copy if cc % 2 == 0 else nc.vector.tensor_copy
                    eng(hT[:, cc, tb * 128:(tb + 1) * 128], pt[:])

        # ============ Stage B1: V (token-major, bf16, ones-augmented) ============
        es_qkv = ExitStack()
        vp = es_qkv.enter_context(tc.tile_pool(name="Vp", bufs=1))
        V_sb = vp.tile([128, KB, H, 65], F32R)
        ones_f = vp.tile([128, 1], F32)
        nc.vector.memset(ones_f[:], 1.0)
        ones_r = vp.tile([128, 1], F32R)
        nc.vector.tensor_copy(ones_r[:], ones_f[:])
        nc.vector.tensor_copy(V_sb[:, :, :, 64:65],
                              ones_r[:, 0:1, None, None].to_broadcast([128, KB, H, 1]))
        with tc.tile_pool(name="stB1a", bufs=2) as stB1a, \
             tc.tile_pool(name="stB1c", bufs=1) as stB1c, \
             tc.tile_pool(name="stB1_ps", bufs=2, space="PSUM") as psB1:
            vb_b = stB1c.tile([128, C], F32)
            nc.sync.dma_start(vb_b[:], vb_d.ap().to_broadcast([128, C]))
            for grp in range(2):
                wv_g = stB1a.tile([128, CCH, 512], F32R, tag="wv_g")
                nc.sync.dma_start(wv_g[:], Wvv.transpose([1, 0, 2])[:, :, grp * 512:(grp + 1) * 512])
                for tb in range(KB):
                    pv = psB1.tile([128, 512], F32, tag="pv")
                    for cc in range(CCH):
                        nc.tensor.matmul(pv[:], hT[:, cc, tb * 128:(tb + 1) * 128],
                                         wv_g[:, cc], start=(cc == 0), stop=(cc == CCH - 1))
                    nc.vector.tensor_tensor(
                        V_sb[:, tb, grp * 8:(grp + 1) * 8, 0:64],
                        pv[:].rearrange("p (h d) -> p h d", d=64),
                        vb_b[:, grp * 512:(grp + 1) * 512].rearrange("p (h d) -> p h d", d=64),
                        ALU.add)

        # ============ Stage B2: KT [128(dh pair-stacked), pair, T] f32r ============
        ktp = es_qkv.enter_context(tc.tile_pool(name="KTp", bufs=1))
        KT = ktp.tile([128, CCH, T], F32R)
        with tc.tile_pool(name="stB2", bufs=2) as stB2, \
             tc.tile_pool(name="stB2c", bufs=1) as stB2c, \
             tc.tile_pool(name="stB2_ps", bufs=3, space="PSUM") as psB2:
            kb_sb = stB2c.tile([128, NB], F32)
            nc.sync.dma_start(kb_sb[:], kb_d.ap().rearrange("o p -> p o"))
            for pair in range(CCH):
                wk_p = stB2.tile([128, CCH, 128], F32R, tag="wk_p")
                nc.sync.dma_start(wk_p[:], Wkv.transpose([1, 0, 2])[:, :, pair * 128:(pair + 1) * 128])
                for nt in range(T // 512):
                    pk = psB2.tile([128, 512], F32, tag="pk")
                    for cc in range(CCH):
                        nc.tensor.matmul(pk[:], wk_p[:, cc],
                                         hT[:, cc, nt * 512:(nt + 1) * 512],
                                         start=(cc == 0), stop=(cc == CCH - 1))
                    nc.vector.tensor_scalar(KT[:, pair, nt * 512:(nt + 1) * 512], pk[:],
                                            kb_sb[:, pair:pair + 1], None, op0=ALU.add)

        # ============ Stage A': LN1 of own rows -> hTown; then B3: QT ============
        es_h.close()  # free hT
        es_ho = ExitStack()
        hop = es_ho.enter_context(tc.tile_pool(name="hTown", bufs=1, side="right"))
        hTown = hop.tile([128, CCH, TOK], F32R)
        with tc.tile_pool(name="stA2", bufs=3) as stA2, \
             tc.tile_pool(name="stA2_ps", bufs=3, space="PSUM") as psA2:
            for tb in range(NB):
                x_t = stA2.tile([128, C], F32, tag="x_t2")
                nc.sync.dma_start(x_t[:], xo[tb * 128:(tb + 1) * 128, :])
                mean, rstd = ln_stats(nc, stA2, x_t[:])
                hrow = stA2.tile([128, C], F32, tag="hrow2")
                ln_apply(nc, stA2, hrow[:], x_t[:], mean, rstd)
                for cc in range(CCH):
                    pt = psA2.tile([128, 128], F32, tag="psA2_t")
                    nc.tensor.transpose(pt[:], hrow[:, cc * 128:(cc + 1) * 128], ident[:])
                    eng = nc.scalar.copy if cc % 2 == 0 else nc.vector.tensor_copy
                    eng(hTown[:, cc, tb * 128:(tb + 1) * 128], pt[:])

        qtp = es_qkv.enter_context(tc.tile_pool(name="QTp", bufs=1))
        QT = qtp.tile([128, CCH, TOK], F32R)
        with tc.tile_pool(name="stB3", bufs=2) as stB3, \
             tc.tile_pool(name="stB3c", bufs=1) as stB3c, \
             tc.tile_pool(name="stB3_ps", bufs=3, space="PSUM") as psB3:
            qb_sb = stB3c.tile([128, NB], F32)
            nc.sync.dma_start(qb_sb[:], qb_d.ap().rearrange("o p -> p o"))
            for pair in range(CCH):
                wq_p = stB3.tile([128, CCH, 128], F32R, tag="wq_p")
                nc.sync.dma_start(wq_p[:], Wqv.transpose([1, 0, 2])[:, :, pair * 128:(pair + 1) * 128])
                for nt in range(TOK // 512):
                    pq = psB3.tile([128, 512], F32, tag="pq")
                    for cc in range(CCH):
                        nc.tensor.matmul(pq[:], wq_p[:, cc],
                                         hTown[:, cc, nt * 512:(nt + 1) * 512],
                                         start=(cc == 0), stop=(cc == CCH - 1))
                    nc.vector.tensor_scalar(QT[:, pair, nt * 512:(nt + 1) * 512], pq[:],
                                            qb_sb[:, pair:pair + 1], None, op0=ALU.add)
        es_ho.close()  # free hTown

        # ---------- mask constants (scoped to attention) ----------
        es_mask = ExitStack()
        maskp = es_mask.enter_context(tc.tile_pool(name="maskp", bufs=1, side="right"))
        kp_i = maskp.tile([128, KB], mybir.dt.int32)
        nc.gpsimd.iota(kp_i[:], pattern=[[128, KB]], base=0, channel_multiplier=1)
        kp_f = maskp.tile([128, KB], F32)
        nc.vector.tensor_copy(kp_f[:], kp_i[:])
        qb = maskp.tile([128, NB, 128], F32)
        for j in range(NB):
            nc.sync.dma_start(qb[:, j], qpos_d.ap()[j:j + 1, :].to_broadcast([128, 128]))
        biasm = maskp.tile([128, NB, 2, 128], F32)
        for j in range(NB):
            for t in range(2):
                # m01[p_key, f_q] = (qpos_j[f] >= keypos(k=2j+t)[p])
                nc.vector.tensor_scalar(
                    biasm[:, j, t], qb[:, j], kp_f[:, 2 * j + t:2 * j + t + 1], None,
                    op0=ALU.is_ge)

        # ============ Stage C: attention ============
        with tc.tile_pool(name="stC", bufs=3) as stC, \
             tc.tile_pool(name="stC_att_ps", bufs=2, space="PSUM") as psCa, \
             tc.tile_pool(name="stC_s_ps", bufs=2, space="PSUM") as psCs, \
             tc.tile_pool(name="stC_t_ps", bufs=2, space="PSUM") as psCt:
            for h in range(H):
                pair, off = h // 2, 64 * (h % 2)
                ps_att = psCa.tile([128, TOK], F32, tag="ps_att")
                for k in range(KB):
                    jmin = k // 2
                    q0 = jmin * 128
                    nq = TOK - q0
                    weiT = stC.tile([128, TOK], F32R, tag="weiT")
                    qa = 0
                    while qa < nq:  # one 1-bank psum tile + one exp per 512 cols
                        qn = min(512, nq - qa)
                        ps_s = psCs.tile([128, 512], F32, tag="ps_s")
                        nc.tensor.matmul(
                            ps_s[:, 0:qn],
                            KT[off:off + 64, pair, k * 128:(k + 1) * 128],
                            QT[off:off + 64, pair, q0 + qa:q0 + qa + qn],
                            start=True, stop=True)
                        nc.scalar.activation(weiT[:, qa:qa + qn], ps_s[:, 0:qn],
                                             AF.Exp, scale=0.125)
                        qa += qn
                    nc.vector.tensor_tensor(weiT[:, 0:128], weiT[:, 0:128],
                                            biasm[:, jmin, k - 2 * jmin], ALU.mult)
                    # AV: one matmul per 512-col PSUM bank (start=True must
                    # clear a whole bank, so groups are bank-aligned)
                    if k <= 7:  # bank 0: q cols [q0, 512)
                        nc.tensor.matmul(
                            ps_att[0:65, q0:512],
                            V_sb[:, k, h, :],
                            weiT[:, 0:512 - q0],
                            start=(k == 0), stop=(k == 7))
                    b1lo = max(512, q0)  # bank 1: q cols [b1lo, 1024)
                    nc.tensor.matmul(
                        ps_att[0:65, b1lo:TOK],
                        V_sb[:, k, h, :],
                        weiT[:, b1lo - q0:TOK - q0],
                        start=(k == 0), stop=(k == KB - 1))
                # normalize + transpose back into attnT
                for j in range(NB):
                    sb_at = stC.tile([128, 128], F32, tag="sb_at")
                    nc.vector.tensor_copy(sb_at[0:65, :], ps_att[0:65, j * 128:(j + 1) * 128])
                    pt1 = psCt.tile([128, 128], F32, tag="ptn")
                    nc.tensor.transpose(pt1[:], sb_at[:], ident[:])
                    recip = stC.tile([128, 1], F32, tag="recip")
                    nc.vector.reciprocal(recip[:], pt1[:, 64:65])
                    attn_j = stC.tile([128, 64], F32, tag="attn_j")
                    nc.vector.tensor_scalar_mul(attn_j[:], pt1[:, 0:64], recip[:])
                    pt2 = psCt.tile([128, 128], F32, tag="ptn")
                    nc.tensor.transpose(pt2[0:64, :], attn_j[:], ident[:])
                    att_st = stC.tile([64, 128], F32R, tag="att_st")
                    nc.vector.tensor_copy(att_st[:], pt2[0:64, :])
                    nc.sync.dma_start(
                        attnT_d.ap()[pair * 128 + off:pair * 128 + off + 64,
                                     j * 128:(j + 1) * 128], att_st[:])
        es_qkv.close()   # free V, KT, QT
        attnTv = attnT_d.ap().rearrange("(o p) t -> o p t", p=128)

        # ============ Stage D: Wp proj + residual + LN2 ============
        es_x2 = ExitStack()
        x2p = es_x2.enter_context(tc.tile_pool(name="x2h2", bufs=1))
        h2T = x2p.tile([128, CCH, TOK], F32R)
        with tc.tile_pool(name="stD", bufs=2) as stD, \
             tc.tile_pool(name="stD_c", bufs=1) as stDc, \
             tc.tile_pool(name="stD_ps", bufs=2, space="PSUM") as psD, \
             tc.tile_pool(name="stD_t_ps", bufs=2, space="PSUM") as psDt:
            bpb = stDc.tile([128, C], F32)
            nc.sync.dma_start(bpb[:], bp_d.ap().to_broadcast([128, C]))
            for nt in range(TOK // 512):
                pT_sb = stD.tile([128, CCH, 512], F32, tag="pT_sb")
                at_nt = stD.tile([128, CCH, 512], F32R, tag="at_nt")
                nc.sync.dma_start(at_nt[:],
                                  attnTv.transpose([1, 0, 2])[:, :, nt * 512:(nt + 1) * 512])
                for co in range(CCH):
                    pp = psD.tile([128, 512], F32, tag="pp")
                    wp_c = stD.tile([128, CCH, 128], F32R, tag="wp_c")
                    nc.sync.dma_start(wp_c[:], Wpv.transpose([1, 0, 2])[:, :, co * 128:(co + 1) * 128])
                    for cc in range(CCH):
                        nc.tensor.matmul(pp[:], wp_c[:, cc],
                                         at_nt[:, cc],
                                         start=(cc == 0), stop=(cc == CCH - 1))
                    nc.scalar.copy(pT_sb[:, co], pp[:])
                for sub in range(4):
                    tb = nt * 4 + sub
                    x2_t = stD.tile([128, C], F32, tag="x2_t")
                    xo_t = stD.tile([128, C], F32, tag="xo_t")
                    nc.sync.dma_start(xo_t[:], xo[tb * 128:(tb + 1) * 128, :])
                    for co in range(CCH):
                        ptd = psDt.tile([128, 128], F32, tag="ptd")
                        nc.tensor.transpose(ptd[:], pT_sb[:, co, sub * 128:(sub + 1) * 128],
                                            ident[:])
                        nc.vector.tensor_tensor(x2_t[:, co * 128:(co + 1) * 128], ptd[:],
                                                xo_t[:, co * 128:(co + 1) * 128], ALU.add)
                    nc.vector.tensor_tensor(x2_t[:], x2_t[:], bpb[:], ALU.add)
                    nc.sync.dma_start(x2_d.ap()[tb * 128:(tb + 1) * 128, :], x2_t[:])
                    # LN2
                    mean, rstd = ln_stats(nc, stD, x2_t[:])
                    h2row = stD.tile([128, C], F32, tag="h2row")
                    ln_apply(nc, stD, h2row[:], x2_t[:], mean, rstd)
                    for cc in range(CCH):
                        pt = psDt.tile([128, 128], F32, tag="ptd2")
                        nc.tensor.transpose(pt[:], h2row[:, cc * 128:(cc + 1) * 128], ident[:])
                        eng = nc.scalar.copy if cc % 2 == 0 else nc.vector.tensor_copy
                        eng(h2T[:, cc, tb * 128:(tb + 1) * 128], pt[:])
        es_mask.close()  # free mask constants

        # ============ Stage E: MLP split by ff-halves (W1/W2 streamed once) ====
        # ff2_sb accumulates the two ff-half partial products in SBUF.
        es_ff2 = ExitStack()
        ff2p = es_ff2.enter_context(tc.tile_pool(name="ff2sb", bufs=1))
        ff2_sb = ff2p.tile([128, CCH, TOK], F32)
        with tc.tile_pool(name="stF_c", bufs=1) as stFc:
            b1p = stFc.tile([128, FCH], F32)
            nc.sync.dma_start(b1p[:], b1_d.ap().rearrange("x (o p) -> p (x o)", p=128))
            b2b = stFc.tile([128, C], F32)
            nc.sync.dma_start(b2b[:], b2_d.ap().to_broadcast([128, C]))
            FH = FCH // 2  # 16 ff chunks per half
            for fh in range(2):
                es_half = ExitStack()
                ffp = es_half.enter_context(tc.tile_pool(name="ff1T", bufs=1))
                ff1T = ffp.tile([128, FH, TOK], F32R)
                with tc.tile_pool(name="stE1", bufs=2) as stE1, \
                     tc.tile_pool(name="stE1_ps", bufs=2, space="PSUM") as psE1:
                    for fog in range(4):
                        w1g = stE1.tile([128, CCH, 512], F32R, tag="w1g")
                        nc.sync.dma_start(
                            w1g[:], W1v.transpose([1, 0, 2])
                            [:, :, fh * 2048 + fog * 512:fh * 2048 + (fog + 1) * 512])
                        for f4 in range(4):
                            fo = fog * 4 + f4          # local ff chunk in this half
                            for nt in range(TOK // 512):
                                pf = psE1.tile([128, 512], F32, tag="pf")
                                for cc in range(CCH):
                                    nc.tensor.matmul(
                                        pf[:], w1g[:, cc, f4 * 128:(f4 + 1) * 128],
                                        h2T[:, cc, nt * 512:(nt + 1) * 512],
                                        start=(cc == 0), stop=(cc == CCH - 1))
                                nc.scalar.activation(
                                    ff1T[:, fo, nt * 512:(nt + 1) * 512], pf[:], AF.Relu,
                                    bias=b1p[:, fh * FH + fo:fh * FH + fo + 1])
                with tc.tile_pool(name="stE2", bufs=2) as stE2, \
                     tc.tile_pool(name="stE2_ps", bufs=2, space="PSUM") as psE2:
                    for co in range(CCH):
                        w2c = stE2.tile([128, FH, 128], F32R, tag="w2c")
                        nc.sync.dma_start(
                            w2c[:], W2v[:, fh * FH:(fh + 1) * FH, co * 128:(co + 1) * 128])
                        for nt in range(TOK // 512):
                            p2 = psE2.tile([128, 512], F32, tag="p2")
                            for fo in range(FH):
                                nc.tensor.matmul(p2[:], w2c[:, fo],
                                                 ff1T[:, fo, nt * 512:(nt + 1) * 512],
                                                 start=(fo == 0), stop=(fo == FH - 1))
                            dst = ff2_sb[:, co, nt * 512:(nt + 1) * 512]
                            if fh == 0:
                                nc.scalar.copy(dst, p2[:])
                            else:
                                nc.vector.tensor_tensor(dst, dst, p2[:], ALU.add)
                es_half.close()
            # ============ Stage F: transpose + residual + output ============
            with tc.tile_pool(name="stF", bufs=2) as stF, \
                 tc.tile_pool(name="stF_ps", bufs=2, space="PSUM") as psF:
                for tb in range(NB):
                    out_t = stF.tile([128, C], F32, tag="out_t")
                    x2_t = stF.tile([128, C], F32, tag="x2r_t")
                    nc.sync.dma_start(x2_t[:], x2_d.ap()[tb * 128:(tb + 1) * 128, :])
                    sub = tb % 4
                    for co in range(CCH):
                        ptf = psF.tile([128, 128], F32, tag="ptf")
                        nc.tensor.transpose(
                            ptf[:], ff2_sb[:, co, tb * 128:(tb + 1) * 128], ident[:])
                        nc.vector.tensor_tensor(out_t[:, co * 128:(co + 1) * 128], ptf[:],
                                                x2_t[:, co * 128:(co + 1) * 128], ALU.add)
                    nc.vector.tensor_tensor(out_t[:], out_t[:], b2b[:], ALU.add)
                    nc.sync.dma_start(out_d.ap()[tb * 128:(tb + 1) * 128, :], out_t[:])
        es_ff2.close()
        es_x2.close()


def make_nc():
    nc = bacc.Bacc("TRN2", target_bir_lowering=False, debug=False,
                   num_devices=N_CORES)
    build(nc)
    nc.compile()
    return nc


def shard_inputs(inputs):
    """Full inputs dict -> list of 8 per-core in_maps.

    Folds LN1 gain/bias into Wq/Wk/Wv (weights scaled by g1 per input channel,
    be1 contribution becomes an additive bias on q/k/v) and LN2's into W1/b1.
    """
    x = np.asarray(inputs["x"], np.float32)
    assert x.shape == (B, T, C)
    f64 = np.float64
    Wq = np.asarray(inputs["Wq"], f64); Wk = np.asarray(inputs["Wk"], f64)
    Wv = np.asarray(inputs["Wv"], f64); Wp = np.asarray(inputs["Wp"], np.float32)
    W1 = np.asarray(inputs["W1"], f64); W2 = np.asarray(inputs["W2"], np.float32)
    g1 = np.asarray(inputs["g1"], f64); be1 = np.asarray(inputs["be1"], f64)
    g2 = np.asarray(inputs["g2"], f64); be2 = np.asarray(inputs["be2"], f64)
    b1 = np.asarray(inputs["b1"], f64)
    shared = {
        "Wq": (g1[:, None] * Wq).astype(np.float32),
        "Wk": (g1[:, None] * Wk).astype(np.float32),
        "Wv": (g1[:, None] * Wv).astype(np.float32),
        "Wp": Wp, "W2": W2,
        "W1": (g2[:, None] * W1).astype(np.float32),
        "qbias": (be1 @ Wq).astype(np.float32).reshape(NB, 128),
        "kbias": (be1 @ Wk).astype(np.float32).reshape(NB, 128),
        "vbias": (be1 @ Wv).astype(np.float32).reshape(1, C),
        "b1": (b1 + be2 @ W1).astype(np.float32).reshape(1, FF),
        "bp": np.asarray(inputs["bp"], np.float32).reshape(1, C),
        "b2": np.asarray(inputs["b2"], np.float32).reshape(1, C),
    }
    in_maps = []
    for c in range(N_CORES):
        b, par = c // 2, c % 2
        gblocks = [2 * j + par for j in range(NB)]
        rows = np.concatenate([x[b, g * 128:(g + 1) * 128, :] for g in gblocks], 0)
        qpos = np.stack([np.arange(g * 128, (g + 1) * 128, dtype=np.float32)
                         for g in gblocks], 0)
        m = {"xfull": np.ascontiguousarray(x[b]),
             "xown": np.ascontiguousarray(rows), "qpos": qpos}
        m.update(shared)
        in_maps.append(m)
    return in_maps


def unshard_outputs(results):
    """list of per-core {'out': [TOK, C]} -> [B, T, C]"""
    out = np.zeros((B, T, C), np.float32)
    for c in range(N_CORES):
        b, par = c // 2, c % 2
        r = np.asarray(results[c]["out"])
        for j in range(NB):
            g = 2 * j + par
            out[b, g * 128:(g + 1) * 128, :] = r[j * 128:(j + 1) * 128, :]
    return out


_NC_CACHE = {}

def _get_nc():
    if "nc" not in _NC_CACHE:
        nc = bacc.Bacc("TRN2", target_bir_lowering=False, debug=False,
                       num_devices=N_CORES)
        build(nc, reps=1)
        nc.compile()
        _NC_CACHE["nc"] = nc
    return _NC_CACHE["nc"]


def kernel(**inputs):
    from concourse.bass_utils import run_bass_kernel_spmd
    nc = _get_nc()
    in_maps = shard_inputs(inputs)
    res = run_bass_kernel_spmd(nc, in_maps, core_ids=list(range(N_CORES)))
    return unshard_outputs(res.results)



# revision 34
# speedup vs baseline: 79.5088x; 2.4240x over previous
"""Self-contained Trainium2 kernel for the dense transformer block problem.

kernel(**inputs) takes the FULL inputs (as produced by the reference
setup_inputs), shards them across 8 NeuronCores (2 cores per batch element,
causal-balanced parity split of query blocks), runs a Bass/Tile SPMD kernel,
and reassembles the full [B, T, C] output.

Design (v2):
  - Wq/Wk/Wv/Wp are fp8e4m3 (host-quantized at 32x scale, LN1 gain folded in)
    and SBUF-resident; projections run as DoubleRow fp8 matmuls (2 contraction
    chunks per instruction).
  - LN1 -> transpose -> K/V flow is fused per 512-token group; Q is computed
    for the full sequence (same code path as K) and attention reads the own
    (parity) query blocks via strided views.
  - Scores run per head-PAIR: two row-tiled matmuls (contraction rows 0-63 /
    64-127 of the PE array) execute concurrently on hardware; one Exp
    activation covers both heads' score chunks.
  - V carries 32x values plus a ones column; AV accumulates [32*attn; rowsum]
    in PSUM; normalization folds the 1/32 into the per-token reciprocal
    multiply. attnT is fp8 in SBUF (no DRAM round trip), feeding a DoubleRow
    Wp projection.
  - x2 (attention residual) stays in SBUF; MLP is a single pass over bf16
    W1/W2 with bf16 ff1 activations; b2 and the final residual are folded
    into the output assembly.
"""
import sys
sys.path.insert(0, '/opt/trn_rl_repo')
import numpy as np
from contextlib import ExitStack

import concourse.bacc as bacc
import concourse.tile as tile
import concourse.mybir as mybir
from concourse.masks import make_identity

F32 = mybir.dt.float32
F32R = mybir.dt.float32r
BF16 = mybir.dt.bfloat16
F8 = mybir.dt.float8e4
AF = mybir.ActivationFunctionType
ALU = mybir.AluOpType
PM = mybir.MatmulPerfMode

B, T, C, H, DH = 4, 2048, 1024, 16, 64
N_CORES = 8
TOK = 1024          # own tokens per core
NB = TOK // 128     # 8 own query blocks
KB = T // 128       # 16 key blocks
CCH = C // 128      # 8 channel chunks
CP = CCH // 2       # 4 chunk pairs (DoubleRow)
FF = 4 * C          # 4096
FCH = FF // 128     # 32 ff chunks
EPS = 1e-5
WSC = 32.0          # fp8 weight scale

IN_NAMES = ["xfull", "xown", "qpos", "kposb", "Wq", "Wk", "Wv", "Wp", "bp",
            "W1", "b1", "W2", "b2", "qbias", "kbias", "vbias"]


def build(nc, reps=1, loop_reps=None):
    """Trace the SPMD program into nc (a bacc.Bacc). Call nc.compile() after.

    Weight inputs arrive pre-folded on the host:
      Wq/Wk/Wv = fp8(32 * diag(g1) @ W);  qbias/kbias = be1 @ W
      Wp = fp8(32 * Wp);  vbias = 32 * be1 @ Wv
      W1 = bf16(diag(g2) @ W1);  b1 = b1 + be2 @ W1
      W2 = bf16(W2);  b2 arranged [128, CCH].
    """
    def din(name, shape, dt=F32):
        return nc.dram_tensor(name, shape, dt, kind="ExternalInput")

    xfull_d = din("xfull", [T, C])
    xown_d = din("xown", [TOK, C])
    qpos_d = din("qpos", [NB, 128])
    kposb_d = din("kposb", [1, KB])
    Wq_d = din("Wq", [C, C], F8); Wk_d = din("Wk", [C, C], F8)
    Wv_d = din("Wv", [C, C], F8); Wp_d = din("Wp", [C, C], F8)
    bp_d = din("bp", [1, C]); W1_d = din("W1", [C, FF], BF16); b1_d = din("b1", [1, FF])
    W2_d = din("W2", [FF, C], BF16); b2_d = din("b2", [128, CCH])
    qb_d = din("qbias", [NB, 128])   # be1 @ Wq, laid out [pair, within]
    kb_d = din("kbias", [NB, 128])   # be1 @ Wk
    vb_d = din("vbias", [1, C])      # 32 * be1 @ Wv
    out_d = nc.dram_tensor("out", [TOK, C], F32, kind="ExternalOutput")

    Wqv = Wq_d.ap().rearrange("(o p) m -> p o m", p=128)
    Wkv = Wk_d.ap().rearrange("(o p) m -> p o m", p=128)
    Wvv = Wv_d.ap().rearrange("(o p) m -> p o m", p=128)
    Wpv = Wp_d.ap().rearrange("(o p) m -> p o m", p=128)
    W1v = W1_d.ap().rearrange("(o p) m -> o p m", p=128)
    W2v = W2_d.ap().rearrange("(o p) m -> p o m", p=128)  # [128, 32, 1024]
    xf = xfull_d.ap()
    xo = xown_d.ap()

    env = locals()
    if loop_reps is not None:
        with tile.TileContext(nc) as tc:
            with tc.For_i(0, loop_reps, 1):
                with ExitStack() as top:
                    _build_body(nc, tc, top, env)
    else:
        for _rep in range(reps):
            with tile.TileContext(nc) as tc, ExitStack() as top:
                _build_body(nc, tc, top, env)
    return IN_NAMES


def _build_body(nc, tc, top, env):
    (xfull_d, xown_d, qpos_d, kposb_d, Wq_d, Wk_d, Wv_d, Wp_d, bp_d, W1_d,
     b1_d, W2_d, b2_d, qb_d, kb_d, vb_d, out_d, Wqv, Wkv, Wvv, Wpv, W1v, W2v,
     xf, xo) = (
        env[k] for k in ["xfull_d", "xown_d", "qpos_d", "kposb_d", "Wq_d",
                         "Wk_d", "Wv_d", "Wp_d", "bp_d", "W1_d", "b1_d", "W2_d",
                         "b2_d", "qb_d", "kb_d", "vb_d", "out_d", "Wqv", "Wkv",
                         "Wvv", "Wpv", "W1v", "W2v", "xf", "xo"])

    const = top.enter_context(tc.tile_pool(name="const", bufs=1))
    identf = const.tile([128, 128], F32)
    make_identity(nc, identf[:])
    identb = const.tile([128, 128], BF16)
    nc.vector.tensor_copy(identb[:], identf[:])
    eps_t = const.tile([128, 1], F32)
    nc.vector.memset(eps_t[:], EPS)

    def ln_stats(pool, x_ap):
        n = x_ap.shape[-1] // 512
        xg = x_ap.rearrange("p (n f) -> p n f", f=512)
        stats = pool.tile([128, n, 6], F32, tag="ln_stats")
        mv = pool.tile([128, 2], F32, tag="ln_mv")
        for i in range(n):
            nc.vector.bn_stats(stats[:, i], xg[:, i])
        nc.vector.bn_aggr(mv[:], stats[:])
        rstd = pool.tile([128, 1], F32, tag="ln_rstd")
        nc.scalar.activation(rstd[:], mv[:, 1:2], AF.Sqrt, bias=eps_t[:])
        nc.vector.reciprocal(rstd[:], rstd[:])
        return mv[:, 0:1], rstd

    def ln_apply(pool, out_ap, x_ap, mean, rstd):
        # out = (x - mu) * rstd on ACT: Identity(x * rstd + (-mu * rstd))
        nmr = pool.tile([128, 1], F32, tag="ln_nmr")
        nc.vector.tensor_scalar(nmr[:], mean, rstd[:], -1.0,
                                op0=ALU.mult, op1=ALU.mult)
        nc.scalar.activation(out_ap, x_ap, AF.Identity,
                             bias=nmr[:], scale=rstd[:])

    # ---------------- resident pools ----------------
    # left side: pools that live to the end of the body (LIFO close order)
    es_wp = ExitStack()
    wpp = es_wp.enter_context(tc.tile_pool(name="wpp", bufs=1))
    Wp_sb = wpp.tile([128, CCH, C], F8)
    es_at = ExitStack()
    atp = es_at.enter_context(tc.tile_pool(name="attnT", bufs=1))
    attnT8 = atp.tile([128, CCH, TOK], F8)

    # right side: big transients, ordered by lifetime (longest first)
    es_kqv = ExitStack()
    kqvp = es_kqv.enter_context(tc.tile_pool(name="kqv", bufs=1, side="right"))
    KT = kqvp.tile([128, CCH, T], BF16)
    QTf = kqvp.tile([128, CCH, TOK], BF16)
    V_sb = kqvp.tile([128, KB, H, 65], BF16)
    ones_b = kqvp.tile([128, 1], BF16)
    nc.vector.memset(ones_b[:], 1.0)
    nc.vector.tensor_copy(V_sb[:, :, :, 64:65],
                          ones_b[:, 0:1, None, None].to_broadcast([128, KB, H, 1]))

    es_wqkv = ExitStack()
    wqkv = es_wqkv.enter_context(tc.tile_pool(name="wqkv", bufs=1, side="right"))
    Wq_sb = wqkv.tile([128, CCH, C], F8)
    Wk_sb = wqkv.tile([128, CCH, C], F8)
    Wv_sb = wqkv.tile([128, CCH, C], F8)
    nc.sync.dma_start(Wv_sb[:], Wvv)
    nc.sync.dma_start(Wk_sb[:], Wkv)
    nc.sync.dma_start(Wq_sb[:], Wqv)

    es_h = ExitStack()
    hp_ = es_h.enter_context(tc.tile_pool(name="hT8", bufs=1, side="right"))
    hT8 = hp_.tile([128, CCH, T], F8)

    # ============ Phase AB: LN1 + V/K/Q per 512-token group ============
    with tc.tile_pool(name="stAB", bufs=3) as stAB, \
         tc.tile_pool(name="stABc", bufs=1) as stABc, \
         tc.tile_pool(name="stAB_t_ps", bufs=3, space="PSUM") as psT, \
         tc.tile_pool(name="stAB_v_ps", bufs=1, space="PSUM") as psV, \
         tc.tile_pool(name="stAB_k_ps", bufs=2, space="PSUM") as psK:
        vb_b = stABc.tile([128, C], F32)
        nc.sync.dma_start(vb_b[:], vb_d.ap().to_broadcast([128, C]))
        kb_sb = stABc.tile([128, NB], F32)
        nc.sync.dma_start(kb_sb[:], kb_d.ap().rearrange("o p -> p o"))
        qb_sb = stABc.tile([128, NB], F32)
        nc.sync.dma_start(qb_sb[:], qb_d.ap().rearrange("o p -> p o"))
        for g in range(T // 512):
            for tb4 in range(4):
                tb = g * 4 + tb4
                x_t = stAB.tile([128, C], F32, tag="x_t")
                nc.sync.dma_start(x_t[:], xf[tb * 128:(tb + 1) * 128, :])
                mean, rstd = ln_stats(stAB, x_t[:])
                hrow = stAB.tile([128, C], BF16, tag="hrow")
                ln_apply(stAB, hrow[:], x_t[:], mean, rstd)
                for cc in range(CCH):
                    pt = psT.tile([128, 128], BF16, tag="psT_t")
                    nc.tensor.transpose(pt[:], hrow[:, cc * 128:(cc + 1) * 128],
                                        identb[:])
                    eng = nc.scalar.copy if cc % 2 == 0 else nc.vector.tensor_copy
                    eng(hT8[:, cc, tb * 128:(tb + 1) * 128], pt[:])
                # V for this token block: stationary hT8(tb), moving Wv
                pv = psV.tile([128, 2, 512], F32, tag="pv")
                for cp in range(CP):
                    for grp in range(2):
                        nc.tensor.matmul(
                            pv[:, grp], hT8[:, 2 * cp:2 * cp + 2,
                                            tb * 128:(tb + 1) * 128],
                            Wv_sb[:, 2 * cp:2 * cp + 2, grp * 512:(grp + 1) * 512],
                            start=(cp == 0), stop=(cp == CP - 1),
                            perf_mode=PM.DoubleRow)
                for grp in range(2):
                    nc.vector.tensor_tensor(
                        V_sb[:, tb, grp * 8:(grp + 1) * 8, 0:64],
                        pv[:, grp].rearrange("p (h d) -> p h d", d=64),
                        vb_b[:, grp * 512:(grp + 1) * 512].rearrange(
                            "p (h d) -> p h d", d=64),
                        ALU.add)
            # K and Q (full sequence) for this 512-token group
            for pair in range(CCH):
                pk = psK.tile([128, 512], F32, tag="pk")
                for cp in range(CP):
                    nc.tensor.matmul(
                        pk[:], Wk_sb[:, 2 * cp:2 * cp + 2, pair * 128:(pair + 1) * 128],
                        hT8[:, 2 * cp:2 * cp + 2, g * 512:(g + 1) * 512],
                        start=(cp == 0), stop=(cp == CP - 1), perf_mode=PM.DoubleRow)
                nc.scalar.activation(KT[:, pair, g * 512:(g + 1) * 512], pk[:],
                                     AF.Identity, bias=kb_sb[:, pair:pair + 1],
                                     scale=1.0 / WSC)
                if g < TOK // 512:  # own (first) tokens only
                    pq = psK.tile([128, 512], F32, tag="pk")
                    for cp in range(CP):
                        nc.tensor.matmul(
                            pq[:], Wq_sb[:, 2 * cp:2 * cp + 2,
                                         pair * 128:(pair + 1) * 128],
                            hT8[:, 2 * cp:2 * cp + 2, g * 512:(g + 1) * 512],
                            start=(cp == 0), stop=(cp == CP - 1),
                            perf_mode=PM.DoubleRow)
                    nc.scalar.activation(QTf[:, pair, g * 512:(g + 1) * 512], pq[:],
                                         AF.Identity, bias=qb_sb[:, pair:pair + 1],
                                         scale=1.0 / WSC)
    es_h.close()    # free hT8
    es_wqkv.close() # free Wq/Wk/Wv

    # ---------- mask constants (key positions are per-core data) ----------
    es_mask = ExitStack()
    maskp = es_mask.enter_context(tc.tile_pool(name="maskp", bufs=1, side="right"))
    negm = maskp.tile([128, KB, 128], BF16)   # -1e9 where masked, else 0
    with tc.tile_pool(name="mtmp", bufs=1) as mtmp:
        pi_i = mtmp.tile([128, 1], mybir.dt.int32)
        nc.gpsimd.iota(pi_i[:], pattern=[[1, 1]], base=0, channel_multiplier=1)
        pi_f = mtmp.tile([128, 1], F32)
        nc.vector.tensor_copy(pi_f[:], pi_i[:])
        kpb = mtmp.tile([128, KB], F32)
        nc.sync.dma_start(kpb[:], kposb_d.ap().to_broadcast([128, KB]))
        kp_f = mtmp.tile([128, KB], F32)
        nc.vector.tensor_scalar(kp_f[:], kpb[:], pi_f[:], None, op0=ALU.add)
        qb = mtmp.tile([128, NB, 128], F32)
        for j in range(NB):
            nc.sync.dma_start(qb[:, j], qpos_d.ap()[j:j + 1, :].to_broadcast([128, 128]))
        for k in range(KB):
            jmin = k if k < NB else k - NB
            # negm[p_key, f_q] = -1e9 * (qpos_jmin[f] < keypos(block k)[p])
            nc.vector.tensor_scalar(
                negm[:, k], qb[:, jmin], kp_f[:, k:k + 1], -1e9,
                op0=ALU.is_lt, op1=ALU.mult)

    # ============ Phase C: attention (per head pair) ============
    nc.sync.dma_start(Wp_sb[:], Wpv)   # overlaps with attention
    with tc.tile_pool(name="stC", bufs=3) as stC, \
         tc.tile_pool(name="stC_att_ps", bufs=1, space="PSUM") as psCa, \
         tc.tile_pool(name="stC_s_ps", bufs=2, space="PSUM") as psCs, \
         tc.tile_pool(name="stC_t_ps", bufs=2, space="PSUM") as psCt:
        for hp in range(CCH):  # head pair = channel pair chunk
            for qp in range(2):  # query half: own cols [qp*512, qp*512+512)
                qbase = qp * 512
                klist = [k for k in range(KB)
                         if (k if k < NB else k - NB) * 128 < qbase + 512]
                ps_att = psCa.tile([128, 2, 512], F32, tag="ps_att")
                for ki, k in enumerate(klist):
                    jmin = k if k < NB else k - NB
                    q0 = jmin * 128
                    qlo = max(q0, qbase)
                    nq = qbase + 512 - qlo
                    weiT2 = stC.tile([128, 2, 512], BF16, tag="weiT")
                    ps_s = psCs.tile([128, 2, 512], F32, tag="ps_s")
                    diag = qlo == q0
                    for t in range(2):
                        nc.tensor.matmul(
                            ps_s[:, t, 0:nq],
                            KT[64 * t:64 * t + 64, hp, k * 128:(k + 1) * 128],
                            QTf[64 * t:64 * t + 64, hp, qlo:qlo + nq],
                            start=True, stop=not diag)
                        if diag:
                            # causal mask: accumulate -1e9 into the diagonal
                            # block via PE (identity @ negm)
                            nc.tensor.matmul(
                                ps_s[:, t, 0:128], identb[:], negm[:, k],
                                start=False, stop=True)
                    nc.scalar.activation(weiT2[:, :, 0:nq],
                                         ps_s[:, :, 0:nq], AF.Exp, scale=0.125)
                    # AV: accumulate [32*attn ; rowsum] per head
                    for t in range(2):
                        nc.tensor.matmul(
                            ps_att[0:65, t, qlo - qbase:512],
                            V_sb[:, k, 2 * hp + t, :],
                            weiT2[:, t, 0:nq],
                            start=(ki == 0), stop=(ki == len(klist) - 1))
                # normalize + transpose into attnT8 (copies on DVE: keep the
                # Act engine exp-only during attention to avoid table thrash)
                for t in range(2):
                    sb_at = stC.tile([128, 512], BF16, tag="sb_at")
                    nc.vector.tensor_copy(sb_at[0:65, :], ps_att[0:65, t, :])
                    for jj in range(4):
                        j = qp * 4 + jj
                        pt1 = psCt.tile([128, 128], BF16, tag="ptn")
                        nc.tensor.transpose(pt1[:], sb_at[:, jj * 128:(jj + 1) * 128],
                                            identb[:])
                        recip = stC.tile([128, 1], F32, tag="recip")
                        nc.vector.reciprocal(recip[:], pt1[:, 64:65])
                        attn_j = stC.tile([128, 64], BF16, tag="attn_j")
                        nc.vector.tensor_scalar(attn_j[:], pt1[:, 0:64], recip[:],
                                                1.0 / WSC, op0=ALU.mult, op1=ALU.mult)
                        pt2 = psCt.tile([128, 128], BF16, tag="ptn")
                        nc.tensor.transpose(pt2[0:64, :], attn_j[:], identb[:])
                        nc.vector.tensor_copy(
                            attnT8[64 * t:64 * t + 64, hp, j * 128:(j + 1) * 128],
                            pt2[0:64, :])
    es_mask.close()
    es_kqv.close()   # free KT, QTf, V

    # ============ Phase D: Wp proj + residual + LN2 ============
    es_x2 = ExitStack()
    x2p = es_x2.enter_context(tc.tile_pool(name="x2h2", bufs=1))
    x2_sb = x2p.tile([128, NB, C], F32)     # token-major
    h2T = x2p.tile([128, CCH, TOK], BF16)   # channel-major (for MLP)
    with tc.tile_pool(name="stD", bufs=2) as stD, \
         tc.tile_pool(name="stD_c", bufs=1) as stDc, \
         tc.tile_pool(name="stD_ps", bufs=2, space="PSUM") as psD, \
         tc.tile_pool(name="stD_t_ps", bufs=3, space="PSUM") as psDt:
        for nt in range(TOK // 512):
            pT_sb = stD.tile([128, CCH, 512], BF16, tag="pT_sb")
            for co in range(CCH):
                pp = psD.tile([128, 512], F32, tag="pp")
                for cp in range(CP):
                    nc.tensor.matmul(
                        pp[:], Wp_sb[:, 2 * cp:2 * cp + 2, co * 128:(co + 1) * 128],
                        attnT8[:, 2 * cp:2 * cp + 2, nt * 512:(nt + 1) * 512],
                        start=(cp == 0), stop=(cp == CP - 1), perf_mode=PM.DoubleRow)
                nc.scalar.activation(pT_sb[:, co], pp[:], AF.Identity,
                                     scale=1.0 / WSC)
            for sub in range(4):
                tb = nt * 4 + sub
                xo_t = stD.tile([128, C], F32, tag="xo_t")
                nc.sync.dma_start(xo_t[:], xo[tb * 128:(tb + 1) * 128, :])
                for co in range(CCH):
                    ptd = psDt.tile([128, 128], BF16, tag="ptd")
                    nc.tensor.transpose(ptd[:], pT_sb[:, co, sub * 128:(sub + 1) * 128],
                                        identb[:])
                    nc.vector.tensor_tensor(x2_sb[:, tb, co * 128:(co + 1) * 128],
                                            ptd[:], xo_t[:, co * 128:(co + 1) * 128],
                                            ALU.add)
                # LN2
                mean, rstd = ln_stats(stD, x2_sb[:, tb, :])
                h2row = stD.tile([128, C], BF16, tag="h2row")
                ln_apply(stD, h2row[:], x2_sb[:, tb, :], mean, rstd)
                for cc in range(CCH):
                    pt = psDt.tile([128, 128], BF16, tag="ptd2")
                    nc.tensor.transpose(pt[:], h2row[:, cc * 128:(cc + 1) * 128],
                                        identb[:])
                    eng = nc.scalar.copy if cc % 2 == 0 else nc.vector.tensor_copy
                    eng(h2T[:, cc, tb * 128:(tb + 1) * 128], pt[:])

    # ============ Phase E: MLP (single pass, bf16) ============
    es_ff = ExitStack()
    ffp = es_ff.enter_context(tc.tile_pool(name="ff1T", bufs=1))
    ff1T = ffp.tile([128, FCH, TOK], BF16)
    with tc.tile_pool(name="stE_c", bufs=1) as stEc:
        b1p = stEc.tile([128, FCH], F32)
        nc.sync.dma_start(b1p[:], b1_d.ap().rearrange("x (o p) -> p (x o)", p=128))
        b2p = stEc.tile([128, CCH], F32)
        nc.sync.dma_start(b2p[:], b2_d.ap())
        with tc.tile_pool(name="stE1", bufs=2) as stE1, \
             tc.tile_pool(name="stE1_ps", bufs=2, space="PSUM") as psE1, \
             tc.tile_pool(name="stE2", bufs=2) as stE2, \
             tc.tile_pool(name="stE2_ps", bufs=2, space="PSUM") as psE2, \
             tc.tile_pool(name="stE2_t_ps", bufs=2, space="PSUM") as psEt:
            for fog in range(8):
                w1g = stE1.tile([128, CCH, 512], BF16, tag="w1g")
                nc.sync.dma_start(
                    w1g[:], W1v.transpose([1, 0, 2])[:, :, fog * 512:(fog + 1) * 512])
                for f4 in range(4):
                    fo = fog * 4 + f4
                    for nt in range(TOK // 512):
                        pf = psE1.tile([128, 512], F32, tag="pf")
                        for cc in range(CCH):
                            nc.tensor.matmul(
                                pf[:], w1g[:, cc, f4 * 128:(f4 + 1) * 128],
                                h2T[:, cc, nt * 512:(nt + 1) * 512],
                                start=(cc == 0), stop=(cc == CCH - 1))
                        nc.scalar.activation(
                            ff1T[:, fo, nt * 512:(nt + 1) * 512], pf[:], AF.Relu,
                            bias=b1p[:, fo:fo + 1])
            for co in range(CCH):
                w2c = stE2.tile([128, FCH, 128], BF16, tag="w2c")
                nc.sync.dma_start(
                    w2c[:], W2v[:, :, co * 128:(co + 1) * 128])
                for nt in range(TOK // 512):
                    p2 = psE2.tile([128, 512], F32, tag="p2")
                    for fo in range(FCH):
                        nc.tensor.matmul(p2[:], w2c[:, fo],
                                         ff1T[:, fo, nt * 512:(nt + 1) * 512],
                                         start=(fo == 0), stop=(fo == FCH - 1))
                    ff2_c = stE2.tile([128, 512], BF16, tag="ff2_c")
                    nc.scalar.activation(ff2_c[:], p2[:], AF.Identity,
                                         bias=b2p[:, co:co + 1])
                    for sub in range(4):
                        tb = nt * 4 + sub
                        ptf = psEt.tile([128, 128], BF16, tag="ptf")
                        nc.tensor.transpose(ptf[:], ff2_c[:, sub * 128:(sub + 1) * 128],
                                            identb[:])
                        out_c = stE2.tile([128, 128], F32, tag="out_c")
                        nc.vector.tensor_tensor(
                            out_c[:], ptf[:], x2_sb[:, tb, co * 128:(co + 1) * 128],
                            ALU.add)
                        nc.sync.dma_start(
                            out_d.ap()[tb * 128:(tb + 1) * 128,
                                       co * 128:(co + 1) * 128], out_c[:])
    es_ff.close()
    es_x2.close()
    es_at.close()   # free attnT8
    es_wp.close()   # free Wp


def make_nc():
    nc = bacc.Bacc("TRN2", target_bir_lowering=False, debug=False,
                   num_devices=N_CORES)
    build(nc)
    nc.compile()
    return nc


def shard_inputs(inputs):
    """Full inputs dict -> list of 8 per-core in_maps.

    Folds LN1 gain/bias into Wq/Wk/Wv (weights scaled by g1 per input channel,
    be1 contribution becomes an additive bias on q/k/v) and LN2's into W1/b1.
    Wq/Wk/Wv/Wp are fp8 at 32x scale; W1/W2 are bf16.
    """
    E4 = mybir.dt.np(F8)
    BF = mybir.dt.np(BF16)
    x = np.asarray(inputs["x"], np.float32)
    assert x.shape == (B, T, C)
    f64 = np.float64
    Wq = np.asarray(inputs["Wq"], f64); Wk = np.asarray(inputs["Wk"], f64)
    Wv = np.asarray(inputs["Wv"], f64); Wp = np.asarray(inputs["Wp"], f64)
    W1 = np.asarray(inputs["W1"], f64); W2 = np.asarray(inputs["W2"], np.float32)
    g1 = np.asarray(inputs["g1"], f64); be1 = np.asarray(inputs["be1"], f64)
    g2 = np.asarray(inputs["g2"], f64); be2 = np.asarray(inputs["be2"], f64)
    b1 = np.asarray(inputs["b1"], f64)
    shared = {
        "Wq": (WSC * g1[:, None] * Wq).astype(np.float32).astype(E4),
        "Wk": (WSC * g1[:, None] * Wk).astype(np.float32).astype(E4),
        "Wv": (WSC * g1[:, None] * Wv).astype(np.float32).astype(E4),
        "Wp": (WSC * Wp).astype(np.float32).astype(E4),
        "W1": (g2[:, None] * W1).astype(np.float32).astype(BF),
        "W2": W2.astype(BF),
        "qbias": (be1 @ Wq).astype(np.float32).reshape(NB, 128),
        "kbias": (be1 @ Wk).astype(np.float32).reshape(NB, 128),
        "vbias": (WSC * (be1 @ Wv)).astype(np.float32).reshape(1, C),
        "b1": (b1 + be2 @ W1).astype(np.float32).reshape(1, FF),
        "bp": np.asarray(inputs["bp"], np.float32).reshape(1, C),
        "b2": np.ascontiguousarray(
            np.asarray(inputs["b2"], np.float32).reshape(CCH, 128).T),
    }
    in_maps = []
    for c in range(N_CORES):
        b, par = c // 2, c % 2
        # permuted sequence: own (parity) blocks first, partner blocks after,
        # so the program is parity-independent; key positions ride as data
        gblocks = [2 * j + par for j in range(NB)] + \
                  [2 * j + (1 - par) for j in range(NB)]
        xperm = np.concatenate([x[b, g * 128:(g + 1) * 128, :] for g in gblocks], 0)
        # bp is folded into the residual rows (x2 = xown + bp + attn @ Wp)
        rows = xperm[:TOK] + np.asarray(inputs["bp"], np.float32)[None, :]
        qpos = np.stack([np.arange(g * 128, (g + 1) * 128, dtype=np.float32)
                         for g in gblocks[:NB]], 0)
        kposb = np.array([[g * 128 for g in gblocks]], dtype=np.float32)
        m = {"xfull": np.ascontiguousarray(xperm),
             "xown": np.ascontiguousarray(rows), "qpos": qpos, "kposb": kposb}
        m.update(shared)
        in_maps.append(m)
    return in_maps


def unshard_outputs(results):
    """list of per-core {'out': [TOK, C]} -> [B, T, C]"""
    out = np.zeros((B, T, C), np.float32)
    for c in range(N_CORES):
        b, par = c // 2, c % 2
        r = np.asarray(results[c]["out"])
        for j in range(NB):
            g = 2 * j + par
            out[b, g * 128:(g + 1) * 128, :] = r[j * 128:(j + 1) * 128, :]
    return out


_NC_CACHE = {}

def _get_nc():
    if "nc" not in _NC_CACHE:
        nc = bacc.Bacc("TRN2", target_bir_lowering=False, debug=False,
                       num_devices=N_CORES)
        build(nc, reps=1)
        nc.compile()
        _NC_CACHE["nc"] = nc
    return _NC_CACHE["nc"]


def kernel(**inputs):
    from concourse.bass_utils import run_bass_kernel_spmd
    nc = _get_nc()
    in_maps = shard_inputs(inputs)
    res = run_bass_kernel_spmd(nc, in_maps, core_ids=list(range(N_CORES)))
    return unshard_outputs(res.results)


# revision 40
# speedup vs baseline: 266.4906x; 3.3517x over previous
"""Self-contained Trainium2 kernel for the dense transformer block problem.

kernel(**inputs) takes the FULL inputs (as produced by the reference
setup_inputs), shards them across 8 NeuronCores (2 cores per batch element,
causal-balanced parity split of query blocks), runs a Bass/Tile SPMD kernel,
and reassembles the full [B, T, C] output.

Design (v2):
  - Wq/Wk/Wv/Wp are fp8e4m3 (host-quantized at 32x scale, LN1 gain folded in)
    and SBUF-resident; projections run as DoubleRow fp8 matmuls (2 contraction
    chunks per instruction).
  - LN1 -> transpose -> K/V flow is fused per 512-token group; Q is computed
    for the full sequence (same code path as K) and attention reads the own
    (parity) query blocks via strided views.
  - Scores run per head-PAIR: two row-tiled matmuls (contraction rows 0-63 /
    64-127 of the PE array) execute concurrently on hardware; one Exp
    activation covers both heads' score chunks.
  - V carries 32x values plus a ones column; AV accumulates [32*attn; rowsum]
    in PSUM; normalization folds the 1/32 into the per-token reciprocal
    multiply. attnT is fp8 in SBUF (no DRAM round trip), feeding a DoubleRow
    Wp projection.
  - x2 (attention residual) stays in SBUF; MLP is a single pass over bf16
    W1/W2 with bf16 ff1 activations; b2 and the final residual are folded
    into the output assembly.
"""
import sys
sys.path.insert(0, '/opt/trn_rl_repo')
import numpy as np
from contextlib import ExitStack

import concourse.bacc as bacc
import concourse.tile as tile
import concourse.mybir as mybir
from concourse.masks import make_identity

F32 = mybir.dt.float32
F32R = mybir.dt.float32r
BF16 = mybir.dt.bfloat16
F8 = mybir.dt.float8e4
AF = mybir.ActivationFunctionType
ALU = mybir.AluOpType
PM = mybir.MatmulPerfMode

B, T, C, H, DH = 4, 2048, 1024, 16, 64
N_CORES = 8
TOK = 1024          # own tokens per core
NB = TOK // 128     # 8 own query blocks
KB = T // 128       # 16 key blocks
CCH = C // 128      # 8 channel chunks
CP = CCH // 2       # 4 chunk pairs (DoubleRow)
FF = 4 * C          # 4096
FCH = FF // 128     # 32 ff chunks
EPS = 1e-5
WSC = 32.0          # fp8 weight scale

IN_NAMES = ["xfull", "xown", "qpos", "kposb", "Wq", "Wk", "Wv", "Wp", "bp",
            "W1", "b1", "W2", "b2", "qbias", "kbias", "vbias"]


def build(nc, reps=1, loop_reps=None):
    """Trace the SPMD program into nc (a bacc.Bacc). Call nc.compile() after.

    Weight inputs arrive pre-folded on the host:
      Wq/Wk/Wv = fp8(32 * diag(g1) @ W);  qbias/kbias = be1 @ W
      Wp = fp8(32 * Wp);  vbias = 32 * be1 @ Wv
      W1 = bf16(diag(g2) @ W1);  b1 = b1 + be2 @ W1
      W2 = bf16(W2);  b2 arranged [128, CCH].
    """
    def din(name, shape, dt=F32):
        return nc.dram_tensor(name, shape, dt, kind="ExternalInput")

    xfull_d = din("xfull", [T, C])
    xown_d = din("xown", [TOK, C])
    qpos_d = din("qpos", [NB, 128])
    kposb_d = din("kposb", [1, KB])
    Wq_d = din("Wq", [C, C], F8); Wk_d = din("Wk", [C, C], F8)
    Wv_d = din("Wv", [C, C], F8); Wp_d = din("Wp", [C, C], F8)
    bp_d = din("bp", [1, C]); W1_d = din("W1", [C, FF], BF16); b1_d = din("b1", [1, FF])
    W2_d = din("W2", [FF, C], BF16); b2_d = din("b2", [128, CCH])
    qb_d = din("qbias", [NB, 128])   # be1 @ Wq, laid out [pair, within]
    kb_d = din("kbias", [NB, 128])   # be1 @ Wk
    vb_d = din("vbias", [1, C])      # 32 * be1 @ Wv
    out_d = nc.dram_tensor("out", [TOK, C], F32, kind="ExternalOutput")

    Wqv = Wq_d.ap().rearrange("(o p) m -> p o m", p=128)
    Wkv = Wk_d.ap().rearrange("(o p) m -> p o m", p=128)
    Wvv = Wv_d.ap().rearrange("(o p) m -> p o m", p=128)
    Wpv = Wp_d.ap().rearrange("(o p) m -> p o m", p=128)
    W1v = W1_d.ap().rearrange("(o p) m -> o p m", p=128)
    W2v = W2_d.ap().rearrange("(o p) m -> p o m", p=128)  # [128, 32, 1024]
    xf = xfull_d.ap()
    xo = xown_d.ap()

    env = locals()
    if loop_reps is not None:
        with tile.TileContext(nc) as tc:
            with tc.For_i(0, loop_reps, 1):
                with ExitStack() as top:
                    _build_body(nc, tc, top, env)
    else:
        for _rep in range(reps):
            with tile.TileContext(nc) as tc, ExitStack() as top:
                _build_body(nc, tc, top, env)
    return IN_NAMES


def _build_body(nc, tc, top, env):
    (xfull_d, xown_d, qpos_d, kposb_d, Wq_d, Wk_d, Wv_d, Wp_d, bp_d, W1_d,
     b1_d, W2_d, b2_d, qb_d, kb_d, vb_d, out_d, Wqv, Wkv, Wvv, Wpv, W1v, W2v,
     xf, xo) = (
        env[k] for k in ["xfull_d", "xown_d", "qpos_d", "kposb_d", "Wq_d",
                         "Wk_d", "Wv_d", "Wp_d", "bp_d", "W1_d", "b1_d", "W2_d",
                         "b2_d", "qb_d", "kb_d", "vb_d", "out_d", "Wqv", "Wkv",
                         "Wvv", "Wpv", "W1v", "W2v", "xf", "xo"])

    const = top.enter_context(tc.tile_pool(name="const", bufs=1))
    identf = const.tile([128, 128], F32)
    make_identity(nc, identf[:])
    identb = const.tile([128, 128], BF16)
    nc.vector.tensor_copy(identb[:], identf[:])
    eps_t = const.tile([128, 1], F32)
    nc.vector.memset(eps_t[:], EPS)

    def ln_stats(pool, x_ap):
        n = x_ap.shape[-1] // 512
        xg = x_ap.rearrange("p (n f) -> p n f", f=512)
        stats = pool.tile([128, n, 6], F32, tag="ln_stats")
        mv = pool.tile([128, 2], F32, tag="ln_mv")
        for i in range(n):
            nc.vector.bn_stats(stats[:, i], xg[:, i])
        nc.vector.bn_aggr(mv[:], stats[:])
        rstd = pool.tile([128, 1], F32, tag="ln_rstd")
        nc.scalar.activation(rstd[:], mv[:, 1:2], AF.Sqrt, bias=eps_t[:])
        nc.vector.reciprocal(rstd[:], rstd[:])
        return mv[:, 0:1], rstd

    def ln_apply(pool, out_ap, x_ap, mean, rstd):
        # out = (x - mu) * rstd on ACT: Identity(x * rstd + (-mu * rstd))
        nmr = pool.tile([128, 1], F32, tag="ln_nmr")
        nc.vector.tensor_scalar(nmr[:], mean, rstd[:], -1.0,
                                op0=ALU.mult, op1=ALU.mult)
        nc.scalar.activation(out_ap, x_ap, AF.Identity,
                             bias=nmr[:], scale=rstd[:])

    # ---------------- resident pools ----------------
    # left side: pools that live to the end of the body (LIFO close order)
    es_wp = ExitStack()
    wpp = es_wp.enter_context(tc.tile_pool(name="wpp", bufs=1))
    Wp_sb = wpp.tile([128, CCH, C], F8)
    es_at = ExitStack()
    atp = es_at.enter_context(tc.tile_pool(name="attnT", bufs=1))
    attnT8 = atp.tile([128, CCH, TOK], F8)

    # right side: big transients, ordered by lifetime (longest first)
    es_kqv = ExitStack()
    kqvp = es_kqv.enter_context(tc.tile_pool(name="kqv", bufs=1, side="right"))
    KT = kqvp.tile([128, CCH, T], BF16)
    QTf = kqvp.tile([128, CCH, TOK], BF16)
    V_sb = kqvp.tile([128, KB, H, 65], BF16)
    ones_b = kqvp.tile([128, 1], BF16)
    nc.vector.memset(ones_b[:], 1.0)
    nc.vector.tensor_copy(V_sb[:, :, :, 64:65],
                          ones_b[:, 0:1, None, None].to_broadcast([128, KB, H, 1]))

    es_wqkv = ExitStack()
    wqkv = es_wqkv.enter_context(tc.tile_pool(name="wqkv", bufs=1, side="right"))
    Wq_sb = wqkv.tile([128, CCH, C], F8)
    Wk_sb = wqkv.tile([128, CCH, C], F8)
    Wv_sb = wqkv.tile([128, CCH, C], F8)
    nc.sync.dma_start(Wv_sb[:], Wvv)
    nc.sync.dma_start(Wk_sb[:], Wkv)
    nc.sync.dma_start(Wq_sb[:], Wqv)

    es_h = ExitStack()
    hp_ = es_h.enter_context(tc.tile_pool(name="hT8", bufs=1, side="right"))
    hT8 = hp_.tile([128, CCH, T], F8)

    # ============ Phase AB: LN1 + V/K/Q per 512-token group ============
    with tc.tile_pool(name="stAB", bufs=3) as stAB, \
         tc.tile_pool(name="stABc", bufs=1) as stABc, \
         tc.tile_pool(name="stAB_t_ps", bufs=3, space="PSUM") as psT, \
         tc.tile_pool(name="stAB_v_ps", bufs=1, space="PSUM") as psV, \
         tc.tile_pool(name="stAB_k_ps", bufs=2, space="PSUM") as psK:
        vb_b = stABc.tile([128, C], F32)
        nc.sync.dma_start(vb_b[:], vb_d.ap().to_broadcast([128, C]))
        kb_sb = stABc.tile([128, NB], F32)
        nc.sync.dma_start(kb_sb[:], kb_d.ap().rearrange("o p -> p o"))
        qb_sb = stABc.tile([128, NB], F32)
        nc.sync.dma_start(qb_sb[:], qb_d.ap().rearrange("o p -> p o"))
        for g in range(T // 512):
            for tb4 in range(4):
                tb = g * 4 + tb4
                x_t = stAB.tile([128, C], F32, tag="x_t")
                nc.sync.dma_start(x_t[:], xf[tb * 128:(tb + 1) * 128, :])
                mean, rstd = ln_stats(stAB, x_t[:])
                hrow = stAB.tile([128, C], BF16, tag="hrow")
                ln_apply(stAB, hrow[:], x_t[:], mean, rstd)
                for cc in range(CCH):
                    pt = psT.tile([128, 128], BF16, tag="psT_t")
                    nc.tensor.transpose(pt[:], hrow[:, cc * 128:(cc + 1) * 128],
                                        identb[:])
                    eng = nc.scalar.copy if cc % 2 == 0 else nc.vector.tensor_copy
                    eng(hT8[:, cc, tb * 128:(tb + 1) * 128], pt[:])
                # V for this token block: stationary hT8(tb), moving Wv
                pv = psV.tile([128, 2, 512], F32, tag="pv")
                for cp in range(CP):
                    for grp in range(2):
                        nc.tensor.matmul(
                            pv[:, grp], hT8[:, 2 * cp:2 * cp + 2,
                                            tb * 128:(tb + 1) * 128],
                            Wv_sb[:, 2 * cp:2 * cp + 2, grp * 512:(grp + 1) * 512],
                            start=(cp == 0), stop=(cp == CP - 1),
                            perf_mode=PM.DoubleRow)
                for grp in range(2):
                    nc.vector.tensor_tensor(
                        V_sb[:, tb, grp * 8:(grp + 1) * 8, 0:64],
                        pv[:, grp].rearrange("p (h d) -> p h d", d=64),
                        vb_b[:, grp * 512:(grp + 1) * 512].rearrange(
                            "p (h d) -> p h d", d=64),
                        ALU.add)
            # K and Q (full sequence) for this 512-token group
            for pair in range(CCH):
                pk = psK.tile([128, 512], F32, tag="pk")
                for cp in range(CP):
                    nc.tensor.matmul(
                        pk[:], Wk_sb[:, 2 * cp:2 * cp + 2, pair * 128:(pair + 1) * 128],
                        hT8[:, 2 * cp:2 * cp + 2, g * 512:(g + 1) * 512],
                        start=(cp == 0), stop=(cp == CP - 1), perf_mode=PM.DoubleRow)
                nc.scalar.activation(KT[:, pair, g * 512:(g + 1) * 512], pk[:],
                                     AF.Identity, bias=kb_sb[:, pair:pair + 1],
                                     scale=1.0 / WSC)
                if g < TOK // 512:  # own (first) tokens only
                    pq = psK.tile([128, 512], F32, tag="pk")
                    for cp in range(CP):
                        nc.tensor.matmul(
                            pq[:], Wq_sb[:, 2 * cp:2 * cp + 2,
                                         pair * 128:(pair + 1) * 128],
                            hT8[:, 2 * cp:2 * cp + 2, g * 512:(g + 1) * 512],
                            start=(cp == 0), stop=(cp == CP - 1),
                            perf_mode=PM.DoubleRow)
                    nc.scalar.activation(QTf[:, pair, g * 512:(g + 1) * 512], pq[:],
                                         AF.Identity, bias=qb_sb[:, pair:pair + 1],
                                         scale=1.0 / WSC)
    es_h.close()    # free hT8
    es_wqkv.close() # free Wq/Wk/Wv

    # ---------- mask constants (key positions are per-core data) ----------
    es_mask = ExitStack()
    maskp = es_mask.enter_context(tc.tile_pool(name="maskp", bufs=1, side="right"))
    negm = maskp.tile([128, KB, 128], BF16)   # -1e9 where masked, else 0
    with tc.tile_pool(name="mtmp", bufs=1) as mtmp:
        pi_i = mtmp.tile([128, 1], mybir.dt.int32)
        nc.gpsimd.iota(pi_i[:], pattern=[[1, 1]], base=0, channel_multiplier=1)
        pi_f = mtmp.tile([128, 1], F32)
        nc.vector.tensor_copy(pi_f[:], pi_i[:])
        kpb = mtmp.tile([128, KB], F32)
        nc.sync.dma_start(kpb[:], kposb_d.ap().to_broadcast([128, KB]))
        kp_f = mtmp.tile([128, KB], F32)
        nc.vector.tensor_scalar(kp_f[:], kpb[:], pi_f[:], None, op0=ALU.add)
        qb = mtmp.tile([128, NB, 128], F32)
        for j in range(NB):
            nc.sync.dma_start(qb[:, j], qpos_d.ap()[j:j + 1, :].to_broadcast([128, 128]))
        for k in range(KB):
            jmin = k if k < NB else k - NB
            # negm[p_key, f_q] = -1e9 * (qpos_jmin[f] < keypos(block k)[p])
            nc.vector.tensor_scalar(
                negm[:, k], qb[:, jmin], kp_f[:, k:k + 1], -1e9,
                op0=ALU.is_lt, op1=ALU.mult)

    # ============ Phase C: attention (per head pair) ============
    nc.sync.dma_start(Wp_sb[:], Wpv)   # overlaps with attention
    with tc.tile_pool(name="stC", bufs=3) as stC, \
         tc.tile_pool(name="stC_att_ps", bufs=1, space="PSUM") as psCa, \
         tc.tile_pool(name="stC_s_ps", bufs=2, space="PSUM") as psCs, \
         tc.tile_pool(name="stC_t_ps", bufs=2, space="PSUM") as psCt:
        for hp in range(CCH):  # head pair = channel pair chunk
            for qp in range(2):  # query half: own cols [qp*512, qp*512+512)
                qbase = qp * 512
                klist = [k for k in range(KB)
                         if (k if k < NB else k - NB) * 128 < qbase + 512]
                ps_att = psCa.tile([128, 2, 512], F32, tag="ps_att")
                for ki, k in enumerate(klist):
                    jmin = k if k < NB else k - NB
                    q0 = jmin * 128
                    qlo = max(q0, qbase)
                    nq = qbase + 512 - qlo
                    weiT2 = stC.tile([128, 2, 512], BF16, tag="weiT")
                    ps_s = psCs.tile([128, 2, 512], F32, tag="ps_s")
                    diag = qlo == q0
                    for t in range(2):
                        nc.tensor.matmul(
                            ps_s[:, t, 0:nq],
                            KT[64 * t:64 * t + 64, hp, k * 128:(k + 1) * 128],
                            QTf[64 * t:64 * t + 64, hp, qlo:qlo + nq],
                            start=True, stop=not diag)
                        if diag:
                            # causal mask: accumulate -1e9 into the diagonal
                            # block via PE (identity @ negm)
                            nc.tensor.matmul(
                                ps_s[:, t, 0:128], identb[:], negm[:, k],
                                start=False, stop=True)
                    nc.scalar.activation(weiT2[:, :, 0:nq],
                                         ps_s[:, :, 0:nq], AF.Exp, scale=0.125)
                    # AV: accumulate [32*attn ; rowsum] per head
                    for t in range(2):
                        nc.tensor.matmul(
                            ps_att[0:65, t, qlo - qbase:512],
                            V_sb[:, k, 2 * hp + t, :],
                            weiT2[:, t, 0:nq],
                            start=(ki == 0), stop=(ki == len(klist) - 1))
                # normalize + transpose into attnT8 (copies on DVE: keep the
                # Act engine exp-only during attention to avoid table thrash)
                for t in range(2):
                    sb_at = stC.tile([128, 512], BF16, tag="sb_at")
                    nc.vector.tensor_copy(sb_at[0:65, :], ps_att[0:65, t, :])
                    for jj in range(4):
                        j = qp * 4 + jj
                        pt1 = psCt.tile([128, 128], BF16, tag="ptn")
                        nc.tensor.transpose(pt1[:], sb_at[:, jj * 128:(jj + 1) * 128],
                                            identb[:])
                        recip = stC.tile([128, 1], F32, tag="recip")
                        nc.vector.reciprocal(recip[:], pt1[:, 64:65])
                        attn_j = stC.tile([128, 64], BF16, tag="attn_j")
                        nc.vector.tensor_scalar(attn_j[:], pt1[:, 0:64], recip[:],
                                                1.0 / WSC, op0=ALU.mult, op1=ALU.mult)
                        pt2 = psCt.tile([128, 128], BF16, tag="ptn")
                        nc.tensor.transpose(pt2[0:64, :], attn_j[:], identb[:])
                        nc.vector.tensor_copy(
                            attnT8[64 * t:64 * t + 64, hp, j * 128:(j + 1) * 128],
                            pt2[0:64, :])
    es_mask.close()
    es_kqv.close()   # free KT, QTf, V

    # ============ Phase D: Wp proj + residual + LN2 ============
    es_x2 = ExitStack()
    x2p = es_x2.enter_context(tc.tile_pool(name="x2h2", bufs=1))
    x2_sb = x2p.tile([128, NB, C], F32)     # token-major
    h2T = x2p.tile([128, CCH, TOK], BF16)   # channel-major (for MLP)
    with tc.tile_pool(name="stD", bufs=2) as stD, \
         tc.tile_pool(name="stD_c", bufs=1) as stDc, \
         tc.tile_pool(name="stD_ps", bufs=2, space="PSUM") as psD, \
         tc.tile_pool(name="stD_t_ps", bufs=3, space="PSUM") as psDt:
        for nt in range(TOK // 512):
            pT_sb = stD.tile([128, CCH, 512], BF16, tag="pT_sb")
            for co in range(CCH):
                pp = psD.tile([128, 512], F32, tag="pp")
                for cp in range(CP):
                    nc.tensor.matmul(
                        pp[:], Wp_sb[:, 2 * cp:2 * cp + 2, co * 128:(co + 1) * 128],
                        attnT8[:, 2 * cp:2 * cp + 2, nt * 512:(nt + 1) * 512],
                        start=(cp == 0), stop=(cp == CP - 1), perf_mode=PM.DoubleRow)
                nc.scalar.activation(pT_sb[:, co], pp[:], AF.Identity,
                                     scale=1.0 / WSC)
            for sub in range(4):
                tb = nt * 4 + sub
                xo_t = stD.tile([128, C], F32, tag="xo_t")
                nc.sync.dma_start(xo_t[:], xo[tb * 128:(tb + 1) * 128, :])
                for co in range(CCH):
                    ptd = psDt.tile([128, 128], BF16, tag="ptd")
                    nc.tensor.transpose(ptd[:], pT_sb[:, co, sub * 128:(sub + 1) * 128],
                                        identb[:])
                    nc.vector.tensor_tensor(x2_sb[:, tb, co * 128:(co + 1) * 128],
                                            ptd[:], xo_t[:, co * 128:(co + 1) * 128],
                                            ALU.add)
                # LN2
                mean, rstd = ln_stats(stD, x2_sb[:, tb, :])
                h2row = stD.tile([128, C], BF16, tag="h2row")
                ln_apply(stD, h2row[:], x2_sb[:, tb, :], mean, rstd)
                for cc in range(CCH):
                    pt = psDt.tile([128, 128], BF16, tag="ptd2")
                    nc.tensor.transpose(pt[:], h2row[:, cc * 128:(cc + 1) * 128],
                                        identb[:])
                    eng = nc.scalar.copy if cc % 2 == 0 else nc.vector.tensor_copy
                    eng(h2T[:, cc, tb * 128:(tb + 1) * 128], pt[:])

    # ============ Phase E: MLP (single pass, bf16) ============
    es_ff = ExitStack()
    ffp = es_ff.enter_context(tc.tile_pool(name="ff1T", bufs=1))
    ff1T = ffp.tile([128, FCH, TOK], BF16)
    with tc.tile_pool(name="stE_c", bufs=1) as stEc:
        b1p = stEc.tile([128, FCH], F32)
        nc.sync.dma_start(b1p[:], b1_d.ap().rearrange("x (o p) -> p (x o)", p=128))
        b2p = stEc.tile([128, CCH], F32)
        nc.sync.dma_start(b2p[:], b2_d.ap())
        with tc.tile_pool(name="stE1", bufs=2) as stE1, \
             tc.tile_pool(name="stE1_ps", bufs=2, space="PSUM") as psE1, \
             tc.tile_pool(name="stE2", bufs=2) as stE2, \
             tc.tile_pool(name="stE2_ps", bufs=2, space="PSUM") as psE2, \
             tc.tile_pool(name="stE2_t_ps", bufs=2, space="PSUM") as psEt:
            for fog in range(8):
                w1g = stE1.tile([128, CCH, 512], BF16, tag="w1g")
                nc.sync.dma_start(
                    w1g[:], W1v.transpose([1, 0, 2])[:, :, fog * 512:(fog + 1) * 512])
                for f4 in range(4):
                    fo = fog * 4 + f4
                    for nt in range(TOK // 512):
                        pf = psE1.tile([128, 512], F32, tag="pf")
                        for cc in range(CCH):
                            nc.tensor.matmul(
                                pf[:], w1g[:, cc, f4 * 128:(f4 + 1) * 128],
                                h2T[:, cc, nt * 512:(nt + 1) * 512],
                                start=(cc == 0), stop=(cc == CCH - 1))
                        nc.scalar.activation(
                            ff1T[:, fo, nt * 512:(nt + 1) * 512], pf[:], AF.Relu,
                            bias=b1p[:, fo:fo + 1])
            for co in range(CCH):
                w2c = stE2.tile([128, FCH, 128], BF16, tag="w2c")
                nc.sync.dma_start(
                    w2c[:], W2v[:, :, co * 128:(co + 1) * 128])
                for nt in range(TOK // 512):
                    p2 = psE2.tile([128, 512], F32, tag="p2")
                    for fo in range(FCH):
                        nc.tensor.matmul(p2[:], w2c[:, fo],
                                         ff1T[:, fo, nt * 512:(nt + 1) * 512],
                                         start=(fo == 0), stop=(fo == FCH - 1))
                    ff2_c = stE2.tile([128, 512], BF16, tag="ff2_c")
                    nc.scalar.activation(ff2_c[:], p2[:], AF.Identity,
                                         bias=b2p[:, co:co + 1])
                    for sub in range(4):
                        tb = nt * 4 + sub
                        ptf = psEt.tile([128, 128], BF16, tag="ptf")
                        nc.tensor.transpose(ptf[:], ff2_c[:, sub * 128:(sub + 1) * 128],
                                            identb[:])
                        out_c = stE2.tile([128, 128], F32, tag="out_c")
                        nc.vector.tensor_tensor(
                            out_c[:], ptf[:], x2_sb[:, tb, co * 128:(co + 1) * 128],
                            ALU.add)
                        nc.sync.dma_start(
                            out_d.ap()[tb * 128:(tb + 1) * 128,
                                       co * 128:(co + 1) * 128], out_c[:])
    es_ff.close()
    es_x2.close()
    es_at.close()   # free attnT8
    es_wp.close()   # free Wp


def make_nc():
    nc = bacc.Bacc("TRN2", target_bir_lowering=False, debug=False,
                   num_devices=N_CORES)
    build(nc)
    nc.compile()
    return nc


def shard_inputs(inputs):
    """Full inputs dict -> list of 8 per-core in_maps.

    Folds LN1 gain/bias into Wq/Wk/Wv (weights scaled by g1 per input channel,
    be1 contribution becomes an additive bias on q/k/v) and LN2's into W1/b1.
    Wq/Wk/Wv/Wp are fp8 at 32x scale; W1/W2 are bf16.
    """
    E4 = mybir.dt.np(F8)
    BF = mybir.dt.np(BF16)
    x = np.asarray(inputs["x"], np.float32)
    assert x.shape == (B, T, C)
    f64 = np.float64
    Wq = np.asarray(inputs["Wq"], f64); Wk = np.asarray(inputs["Wk"], f64)
    Wv = np.asarray(inputs["Wv"], f64); Wp = np.asarray(inputs["Wp"], f64)
    W1 = np.asarray(inputs["W1"], f64); W2 = np.asarray(inputs["W2"], np.float32)
    g1 = np.asarray(inputs["g1"], f64); be1 = np.asarray(inputs["be1"], f64)
    g2 = np.asarray(inputs["g2"], f64); be2 = np.asarray(inputs["be2"], f64)
    b1 = np.asarray(inputs["b1"], f64)
    shared = {
        "Wq": (WSC * g1[:, None] * Wq).astype(np.float32).astype(E4),
        "Wk": (WSC * g1[:, None] * Wk).astype(np.float32).astype(E4),
        "Wv": (WSC * g1[:, None] * Wv).astype(np.float32).astype(E4),
        "Wp": (WSC * Wp).astype(np.float32).astype(E4),
        "W1": (g2[:, None] * W1).astype(np.float32).astype(BF),
        "W2": W2.astype(BF),
        "qbias": (be1 @ Wq).astype(np.float32).reshape(NB, 128),
        "kbias": (be1 @ Wk).astype(np.float32).reshape(NB, 128),
        "vbias": (WSC * (be1 @ Wv)).astype(np.float32).reshape(1, C),
        "b1": (b1 + be2 @ W1).astype(np.float32).reshape(1, FF),
        "bp": np.asarray(inputs["bp"], np.float32).reshape(1, C),
        "b2": np.ascontiguousarray(
            np.asarray(inputs["b2"], np.float32).reshape(CCH, 128).T),
    }
    in_maps = []
    for c in range(N_CORES):
        b, par = c // 2, c % 2
        # permuted sequence: own (parity) blocks first, partner blocks after,
        # so the program is parity-independent; key positions ride as data
        gblocks = [2 * j + par for j in range(NB)] + \
                  [2 * j + (1 - par) for j in range(NB)]
        xperm = np.concatenate([x[b, g * 128:(g + 1) * 128, :] for g in gblocks], 0)
        # bp is folded into the residual rows (x2 = xown + bp + attn @ Wp)
        rows = xperm[:TOK] + np.asarray(inputs["bp"], np.float32)[None, :]
        qpos = np.stack([np.arange(g * 128, (g + 1) * 128, dtype=np.float32)
                         for g in gblocks[:NB]], 0)
        kposb = np.array([[g * 128 for g in gblocks]], dtype=np.float32)
        m = {"xfull": np.ascontiguousarray(xperm),
             "xown": np.ascontiguousarray(rows), "qpos": qpos, "kposb": kposb}
        m.update(shared)
        in_maps.append(m)
    return in_maps


def unshard_outputs(results):
    """list of per-core {'out': [TOK, C]} -> [B, T, C]"""
    out = np.zeros((B, T, C), np.float32)
    for c in range(N_CORES):
        b, par = c // 2, c % 2
        r = np.asarray(results[c]["out"])
        for j in range(NB):
            g = 2 * j + par
            out[b, g * 128:(g + 1) * 128, :] = r[j * 128:(j + 1) * 128, :]
    return out


_NC_CACHE = {}

def _get_nc():
    if "nc" not in _NC_CACHE:
        nc = bacc.Bacc("TRN2", target_bir_lowering=False, debug=False,
                       num_devices=N_CORES)
        build(nc, reps=1)
        nc.compile()
        _NC_CACHE["nc"] = nc
    return _NC_CACHE["nc"]


def kernel(**inputs):
    from concourse.bass_utils import run_bass_kernel_spmd
    nc = _get_nc()
    in_maps = shard_inputs(inputs)
    res = run_bass_kernel_spmd(nc, in_maps, core_ids=list(range(N_CORES)))
    return unshard_outputs(res.results)


# revision 55
# speedup vs baseline: 278.9712x; 1.0468x over previous
"""Self-contained Trainium2 kernel for the dense transformer block problem.

kernel(**inputs) takes the FULL inputs (as produced by the reference
setup_inputs), shards them across 8 NeuronCores (2 cores per batch element,
causal-balanced parity split of query blocks), runs a Bass/Tile SPMD kernel,
and reassembles the full [B, T, C] output.

Design (v2):
  - Wq/Wk/Wv/Wp are fp8e4m3 (host-quantized at 32x scale, LN1 gain folded in)
    and SBUF-resident; projections run as DoubleRow fp8 matmuls (2 contraction
    chunks per instruction).
  - LN1 -> transpose -> K/V flow is fused per 512-token group; Q is computed
    for the full sequence (same code path as K) and attention reads the own
    (parity) query blocks via strided views.
  - Scores run per head-PAIR: two row-tiled matmuls (contraction rows 0-63 /
    64-127 of the PE array) execute concurrently on hardware; one Exp
    activation covers both heads' score chunks.
  - V carries 32x values plus a ones column; AV accumulates [32*attn; rowsum]
    in PSUM; normalization folds the 1/32 into the per-token reciprocal
    multiply. attnT is fp8 in SBUF (no DRAM round trip), feeding a DoubleRow
    Wp projection.
  - x2 (attention residual) stays in SBUF; MLP is a single pass over bf16
    W1/W2 with bf16 ff1 activations; b2 and the final residual are folded
    into the output assembly.
"""
import sys
sys.path.insert(0, '/opt/trn_rl_repo')
import numpy as np
from contextlib import ExitStack

import concourse.bacc as bacc
import concourse.tile as tile
import concourse.mybir as mybir
from concourse.masks import make_identity

F32 = mybir.dt.float32
F32R = mybir.dt.float32r
BF16 = mybir.dt.bfloat16
F8 = mybir.dt.float8e4
AF = mybir.ActivationFunctionType
ALU = mybir.AluOpType
PM = mybir.MatmulPerfMode

B, T, C, H, DH = 4, 2048, 1024, 16, 64
N_CORES = 8
TOK = 1024          # own tokens per core
NB = TOK // 128     # 8 own query blocks
KB = T // 128       # 16 key blocks
CCH = C // 128      # 8 channel chunks
CP = CCH // 2       # 4 chunk pairs (DoubleRow)
FF = 4 * C          # 4096
FCH = FF // 128     # 32 ff chunks
EPS = 1e-5
WSC = 32.0          # fp8 weight scale

IN_NAMES = ["xfull", "xown", "qpos", "kposb", "Wq", "Wk", "Wv", "Wp", "bp",
            "W1", "W1lo", "b1", "W2", "W2lo", "b2", "qbias", "kbias", "vbias"]
W2SC = 64.0         # fp8 W2 scale (split hi/lo representation)


def build(nc, reps=1, loop_reps=None):
    """Trace the SPMD program into nc (a bacc.Bacc). Call nc.compile() after.

    Weight inputs arrive pre-folded on the host:
      Wq/Wk/Wv = fp8(32 * diag(g1) @ W);  qbias/kbias = be1 @ W
      Wp = fp8(32 * Wp);  vbias = 32 * be1 @ Wv
      W1 = bf16(diag(g2) @ W1);  b1 = b1 + be2 @ W1
      W2 = bf16(W2);  b2 arranged [128, CCH].
    """
    def din(name, shape, dt=F32):
        return nc.dram_tensor(name, shape, dt, kind="ExternalInput")

    xfull_d = din("xfull", [T, C])
    xown_d = din("xown", [TOK, C])
    qpos_d = din("qpos", [NB, 128])
    kposb_d = din("kposb", [1, KB])
    Wq_d = din("Wq", [C, C], F8); Wk_d = din("Wk", [C, C], F8)
    Wv_d = din("Wv", [C, C], F8); Wp_d = din("Wp", [C, C], F8)
    bp_d = din("bp", [1, C]); W1_d = din("W1", [C, FF], F8)
    W1lo_d = din("W1lo", [C, FF], F8); b1_d = din("b1", [1, FF])
    W2_d = din("W2", [FF, C], F8); W2lo_d = din("W2lo", [FF, C], F8)
    b2_d = din("b2", [128, CCH])
    qb_d = din("qbias", [NB, 128])   # be1 @ Wq, laid out [pair, within]
    kb_d = din("kbias", [NB, 128])   # be1 @ Wk
    vb_d = din("vbias", [1, C])      # 32 * be1 @ Wv
    out_d = nc.dram_tensor("out", [TOK, C], F32, kind="ExternalOutput")

    Wqv = Wq_d.ap().rearrange("(o p) m -> p o m", p=128)
    Wkv = Wk_d.ap().rearrange("(o p) m -> p o m", p=128)
    Wvv = Wv_d.ap().rearrange("(o p) m -> p o m", p=128)
    Wpv = Wp_d.ap().rearrange("(o p) m -> p o m", p=128)
    W1v = W1_d.ap().rearrange("(o p) m -> o p m", p=128)
    W1lov = W1lo_d.ap().rearrange("(o p) m -> o p m", p=128)
    W2v = W2_d.ap().rearrange("(o p) m -> p o m", p=128)  # [128, 32, 1024]
    W2lov = W2lo_d.ap().rearrange("(o p) m -> p o m", p=128)
    xf = xfull_d.ap()
    xo = xown_d.ap()

    env = locals()
    if loop_reps is not None:
        with tile.TileContext(nc) as tc:
            with tc.For_i(0, loop_reps, 1):
                with ExitStack() as top:
                    _build_body(nc, tc, top, env)
    else:
        for _rep in range(reps):
            with tile.TileContext(nc) as tc, ExitStack() as top:
                _build_body(nc, tc, top, env)
    return IN_NAMES


def _build_body(nc, tc, top, env):
    (xfull_d, xown_d, qpos_d, kposb_d, Wq_d, Wk_d, Wv_d, Wp_d, bp_d, W1_d,
     b1_d, W2_d, b2_d, qb_d, kb_d, vb_d, out_d, Wqv, Wkv, Wvv, Wpv, W1v, W1lov,
     W2v, W2lov, xf, xo) = (
        env[k] for k in ["xfull_d", "xown_d", "qpos_d", "kposb_d", "Wq_d",
                         "Wk_d", "Wv_d", "Wp_d", "bp_d", "W1_d", "b1_d", "W2_d",
                         "b2_d", "qb_d", "kb_d", "vb_d", "out_d", "Wqv", "Wkv",
                         "Wvv", "Wpv", "W1v", "W1lov", "W2v", "W2lov", "xf",
                         "xo"])

    const = top.enter_context(tc.tile_pool(name="const", bufs=1))
    identf = const.tile([128, 128], F32)
    make_identity(nc, identf[:])
    identb = const.tile([128, 128], BF16)
    nc.vector.tensor_copy(identb[:], identf[:])
    eps_t = const.tile([128, 1], F32)
    nc.vector.memset(eps_t[:], EPS)

    def ln_stats(pool, x_ap):
        n = x_ap.shape[-1] // 512
        xg = x_ap.rearrange("p (n f) -> p n f", f=512)
        stats = pool.tile([128, n, 6], F32, tag="ln_stats")
        mv = pool.tile([128, 2], F32, tag="ln_mv")
        for i in range(n):
            nc.vector.bn_stats(stats[:, i], xg[:, i])
        nc.vector.bn_aggr(mv[:], stats[:])
        rstd = pool.tile([128, 1], F32, tag="ln_rstd")
        nc.scalar.activation(rstd[:], mv[:, 1:2], AF.Sqrt, bias=eps_t[:])
        nc.vector.reciprocal(rstd[:], rstd[:])
        return mv[:, 0:1], rstd

    def ln_apply(pool, out_ap, x_ap, mean, rstd):
        # out = (x - mu) * rstd on ACT: Identity(x * rstd + (-mu * rstd))
        nmr = pool.tile([128, 1], F32, tag="ln_nmr")
        nc.vector.tensor_scalar(nmr[:], mean, rstd[:], -1.0,
                                op0=ALU.mult, op1=ALU.mult)
        nc.scalar.activation(out_ap, x_ap, AF.Identity,
                             bias=nmr[:], scale=rstd[:])

    # ---------------- resident pools ----------------
    # left side: pools that live to the end of the body (LIFO close order)
    es_wp = ExitStack()
    wpp = es_wp.enter_context(tc.tile_pool(name="wpp", bufs=1))
    Wp_sb = wpp.tile([128, CCH, C], F8)
    es_at = ExitStack()
    atp = es_at.enter_context(tc.tile_pool(name="attnT", bufs=1))
    attnT8 = atp.tile([128, CCH, TOK], F8)

    # right side: big transients, ordered by lifetime (longest first)
    es_kqv = ExitStack()
    kqvp = es_kqv.enter_context(tc.tile_pool(name="kqv", bufs=1, side="right"))
    KT = kqvp.tile([128, CCH, T], BF16)
    QTf = kqvp.tile([128, CCH, TOK], BF16)
    V_sb = kqvp.tile([128, KB, H, 65], BF16)
    ones_b = kqvp.tile([128, 1], BF16)
    nc.vector.memset(ones_b[:], 1.0)
    nc.vector.tensor_copy(V_sb[:, :, :, 64:65],
                          ones_b[:, 0:1, None, None].to_broadcast([128, KB, H, 1]))

    es_wqkv = ExitStack()
    wqkv = es_wqkv.enter_context(tc.tile_pool(name="wqkv", bufs=1, side="right"))
    Wq_sb = wqkv.tile([128, CCH, C], F8)
    Wk_sb = wqkv.tile([128, CCH, C], F8)
    Wv_sb = wqkv.tile([128, CCH, C], F8)
    nc.sync.dma_start(Wv_sb[:], Wvv)
    nc.sync.dma_start(Wk_sb[:], Wkv)
    nc.sync.dma_start(Wq_sb[:], Wqv)

    es_h = ExitStack()
    hp_ = es_h.enter_context(tc.tile_pool(name="hT8", bufs=1, side="right"))
    hT8 = hp_.tile([128, CCH, T], F8)

    # ============ Phase AB: LN1 + V/K/Q per 512-token group ============
    with tc.tile_pool(name="stAB", bufs=3) as stAB, \
         tc.tile_pool(name="stABc", bufs=1) as stABc, \
         tc.tile_pool(name="stAB_t_ps", bufs=3, space="PSUM") as psT, \
         tc.tile_pool(name="stAB_v_ps", bufs=1, space="PSUM") as psV, \
         tc.tile_pool(name="stAB_k_ps", bufs=2, space="PSUM") as psK:
        vb_b = stABc.tile([128, C], F32)
        nc.sync.dma_start(vb_b[:], vb_d.ap().to_broadcast([128, C]))
        kb_sb = stABc.tile([128, NB], F32)
        nc.sync.dma_start(kb_sb[:], kb_d.ap().rearrange("o p -> p o"))
        qb_sb = stABc.tile([128, NB], F32)
        nc.sync.dma_start(qb_sb[:], qb_d.ap().rearrange("o p -> p o"))
        for g in range(T // 512):
            for tb4 in range(4):
                tb = g * 4 + tb4
                x_t = stAB.tile([128, C], F32, tag="x_t")
                nc.sync.dma_start(x_t[:], xf[tb * 128:(tb + 1) * 128, :])
                mean, rstd = ln_stats(stAB, x_t[:])
                hrow = stAB.tile([128, C], BF16, tag="hrow")
                ln_apply(stAB, hrow[:], x_t[:], mean, rstd)
                for cc in range(CCH):
                    pt = psT.tile([128, 128], BF16, tag="psT_t")
                    nc.tensor.transpose(pt[:], hrow[:, cc * 128:(cc + 1) * 128],
                                        identb[:])
                    eng = nc.scalar.copy if cc % 2 == 0 else nc.vector.tensor_copy
                    eng(hT8[:, cc, tb * 128:(tb + 1) * 128], pt[:])
                # V for this token block: stationary hT8(tb), moving Wv
                pv = psV.tile([128, 2, 512], F32, tag="pv")
                for cp in range(CP):
                    for grp in range(2):
                        nc.tensor.matmul(
                            pv[:, grp], hT8[:, 2 * cp:2 * cp + 2,
                                            tb * 128:(tb + 1) * 128],
                            Wv_sb[:, 2 * cp:2 * cp + 2, grp * 512:(grp + 1) * 512],
                            start=(cp == 0), stop=(cp == CP - 1),
                            perf_mode=PM.DoubleRow)
                for grp in range(2):
                    nc.vector.tensor_tensor(
                        V_sb[:, tb, grp * 8:(grp + 1) * 8, 0:64],
                        pv[:, grp].rearrange("p (h d) -> p h d", d=64),
                        vb_b[:, grp * 512:(grp + 1) * 512].rearrange(
                            "p (h d) -> p h d", d=64),
                        ALU.add)
            # K and Q (full sequence) for this 512-token group
            for pair in range(CCH):
                pk = psK.tile([128, 512], F32, tag="pk")
                for cp in range(CP):
                    nc.tensor.matmul(
                        pk[:], Wk_sb[:, 2 * cp:2 * cp + 2, pair * 128:(pair + 1) * 128],
                        hT8[:, 2 * cp:2 * cp + 2, g * 512:(g + 1) * 512],
                        start=(cp == 0), stop=(cp == CP - 1), perf_mode=PM.DoubleRow)
                nc.scalar.activation(KT[:, pair, g * 512:(g + 1) * 512], pk[:],
                                     AF.Identity, bias=kb_sb[:, pair:pair + 1],
                                     scale=1.0 / WSC)
                if g < TOK // 512:  # own (first) tokens only
                    pq = psK.tile([128, 512], F32, tag="pk")
                    for cp in range(CP):
                        nc.tensor.matmul(
                            pq[:], Wq_sb[:, 2 * cp:2 * cp + 2,
                                         pair * 128:(pair + 1) * 128],
                            hT8[:, 2 * cp:2 * cp + 2, g * 512:(g + 1) * 512],
                            start=(cp == 0), stop=(cp == CP - 1),
                            perf_mode=PM.DoubleRow)
                    nc.scalar.activation(QTf[:, pair, g * 512:(g + 1) * 512], pq[:],
                                         AF.Identity, bias=qb_sb[:, pair:pair + 1],
                                         scale=1.0 / WSC)
    es_h.close()    # free hT8
    es_wqkv.close() # free Wq/Wk/Wv

    # ---------- mask constants (key positions are per-core data) ----------
    es_mask = ExitStack()
    maskp = es_mask.enter_context(tc.tile_pool(name="maskp", bufs=1, side="right"))
    negm = maskp.tile([128, KB, 128], BF16)   # -1e9 where masked, else 0
    with tc.tile_pool(name="mtmp", bufs=1) as mtmp:
        pi_i = mtmp.tile([128, 1], mybir.dt.int32)
        nc.gpsimd.iota(pi_i[:], pattern=[[1, 1]], base=0, channel_multiplier=1)
        pi_f = mtmp.tile([128, 1], F32)
        nc.vector.tensor_copy(pi_f[:], pi_i[:])
        kpb = mtmp.tile([128, KB], F32)
        nc.sync.dma_start(kpb[:], kposb_d.ap().to_broadcast([128, KB]))
        kp_f = mtmp.tile([128, KB], F32)
        nc.vector.tensor_scalar(kp_f[:], kpb[:], pi_f[:], None, op0=ALU.add)
        qb = mtmp.tile([128, NB, 128], F32)
        for j in range(NB):
            nc.sync.dma_start(qb[:, j], qpos_d.ap()[j:j + 1, :].to_broadcast([128, 128]))
        for k in range(KB):
            jmin = k if k < NB else k - NB
            # negm[p_key, f_q] = -1e9 * (qpos_jmin[f] < keypos(block k)[p])
            nc.vector.tensor_scalar(
                negm[:, k], qb[:, jmin], kp_f[:, k:k + 1], -1e9,
                op0=ALU.is_lt, op1=ALU.mult)

    # ============ Phase C: attention (per head pair) ============
    nc.sync.dma_start(Wp_sb[:], Wpv)   # overlaps with attention
    with tc.tile_pool(name="stC", bufs=3) as stC, \
         tc.tile_pool(name="stC_att_ps", bufs=1, space="PSUM") as psCa, \
         tc.tile_pool(name="stC_s_ps", bufs=2, space="PSUM") as psCs, \
         tc.tile_pool(name="stC_t_ps", bufs=2, space="PSUM") as psCt:
        for hp in range(CCH):  # head pair = channel pair chunk
            for qp in range(2):  # query half: own cols [qp*512, qp*512+512)
                qbase = qp * 512
                klist = [k for k in range(KB)
                         if (k if k < NB else k - NB) * 128 < qbase + 512]
                ps_att = psCa.tile([128, 2, 512], F32, tag="ps_att")
                for ki, k in enumerate(klist):
                    jmin = k if k < NB else k - NB
                    q0 = jmin * 128
                    qlo = max(q0, qbase)
                    nq = qbase + 512 - qlo
                    weiT2 = stC.tile([128, 2, 512], BF16, tag="weiT")
                    ps_s = psCs.tile([128, 2, 512], F32, tag="ps_s")
                    diag = qlo == q0
                    for t in range(2):
                        nc.tensor.matmul(
                            ps_s[:, t, 0:nq],
                            KT[64 * t:64 * t + 64, hp, k * 128:(k + 1) * 128],
                            QTf[64 * t:64 * t + 64, hp, qlo:qlo + nq],
                            start=True, stop=not diag)
                        if diag:
                            # causal mask: accumulate -1e9 into the diagonal
                            # block via PE (identity @ negm)
                            nc.tensor.matmul(
                                ps_s[:, t, 0:128], identb[:], negm[:, k],
                                start=False, stop=True)
                    nc.scalar.activation(weiT2[:, :, 0:nq],
                                         ps_s[:, :, 0:nq], AF.Exp, scale=0.125)
                    # AV: accumulate [32*attn ; rowsum] per head
                    for t in range(2):
                        nc.tensor.matmul(
                            ps_att[0:65, t, qlo - qbase:512],
                            V_sb[:, k, 2 * hp + t, :],
                            weiT2[:, t, 0:nq],
                            start=(ki == 0), stop=(ki == len(klist) - 1))
                # normalize + transpose into attnT8 (copies on DVE: keep the
                # Act engine exp-only during attention to avoid table thrash)
                for t in range(2):
                    sb_at = stC.tile([128, 512], BF16, tag="sb_at")
                    nc.vector.tensor_copy(sb_at[0:65, :], ps_att[0:65, t, :])
                    for jj in range(4):
                        j = qp * 4 + jj
                        pt1 = psCt.tile([128, 128], BF16, tag="ptn")
                        nc.tensor.transpose(pt1[:], sb_at[:, jj * 128:(jj + 1) * 128],
                                            identb[:])
                        recip = stC.tile([128, 1], F32, tag="recip")
                        nc.vector.reciprocal(recip[:], pt1[:, 64:65])
                        attn_j = stC.tile([128, 64], BF16, tag="attn_j")
                        nc.vector.tensor_scalar(attn_j[:], pt1[:, 0:64], recip[:],
                                                1.0 / WSC, op0=ALU.mult, op1=ALU.mult)
                        pt2 = psCt.tile([128, 128], BF16, tag="ptn")
                        nc.tensor.transpose(pt2[0:64, :], attn_j[:], identb[:])
                        nc.vector.tensor_copy(
                            attnT8[64 * t:64 * t + 64, hp, j * 128:(j + 1) * 128],
                            pt2[0:64, :])
    es_mask.close()
    es_kqv.close()   # free KT, QTf, V

    # ============ Phase D: Wp proj + residual + LN2 ============
    es_x2 = ExitStack()
    x2p = es_x2.enter_context(tc.tile_pool(name="x2h2", bufs=1))
    x2_sb = x2p.tile([128, NB, C], F32)     # token-major
    h2Th = x2p.tile([128, CCH, TOK], F8)    # channel-major fp8 hi (for MLP)
    h2Tl = x2p.tile([128, CCH, TOK], F8)    # fp8 residual (lo)
    with tc.tile_pool(name="stD", bufs=2) as stD, \
         tc.tile_pool(name="stD_c", bufs=1) as stDc, \
         tc.tile_pool(name="stD_ps", bufs=2, space="PSUM") as psD, \
         tc.tile_pool(name="stD_t_ps", bufs=3, space="PSUM") as psDt:
        for nt in range(TOK // 512):
            pT_sb = stD.tile([128, CCH, 512], BF16, tag="pT_sb")
            for co in range(CCH):
                pp = psD.tile([128, 512], F32, tag="pp")
                for cp in range(CP):
                    nc.tensor.matmul(
                        pp[:], Wp_sb[:, 2 * cp:2 * cp + 2, co * 128:(co + 1) * 128],
                        attnT8[:, 2 * cp:2 * cp + 2, nt * 512:(nt + 1) * 512],
                        start=(cp == 0), stop=(cp == CP - 1), perf_mode=PM.DoubleRow)
                nc.scalar.activation(pT_sb[:, co], pp[:], AF.Identity,
                                     scale=1.0 / WSC)
            for sub in range(4):
                tb = nt * 4 + sub
                xo_t = stD.tile([128, C], F32, tag="xo_t")
                nc.sync.dma_start(xo_t[:], xo[tb * 128:(tb + 1) * 128, :])
                for co in range(CCH):
                    ptd = psDt.tile([128, 128], BF16, tag="ptd")
                    nc.tensor.transpose(ptd[:], pT_sb[:, co, sub * 128:(sub + 1) * 128],
                                        identb[:])
                    nc.vector.tensor_tensor(x2_sb[:, tb, co * 128:(co + 1) * 128],
                                            ptd[:], xo_t[:, co * 128:(co + 1) * 128],
                                            ALU.add)
                # LN2
                mean, rstd = ln_stats(stD, x2_sb[:, tb, :])
                h2row = stD.tile([128, C], BF16, tag="h2row")
                ln_apply(stD, h2row[:], x2_sb[:, tb, :], mean, rstd)
                for cc in range(CCH):
                    pt = psDt.tile([128, 128], BF16, tag="ptd2")
                    nc.tensor.transpose(pt[:], h2row[:, cc * 128:(cc + 1) * 128],
                                        identb[:])
                    hi = h2Th[:, cc, tb * 128:(tb + 1) * 128]
                    eng = nc.scalar.copy if cc % 2 == 0 else nc.vector.tensor_copy
                    eng(hi, pt[:])
                    nc.vector.tensor_tensor(
                        h2Tl[:, cc, tb * 128:(tb + 1) * 128], pt[:], hi,
                        ALU.subtract)

    # ============ Phase E: MLP (single pass, bf16) ============
    es_ff = ExitStack()
    ffp = es_ff.enter_context(tc.tile_pool(name="ff1T", bufs=1))
    ff1Th = ffp.tile([128, FCH, TOK], F8)   # fp8 hi part of relu(ff1)
    ff1Tl = ffp.tile([128, FCH, TOK], F8)   # fp8 residual (lo) part
    with tc.tile_pool(name="stE_c", bufs=1) as stEc:
        b1p = stEc.tile([128, FCH], F32)
        nc.sync.dma_start(b1p[:], b1_d.ap().rearrange("x (o p) -> p (x o)", p=128))
        b2p = stEc.tile([128, CCH], F32)
        nc.sync.dma_start(b2p[:], b2_d.ap())
        with tc.tile_pool(name="stE1", bufs=2) as stE1, \
             tc.tile_pool(name="stE1_ps", bufs=2, space="PSUM") as psE1, \
             tc.tile_pool(name="stE2", bufs=2) as stE2, \
             tc.tile_pool(name="stE2_ps", bufs=2, space="PSUM") as psE2, \
             tc.tile_pool(name="stE2_t_ps", bufs=2, space="PSUM") as psEt:
            for fog in range(8):
                w1g = stE1.tile([128, CCH, 512], F8, tag="w1g")
                nc.sync.dma_start(
                    w1g[:], W1v.transpose([1, 0, 2])[:, :, fog * 512:(fog + 1) * 512])
                w1l = stE1.tile([128, CCH, 512], F8, tag="w1l")
                nc.sync.dma_start(
                    w1l[:], W1lov.transpose([1, 0, 2])[:, :, fog * 512:(fog + 1) * 512])
                for f4 in range(4):
                    fo = fog * 4 + f4
                    for nt in range(TOK // 512):
                        pf = psE1.tile([128, 512], F32, tag="pf")
                        slt = slice(nt * 512, (nt + 1) * 512)
                        slf = slice(f4 * 128, (f4 + 1) * 128)
                        for cp in range(CP):
                            sl2 = slice(2 * cp, 2 * cp + 2)
                            nc.tensor.matmul(pf[:], w1g[:, sl2, slf],
                                             h2Th[:, sl2, slt],
                                             start=(cp == 0), stop=False,
                                             perf_mode=PM.DoubleRow)
                            nc.tensor.matmul(pf[:], w1l[:, sl2, slf],
                                             h2Th[:, sl2, slt],
                                             start=False, stop=False,
                                             perf_mode=PM.DoubleRow)
                            nc.tensor.matmul(pf[:], w1g[:, sl2, slf],
                                             h2Tl[:, sl2, slt],
                                             start=False, stop=(cp == CP - 1),
                                             perf_mode=PM.DoubleRow)
                        ff1b = stE1.tile([128, 512], BF16, tag="ff1b")
                        nc.scalar.activation(ff1b[:], pf[:], AF.Relu,
                                             bias=b1p[:, fo:fo + 1],
                                             scale=1.0 / WSC)
                        hi = ff1Th[:, fo, nt * 512:(nt + 1) * 512]
                        nc.vector.tensor_copy(hi, ff1b[:])
                        nc.vector.tensor_tensor(
                            ff1Tl[:, fo, nt * 512:(nt + 1) * 512],
                            ff1b[:], hi, ALU.subtract)
            for co in range(CCH):
                w2c = stE2.tile([128, FCH, 128], F8, tag="w2c")
                nc.sync.dma_start(w2c[:], W2v[:, :, co * 128:(co + 1) * 128])
                w2l = stE2.tile([128, FCH, 128], F8, tag="w2l")
                nc.sync.dma_start(w2l[:], W2lov[:, :, co * 128:(co + 1) * 128])
                for nt in range(TOK // 512):
                    p2 = psE2.tile([128, 512], F32, tag="p2")
                    for fp in range(FCH // 2):
                        sl2 = slice(2 * fp, 2 * fp + 2)
                        slt = slice(nt * 512, (nt + 1) * 512)
                        nc.tensor.matmul(p2[:], w2c[:, sl2], ff1Th[:, sl2, slt],
                                         start=(fp == 0), stop=False,
                                         perf_mode=PM.DoubleRow)
                        nc.tensor.matmul(p2[:], w2l[:, sl2], ff1Th[:, sl2, slt],
                                         start=False, stop=False,
                                         perf_mode=PM.DoubleRow)
                        nc.tensor.matmul(p2[:], w2c[:, sl2], ff1Tl[:, sl2, slt],
                                         start=False, stop=(fp == FCH // 2 - 1),
                                         perf_mode=PM.DoubleRow)
                    ff2_c = stE2.tile([128, 512], BF16, tag="ff2_c")
                    nc.scalar.activation(ff2_c[:], p2[:], AF.Identity,
                                         bias=b2p[:, co:co + 1], scale=1.0 / W2SC)
                    for sub in range(4):
                        tb = nt * 4 + sub
                        ptf = psEt.tile([128, 128], BF16, tag="ptf")
                        nc.tensor.transpose(ptf[:], ff2_c[:, sub * 128:(sub + 1) * 128],
                                            identb[:])
                        out_c = stE2.tile([128, 128], F32, tag="out_c")
                        nc.vector.tensor_tensor(
                            out_c[:], ptf[:], x2_sb[:, tb, co * 128:(co + 1) * 128],
                            ALU.add)
                        nc.sync.dma_start(
                            out_d.ap()[tb * 128:(tb + 1) * 128,
                                       co * 128:(co + 1) * 128], out_c[:])
    es_ff.close()
    es_x2.close()
    es_at.close()   # free attnT8
    es_wp.close()   # free Wp


def make_nc():
    nc = bacc.Bacc("TRN2", target_bir_lowering=False, debug=False,
                   num_devices=N_CORES)
    build(nc)
    nc.compile()
    return nc


def shard_inputs(inputs):
    """Full inputs dict -> list of 8 per-core in_maps.

    Folds LN1 gain/bias into Wq/Wk/Wv (weights scaled by g1 per input channel,
    be1 contribution becomes an additive bias on q/k/v) and LN2's into W1/b1.
    Wq/Wk/Wv/Wp are fp8 at 32x scale; W1/W2 are bf16.
    """
    E4 = mybir.dt.np(F8)
    BF = mybir.dt.np(BF16)
    x = np.asarray(inputs["x"], np.float32)
    assert x.shape == (B, T, C)
    f64 = np.float64
    Wq = np.asarray(inputs["Wq"], f64); Wk = np.asarray(inputs["Wk"], f64)
    Wv = np.asarray(inputs["Wv"], f64); Wp = np.asarray(inputs["Wp"], f64)
    W1 = np.asarray(inputs["W1"], f64); W2 = np.asarray(inputs["W2"], np.float32)
    g1 = np.asarray(inputs["g1"], f64); be1 = np.asarray(inputs["be1"], f64)
    g2 = np.asarray(inputs["g2"], f64); be2 = np.asarray(inputs["be2"], f64)
    b1 = np.asarray(inputs["b1"], f64)
    shared = {
        "Wq": (WSC * g1[:, None] * Wq).astype(np.float32).astype(E4),
        "Wk": (WSC * g1[:, None] * Wk).astype(np.float32).astype(E4),
        "Wv": (WSC * g1[:, None] * Wv).astype(np.float32).astype(E4),
        "Wp": (WSC * Wp).astype(np.float32).astype(E4),
        "W1": (WSC * g2[:, None] * W1).astype(np.float32).astype(E4),
        "W1lo": ((WSC * g2[:, None] * W1).astype(np.float32)
                 - (WSC * g2[:, None] * W1).astype(np.float32).astype(E4)
                   .astype(np.float32)).astype(E4),
        "W2": (W2SC * W2).astype(E4),
        "W2lo": (W2SC * W2 - (W2SC * W2).astype(E4).astype(np.float32)).astype(E4),
        "qbias": (be1 @ Wq).astype(np.float32).reshape(NB, 128),
        "kbias": (be1 @ Wk).astype(np.float32).reshape(NB, 128),
        "vbias": (WSC * (be1 @ Wv)).astype(np.float32).reshape(1, C),
        "b1": (b1 + be2 @ W1).astype(np.float32).reshape(1, FF),
        "bp": np.asarray(inputs["bp"], np.float32).reshape(1, C),
        "b2": np.ascontiguousarray(
            np.asarray(inputs["b2"], np.float32).reshape(CCH, 128).T),
    }
    in_maps = []
    for c in range(N_CORES):
        b, par = c // 2, c % 2
        # permuted sequence: own (parity) blocks first, partner blocks after,
        # so the program is parity-independent; key positions ride as data
        gblocks = [2 * j + par for j in range(NB)] + \
                  [2 * j + (1 - par) for j in range(NB)]
        xperm = np.concatenate([x[b, g * 128:(g + 1) * 128, :] for g in gblocks], 0)
        # bp is folded into the residual rows (x2 = xown + bp + attn @ Wp)
        rows = xperm[:TOK] + np.asarray(inputs["bp"], np.float32)[None, :]
        qpos = np.stack([np.arange(g * 128, (g + 1) * 128, dtype=np.float32)
                         for g in gblocks[:NB]], 0)
        kposb = np.array([[g * 128 for g in gblocks]], dtype=np.float32)
        m = {"xfull": np.ascontiguousarray(xperm),
             "xown": np.ascontiguousarray(rows), "qpos": qpos, "kposb": kposb}
        m.update(shared)
        in_maps.append(m)
    return in_maps


def unshard_outputs(results):
    """list of per-core {'out': [TOK, C]} -> [B, T, C]"""
    out = np.zeros((B, T, C), np.float32)
    for c in range(N_CORES):
        b, par = c // 2, c % 2
        r = np.asarray(results[c]["out"])
        for j in range(NB):
            g = 2 * j + par
            out[b, g * 128:(g + 1) * 128, :] = r[j * 128:(j + 1) * 128, :]
    return out


_NC_CACHE = {}

def _get_nc():
    if "nc" not in _NC_CACHE:
        nc = bacc.Bacc("TRN2", target_bir_lowering=False, debug=False,
                       num_devices=N_CORES)
        build(nc, reps=1)
        nc.compile()
        _NC_CACHE["nc"] = nc
    return _NC_CACHE["nc"]


def kernel(**inputs):
    from concourse.bass_utils import run_bass_kernel_spmd
    nc = _get_nc()
    in_maps = shard_inputs(inputs)
    res = run_bass_kernel_spmd(nc, in_maps, core_ids=list(range(N_CORES)))
    return unshard_outputs(res.results)


# revision 66
# speedup vs baseline: 285.0891x; 1.0219x over previous
"""Self-contained Trainium2 kernel for the dense transformer block problem.

kernel(**inputs) takes the FULL inputs (as produced by the reference
setup_inputs), shards them across 8 NeuronCores (2 cores per batch element,
causal-balanced parity split of query blocks), runs a Bass/Tile SPMD kernel,
and reassembles the full [B, T, C] output.

Design (v2):
  - Wq/Wk/Wv/Wp are fp8e4m3 (host-quantized at 32x scale, LN1 gain folded in)
    and SBUF-resident; projections run as DoubleRow fp8 matmuls (2 contraction
    chunks per instruction).
  - LN1 -> transpose -> K/V flow is fused per 512-token group; Q is computed
    for the full sequence (same code path as K) and attention reads the own
    (parity) query blocks via strided views.
  - Scores run per head-PAIR: two row-tiled matmuls (contraction rows 0-63 /
    64-127 of the PE array) execute concurrently on hardware; one Exp
    activation covers both heads' score chunks.
  - V carries 32x values plus a ones column; AV accumulates [32*attn; rowsum]
    in PSUM; normalization folds the 1/32 into the per-token reciprocal
    multiply. attnT is fp8 in SBUF (no DRAM round trip), feeding a DoubleRow
    Wp projection.
  - x2 (attention residual) stays in SBUF; MLP is a single pass over bf16
    W1/W2 with bf16 ff1 activations; b2 and the final residual are folded
    into the output assembly.
"""
import sys
sys.path.insert(0, '/opt/trn_rl_repo')
import numpy as np
from contextlib import ExitStack

import concourse.bacc as bacc
import concourse.tile as tile
import concourse.mybir as mybir
from concourse.masks import make_identity

F32 = mybir.dt.float32
F32R = mybir.dt.float32r
BF16 = mybir.dt.bfloat16
F8 = mybir.dt.float8e4
AF = mybir.ActivationFunctionType
ALU = mybir.AluOpType
PM = mybir.MatmulPerfMode

B, T, C, H, DH = 4, 2048, 1024, 16, 64
N_CORES = 8
TOK = 1024          # own tokens per core
NB = TOK // 128     # 8 own query blocks
KB = T // 128       # 16 key blocks
CCH = C // 128      # 8 channel chunks
CP = CCH // 2       # 4 chunk pairs (DoubleRow)
FF = 4 * C          # 4096
FCH = FF // 128     # 32 ff chunks
EPS = 1e-5
WSC = 32.0          # fp8 weight scale

IN_NAMES = ["xfull", "xown", "qpos", "kposb", "Wq", "Wk", "Wv", "Wp", "bp",
            "W1", "W1lo", "b1", "W2", "W2lo", "b2", "qbias", "kbias", "vbias"]
W2SC = 64.0         # fp8 W2 scale (split hi/lo representation)


def build(nc, reps=1, loop_reps=None):
    """Trace the SPMD program into nc (a bacc.Bacc). Call nc.compile() after.

    Weight inputs arrive pre-folded on the host:
      Wq/Wk/Wv = fp8(32 * diag(g1) @ W);  qbias/kbias = be1 @ W
      Wp = fp8(32 * Wp);  vbias = 32 * be1 @ Wv
      W1 = bf16(diag(g2) @ W1);  b1 = b1 + be2 @ W1
      W2 = bf16(W2);  b2 arranged [128, CCH].
    """
    def din(name, shape, dt=F32):
        return nc.dram_tensor(name, shape, dt, kind="ExternalInput")

    xfull_d = din("xfull", [T, C])
    xown_d = din("xown", [TOK, C])
    qpos_d = din("qpos", [NB, 128])
    kposb_d = din("kposb", [1, KB])
    Wq_d = din("Wq", [C, C], F8); Wk_d = din("Wk", [C, C], F8)
    Wv_d = din("Wv", [C, C], F8); Wp_d = din("Wp", [C, C], F8)
    bp_d = din("bp", [1, C]); W1_d = din("W1", [C, FF], F8)
    W1lo_d = din("W1lo", [C, FF], F8); b1_d = din("b1", [1, FF])
    W2_d = din("W2", [FF, C], F8); W2lo_d = din("W2lo", [FF, C], F8)
    b2_d = din("b2", [128, CCH])
    qb_d = din("qbias", [NB, 128])   # be1 @ Wq, laid out [pair, within]
    kb_d = din("kbias", [NB, 128])   # be1 @ Wk
    vb_d = din("vbias", [1, C])      # 32 * be1 @ Wv
    out_d = nc.dram_tensor("out", [TOK, C], F32, kind="ExternalOutput")

    Wqv = Wq_d.ap().rearrange("(o p) m -> p o m", p=128)
    Wkv = Wk_d.ap().rearrange("(o p) m -> p o m", p=128)
    Wvv = Wv_d.ap().rearrange("(o p) m -> p o m", p=128)
    Wpv = Wp_d.ap().rearrange("(o p) m -> p o m", p=128)
    W1v = W1_d.ap().rearrange("(o p) m -> o p m", p=128)
    W1lov = W1lo_d.ap().rearrange("(o p) m -> o p m", p=128)
    W2v = W2_d.ap().rearrange("(o p) m -> p o m", p=128)  # [128, 32, 1024]
    W2lov = W2lo_d.ap().rearrange("(o p) m -> p o m", p=128)
    xf = xfull_d.ap()
    xo = xown_d.ap()

    env = locals()
    if loop_reps is not None:
        with tile.TileContext(nc) as tc:
            with tc.For_i(0, loop_reps, 1):
                with ExitStack() as top:
                    _build_body(nc, tc, top, env)
    else:
        for _rep in range(reps):
            with tile.TileContext(nc) as tc, ExitStack() as top:
                _build_body(nc, tc, top, env)
    return IN_NAMES


def _build_body(nc, tc, top, env):
    (xfull_d, xown_d, qpos_d, kposb_d, Wq_d, Wk_d, Wv_d, Wp_d, bp_d, W1_d,
     b1_d, W2_d, b2_d, qb_d, kb_d, vb_d, out_d, Wqv, Wkv, Wvv, Wpv, W1v, W1lov,
     W2v, W2lov, xf, xo) = (
        env[k] for k in ["xfull_d", "xown_d", "qpos_d", "kposb_d", "Wq_d",
                         "Wk_d", "Wv_d", "Wp_d", "bp_d", "W1_d", "b1_d", "W2_d",
                         "b2_d", "qb_d", "kb_d", "vb_d", "out_d", "Wqv", "Wkv",
                         "Wvv", "Wpv", "W1v", "W1lov", "W2v", "W2lov", "xf",
                         "xo"])

    const = top.enter_context(tc.tile_pool(name="const", bufs=1))
    identf = const.tile([128, 128], F32)
    make_identity(nc, identf[:])
    identb = const.tile([128, 128], BF16)
    nc.vector.tensor_copy(identb[:], identf[:])
    eps_t = const.tile([128, 1], F32)
    nc.vector.memset(eps_t[:], EPS)

    def ln_stats(pool, x_ap):
        n = x_ap.shape[-1] // 512
        xg = x_ap.rearrange("p (n f) -> p n f", f=512)
        stats = pool.tile([128, n, 6], F32, tag="ln_stats")
        mv = pool.tile([128, 2], F32, tag="ln_mv")
        for i in range(n):
            nc.vector.bn_stats(stats[:, i], xg[:, i])
        nc.vector.bn_aggr(mv[:], stats[:])
        rstd = pool.tile([128, 1], F32, tag="ln_rstd")
        nc.scalar.activation(rstd[:], mv[:, 1:2], AF.Sqrt, bias=eps_t[:])
        nc.vector.reciprocal(rstd[:], rstd[:])
        return mv[:, 0:1], rstd

    def ln_apply(pool, out_ap, x_ap, mean, rstd):
        # out = (x - mu) * rstd on ACT: Identity(x * rstd + (-mu * rstd))
        nmr = pool.tile([128, 1], F32, tag="ln_nmr")
        nc.vector.tensor_scalar(nmr[:], mean, rstd[:], -1.0,
                                op0=ALU.mult, op1=ALU.mult)
        nc.scalar.activation(out_ap, x_ap, AF.Identity,
                             bias=nmr[:], scale=rstd[:])

    # ---------------- resident pools ----------------
    # left side: pools that live to the end of the body (LIFO close order)
    es_wp = ExitStack()
    wpp = es_wp.enter_context(tc.tile_pool(name="wpp", bufs=1))
    Wp_sb = wpp.tile([128, CCH, C], F8)
    es_at = ExitStack()
    atp = es_at.enter_context(tc.tile_pool(name="attnT", bufs=1))
    attnT8 = atp.tile([128, CCH, TOK], F8)

    # right side: big transients, ordered by lifetime (longest first)
    es_kqv = ExitStack()
    kqvp = es_kqv.enter_context(tc.tile_pool(name="kqv", bufs=1, side="right"))
    KT = kqvp.tile([128, CCH, T], BF16)
    QTf = kqvp.tile([128, CCH, TOK], BF16)
    V_sb = kqvp.tile([128, KB, H, 65], BF16)
    ones_b = kqvp.tile([128, 1], BF16)
    nc.vector.memset(ones_b[:], 1.0)
    nc.vector.tensor_copy(V_sb[:, :, :, 64:65],
                          ones_b[:, 0:1, None, None].to_broadcast([128, KB, H, 1]))

    es_wqkv = ExitStack()
    wqkv = es_wqkv.enter_context(tc.tile_pool(name="wqkv", bufs=1, side="right"))
    Wq_sb = wqkv.tile([128, CCH, C], F8)
    Wk_sb = wqkv.tile([128, CCH, C], F8)
    Wv_sb = wqkv.tile([128, CCH, C], F8)
    nc.sync.dma_start(Wv_sb[:], Wvv)
    nc.sync.dma_start(Wk_sb[:], Wkv)
    nc.sync.dma_start(Wq_sb[:], Wqv)

    es_h = ExitStack()
    hp_ = es_h.enter_context(tc.tile_pool(name="hT8", bufs=1, side="right"))
    hT8 = hp_.tile([128, CCH, T], F8)

    # ============ Phase AB: LN1 + V/K/Q per 512-token group ============
    with tc.tile_pool(name="stAB", bufs=3) as stAB, \
         tc.tile_pool(name="stABc", bufs=1) as stABc, \
         tc.tile_pool(name="stAB_t_ps", bufs=3, space="PSUM") as psT, \
         tc.tile_pool(name="stAB_v_ps", bufs=1, space="PSUM") as psV, \
         tc.tile_pool(name="stAB_k_ps", bufs=2, space="PSUM") as psK:
        vb_b = stABc.tile([128, C], F32)
        nc.sync.dma_start(vb_b[:], vb_d.ap().to_broadcast([128, C]))
        kb_sb = stABc.tile([128, NB], F32)
        nc.sync.dma_start(kb_sb[:], kb_d.ap().rearrange("o p -> p o"))
        qb_sb = stABc.tile([128, NB], F32)
        nc.sync.dma_start(qb_sb[:], qb_d.ap().rearrange("o p -> p o"))
        for g in range(T // 512):
            for tb4 in range(4):
                tb = g * 4 + tb4
                x_t = stAB.tile([128, C], F32, tag="x_t")
                nc.sync.dma_start(x_t[:], xf[tb * 128:(tb + 1) * 128, :])
                mean, rstd = ln_stats(stAB, x_t[:])
                hrow = stAB.tile([128, C], BF16, tag="hrow")
                ln_apply(stAB, hrow[:], x_t[:], mean, rstd)
                for cc in range(CCH):
                    pt = psT.tile([128, 128], BF16, tag="psT_t")
                    nc.tensor.transpose(pt[:], hrow[:, cc * 128:(cc + 1) * 128],
                                        identb[:])
                    eng = nc.scalar.copy if cc % 2 == 0 else nc.vector.tensor_copy
                    eng(hT8[:, cc, tb * 128:(tb + 1) * 128], pt[:])
                # V for this token block: stationary hT8(tb), moving Wv
                pv = psV.tile([128, 2, 512], F32, tag="pv")
                for cp in range(CP):
                    for grp in range(2):
                        nc.tensor.matmul(
                            pv[:, grp], hT8[:, 2 * cp:2 * cp + 2,
                                            tb * 128:(tb + 1) * 128],
                            Wv_sb[:, 2 * cp:2 * cp + 2, grp * 512:(grp + 1) * 512],
                            start=(cp == 0), stop=(cp == CP - 1),
                            perf_mode=PM.DoubleRow)
                for grp in range(2):
                    nc.vector.tensor_tensor(
                        V_sb[:, tb, grp * 8:(grp + 1) * 8, 0:64],
                        pv[:, grp].rearrange("p (h d) -> p h d", d=64),
                        vb_b[:, grp * 512:(grp + 1) * 512].rearrange(
                            "p (h d) -> p h d", d=64),
                        ALU.add)
            # K and Q (full sequence) for this 512-token group
            for pair in range(CCH):
                pk = psK.tile([128, 512], F32, tag="pk")
                for cp in range(CP):
                    nc.tensor.matmul(
                        pk[:], Wk_sb[:, 2 * cp:2 * cp + 2, pair * 128:(pair + 1) * 128],
                        hT8[:, 2 * cp:2 * cp + 2, g * 512:(g + 1) * 512],
                        start=(cp == 0), stop=(cp == CP - 1), perf_mode=PM.DoubleRow)
                nc.scalar.activation(KT[:, pair, g * 512:(g + 1) * 512], pk[:],
                                     AF.Identity, bias=kb_sb[:, pair:pair + 1],
                                     scale=1.0 / WSC)
                if g < TOK // 512:  # own (first) tokens only
                    pq = psK.tile([128, 512], F32, tag="pk")
                    for cp in range(CP):
                        nc.tensor.matmul(
                            pq[:], Wq_sb[:, 2 * cp:2 * cp + 2,
                                         pair * 128:(pair + 1) * 128],
                            hT8[:, 2 * cp:2 * cp + 2, g * 512:(g + 1) * 512],
                            start=(cp == 0), stop=(cp == CP - 1),
                            perf_mode=PM.DoubleRow)
                    nc.scalar.activation(QTf[:, pair, g * 512:(g + 1) * 512], pq[:],
                                         AF.Identity, bias=qb_sb[:, pair:pair + 1],
                                         scale=1.0 / WSC)
    es_h.close()    # free hT8
    es_wqkv.close() # free Wq/Wk/Wv

    # ---------- mask constants (key positions are per-core data) ----------
    es_mask = ExitStack()
    maskp = es_mask.enter_context(tc.tile_pool(name="maskp", bufs=1, side="right"))
    negm = maskp.tile([128, KB, 128], BF16)   # -1e9 where masked, else 0
    with tc.tile_pool(name="mtmp", bufs=1) as mtmp:
        pi_i = mtmp.tile([128, 1], mybir.dt.int32)
        nc.gpsimd.iota(pi_i[:], pattern=[[1, 1]], base=0, channel_multiplier=1)
        pi_f = mtmp.tile([128, 1], F32)
        nc.vector.tensor_copy(pi_f[:], pi_i[:])
        kpb = mtmp.tile([128, KB], F32)
        nc.sync.dma_start(kpb[:], kposb_d.ap().to_broadcast([128, KB]))
        kp_f = mtmp.tile([128, KB], F32)
        nc.vector.tensor_scalar(kp_f[:], kpb[:], pi_f[:], None, op0=ALU.add)
        qb = mtmp.tile([128, NB, 128], F32)
        for j in range(NB):
            nc.sync.dma_start(qb[:, j], qpos_d.ap()[j:j + 1, :].to_broadcast([128, 128]))
        for k in range(KB):
            jmin = k if k < NB else k - NB
            # negm[p_key, f_q] = -1e9 * (qpos_jmin[f] < keypos(block k)[p])
            nc.vector.tensor_scalar(
                negm[:, k], qb[:, jmin], kp_f[:, k:k + 1], -1e9,
                op0=ALU.is_lt, op1=ALU.mult)

    # ============ Phase C: attention (per head pair) ============
    nc.sync.dma_start(Wp_sb[:], Wpv)   # overlaps with attention
    with tc.tile_pool(name="stC", bufs=3) as stC, \
         tc.tile_pool(name="stC_att_ps", bufs=1, space="PSUM") as psCa, \
         tc.tile_pool(name="stC_s_ps", bufs=2, space="PSUM") as psCs, \
         tc.tile_pool(name="stC_t_ps", bufs=2, space="PSUM") as psCt:
        for hp in range(CCH):  # head pair = channel pair chunk
            for qp in range(2):  # query half: own cols [qp*512, qp*512+512)
                qbase = qp * 512
                klist = [k for k in range(KB)
                         if (k if k < NB else k - NB) * 128 < qbase + 512]
                ps_att = psCa.tile([128, 2, 512], F32, tag="ps_att")
                # software pipeline: emit scores/exp for k before AV of k-1,
                # so the in-order PE queue keeps computing scores while the
                # Act engine runs exp and AV(k-1) waits on it
                pend = None
                for ki, k in enumerate(klist):
                    jmin = k if k < NB else k - NB
                    q0 = jmin * 128
                    qlo = max(q0, qbase)
                    nq = qbase + 512 - qlo
                    weiT2 = stC.tile([128, 2, 512], BF16, tag="weiT")
                    ps_s = psCs.tile([128, 2, 512], F32, tag="ps_s")
                    diag = qlo == q0
                    for t in range(2):
                        nc.tensor.matmul(
                            ps_s[:, t, 0:nq],
                            KT[64 * t:64 * t + 64, hp, k * 128:(k + 1) * 128],
                            QTf[64 * t:64 * t + 64, hp, qlo:qlo + nq],
                            start=True, stop=not diag)
                        if diag:
                            # causal mask: accumulate -1e9 into the diagonal
                            # block via PE (identity @ negm)
                            nc.tensor.matmul(
                                ps_s[:, t, 0:128], identb[:], negm[:, k],
                                start=False, stop=True)
                    nc.scalar.activation(weiT2[:, :, 0:nq],
                                         ps_s[:, :, 0:nq], AF.Exp, scale=0.125)
                    if pend is not None:
                        pw, pqlo, pnq, pki = pend
                        for t in range(2):
                            nc.tensor.matmul(
                                ps_att[0:65, t, pqlo - qbase:512],
                                V_sb[:, klist[pki], 2 * hp + t, :],
                                pw[:, t, 0:pnq],
                                start=(pki == 0), stop=False)
                    pend = (weiT2, qlo, nq, ki)
                pw, pqlo, pnq, pki = pend
                for t in range(2):
                    nc.tensor.matmul(
                        ps_att[0:65, t, pqlo - qbase:512],
                        V_sb[:, klist[pki], 2 * hp + t, :],
                        pw[:, t, 0:pnq],
                        start=(pki == 0), stop=True)
                # normalize + transpose into attnT8 (copies on DVE: keep the
                # Act engine exp-only during attention to avoid table thrash)
                for t in range(2):
                    sb_at = stC.tile([128, 512], BF16, tag="sb_at")
                    nc.vector.tensor_copy(sb_at[0:65, :], ps_att[0:65, t, :])
                    for jj in range(4):
                        j = qp * 4 + jj
                        pt1 = psCt.tile([128, 128], BF16, tag="ptn")
                        nc.tensor.transpose(pt1[:], sb_at[:, jj * 128:(jj + 1) * 128],
                                            identb[:])
                        recip = stC.tile([128, 1], F32, tag="recip")
                        nc.vector.reciprocal(recip[:], pt1[:, 64:65])
                        attn_j = stC.tile([128, 64], BF16, tag="attn_j")
                        nc.vector.tensor_scalar(attn_j[:], pt1[:, 0:64], recip[:],
                                                1.0 / WSC, op0=ALU.mult, op1=ALU.mult)
                        pt2 = psCt.tile([128, 128], BF16, tag="ptn")
                        nc.tensor.transpose(pt2[0:64, :], attn_j[:], identb[:])
                        nc.vector.tensor_copy(
                            attnT8[64 * t:64 * t + 64, hp, j * 128:(j + 1) * 128],
                            pt2[0:64, :])
    es_mask.close()
    es_kqv.close()   # free KT, QTf, V

    # ============ Phase D: Wp proj + residual + LN2 ============
    es_x2 = ExitStack()
    x2p = es_x2.enter_context(tc.tile_pool(name="x2h2", bufs=1))
    x2_sb = x2p.tile([128, NB, C], F32)     # token-major
    # channel-major fp8 h2, split per 512-token half so the MLP can start
    # on the first half while phase D still produces the second
    h2Th = [x2p.tile([128, CCH, 512], F8, name=f"h2Th{i}") for i in range(2)]
    h2Tl = [x2p.tile([128, CCH, 512], F8, name=f"h2Tl{i}") for i in range(2)]
    es_ff = ExitStack()
    ffp = es_ff.enter_context(tc.tile_pool(name="ff1T", bufs=1))
    ff1Th = ffp.tile([128, FCH, TOK], F8)   # fp8 hi part of relu(ff1)
    ff1Tl = ffp.tile([128, FCH, TOK], F8)   # fp8 residual (lo) part
    with tc.tile_pool(name="stE_c", bufs=1) as stEc:
        b1p = stEc.tile([128, FCH], F32)
        nc.sync.dma_start(b1p[:], b1_d.ap().rearrange("x (o p) -> p (x o)", p=128))
        b2p = stEc.tile([128, CCH], F32)
        nc.sync.dma_start(b2p[:], b2_d.ap())
        # ---- Phase D + MLP-E1, interleaved: while phase D's second token
        # half runs its (Act/DVE-bound) LN chains, the first half's ff1
        # matmuls keep the PE busy ----
        with tc.tile_pool(name="stD", bufs=2) as stD, \
             tc.tile_pool(name="stD_ps", bufs=2, space="PSUM") as psD, \
             tc.tile_pool(name="stD_t_ps", bufs=2, space="PSUM") as psDt, \
             tc.tile_pool(name="stE1", bufs=2) as stE1, \
             tc.tile_pool(name="stE1_ps", bufs=2, space="PSUM") as psE1:

            def emit_d_proj(nt):
                pT_sb = stD.tile([128, CCH, 512], BF16, tag="pT_sb")
                for co in range(CCH):
                    pp = psD.tile([128, 512], F32, tag="pp")
                    for cp in range(CP):
                        nc.tensor.matmul(
                            pp[:], Wp_sb[:, 2 * cp:2 * cp + 2, co * 128:(co + 1) * 128],
                            attnT8[:, 2 * cp:2 * cp + 2, nt * 512:(nt + 1) * 512],
                            start=(cp == 0), stop=(cp == CP - 1),
                            perf_mode=PM.DoubleRow)
                    nc.scalar.activation(pT_sb[:, co], pp[:], AF.Identity,
                                         scale=1.0 / WSC)
                return pT_sb

            def emit_d_sub(nt, sub, pT_sb):
                tb = nt * 4 + sub
                xo_t = stD.tile([128, C], F32, tag="xo_t")
                nc.sync.dma_start(xo_t[:], xo[tb * 128:(tb + 1) * 128, :])
                for co in range(CCH):
                    ptd = psDt.tile([128, 128], BF16, tag="ptd")
                    nc.tensor.transpose(ptd[:], pT_sb[:, co, sub * 128:(sub + 1) * 128],
                                        identb[:])
                    nc.vector.tensor_tensor(x2_sb[:, tb, co * 128:(co + 1) * 128],
                                            ptd[:], xo_t[:, co * 128:(co + 1) * 128],
                                            ALU.add)
                # LN2
                mean, rstd = ln_stats(stD, x2_sb[:, tb, :])
                h2row = stD.tile([128, C], BF16, tag="h2row")
                ln_apply(stD, h2row[:], x2_sb[:, tb, :], mean, rstd)
                for cc in range(CCH):
                    pt = psDt.tile([128, 128], BF16, tag="ptd2")
                    nc.tensor.transpose(pt[:], h2row[:, cc * 128:(cc + 1) * 128],
                                        identb[:])
                    hi = h2Th[nt][:, cc, sub * 128:(sub + 1) * 128]
                    eng = nc.scalar.copy if cc % 2 == 0 else nc.vector.tensor_copy
                    eng(hi, pt[:])
                    nc.vector.tensor_tensor(
                        h2Tl[nt][:, cc, sub * 128:(sub + 1) * 128], pt[:], hi,
                        ALU.subtract)

            def emit_e1_fog(nt, fog):
                w1g = stE1.tile([128, CCH, 512], F8, tag="w1g")
                nc.sync.dma_start(
                    w1g[:], W1v.transpose([1, 0, 2])[:, :, fog * 512:(fog + 1) * 512])
                w1l = stE1.tile([128, CCH, 512], F8, tag="w1l")
                nc.sync.dma_start(
                    w1l[:], W1lov.transpose([1, 0, 2])[:, :, fog * 512:(fog + 1) * 512])
                for f4 in range(4):
                    fo = fog * 4 + f4
                    pf = psE1.tile([128, 512], F32, tag="pf")
                    slf = slice(f4 * 128, (f4 + 1) * 128)
                    for cp in range(CP):
                        sl2 = slice(2 * cp, 2 * cp + 2)
                        nc.tensor.matmul(pf[:], w1g[:, sl2, slf],
                                         h2Th[nt][:, sl2, :],
                                         start=(cp == 0), stop=False,
                                         perf_mode=PM.DoubleRow)
                        nc.tensor.matmul(pf[:], w1l[:, sl2, slf],
                                         h2Th[nt][:, sl2, :],
                                         start=False, stop=False,
                                         perf_mode=PM.DoubleRow)
                        nc.tensor.matmul(pf[:], w1g[:, sl2, slf],
                                         h2Tl[nt][:, sl2, :],
                                         start=False, stop=(cp == CP - 1),
                                         perf_mode=PM.DoubleRow)
                    ff1b = stE1.tile([128, 512], BF16, tag="ff1b")
                    nc.scalar.activation(ff1b[:], pf[:], AF.Relu,
                                         bias=b1p[:, fo:fo + 1],
                                         scale=1.0 / WSC)
                    hi = ff1Th[:, fo, nt * 512:(nt + 1) * 512]
                    nc.vector.tensor_copy(hi, ff1b[:])
                    nc.vector.tensor_tensor(
                        ff1Tl[:, fo, nt * 512:(nt + 1) * 512],
                        ff1b[:], hi, ALU.subtract)

            pT0 = emit_d_proj(0)
            for sub in range(4):
                emit_d_sub(0, sub, pT0)
            pT1 = emit_d_proj(1)
            for sub in range(4):
                emit_d_sub(1, sub, pT1)
                emit_e1_fog(0, 2 * sub)
                emit_e1_fog(0, 2 * sub + 1)
            for fog in range(8):
                emit_e1_fog(1, fog)
        with tc.tile_pool(name="stE2", bufs=2) as stE2, \
             tc.tile_pool(name="stE2_ps", bufs=2, space="PSUM") as psE2, \
             tc.tile_pool(name="stE2_t_ps", bufs=2, space="PSUM") as psEt:
            for co in range(CCH):
                w2c = stE2.tile([128, FCH, 128], F8, tag="w2c")
                nc.sync.dma_start(w2c[:], W2v[:, :, co * 128:(co + 1) * 128])
                w2l = stE2.tile([128, FCH, 128], F8, tag="w2l")
                nc.sync.dma_start(w2l[:], W2lov[:, :, co * 128:(co + 1) * 128])
                for nt in range(TOK // 512):
                    p2 = psE2.tile([128, 512], F32, tag="p2")
                    for fp in range(FCH // 2):
                        sl2 = slice(2 * fp, 2 * fp + 2)
                        slt = slice(nt * 512, (nt + 1) * 512)
                        nc.tensor.matmul(p2[:], w2c[:, sl2], ff1Th[:, sl2, slt],
                                         start=(fp == 0), stop=False,
                                         perf_mode=PM.DoubleRow)
                        nc.tensor.matmul(p2[:], w2l[:, sl2], ff1Th[:, sl2, slt],
                                         start=False, stop=False,
                                         perf_mode=PM.DoubleRow)
                        nc.tensor.matmul(p2[:], w2c[:, sl2], ff1Tl[:, sl2, slt],
                                         start=False, stop=(fp == FCH // 2 - 1),
                                         perf_mode=PM.DoubleRow)
                    ff2_c = stE2.tile([128, 512], BF16, tag="ff2_c")
                    nc.scalar.activation(ff2_c[:], p2[:], AF.Identity,
                                         bias=b2p[:, co:co + 1], scale=1.0 / W2SC)
                    for sub in range(4):
                        tb = nt * 4 + sub
                        ptf = psEt.tile([128, 128], BF16, tag="ptf")
                        nc.tensor.transpose(ptf[:], ff2_c[:, sub * 128:(sub + 1) * 128],
                                            identb[:])
                        out_c = stE2.tile([128, 128], F32, tag="out_c")
                        nc.vector.tensor_tensor(
                            out_c[:], ptf[:], x2_sb[:, tb, co * 128:(co + 1) * 128],
                            ALU.add)
                        nc.sync.dma_start(
                            out_d.ap()[tb * 128:(tb + 1) * 128,
                                       co * 128:(co + 1) * 128], out_c[:])
    es_ff.close()
    es_x2.close()
    es_at.close()   # free attnT8
    es_wp.close()   # free Wp


def make_nc():
    nc = bacc.Bacc("TRN2", target_bir_lowering=False, debug=False,
                   num_devices=N_CORES)
    build(nc)
    nc.compile()
    return nc


def shard_inputs(inputs):
    """Full inputs dict -> list of 8 per-core in_maps.

    Folds LN1 gain/bias into Wq/Wk/Wv (weights scaled by g1 per input channel,
    be1 contribution becomes an additive bias on q/k/v) and LN2's into W1/b1.
    Wq/Wk/Wv/Wp are fp8 at 32x scale; W1/W2 are bf16.
    """
    E4 = mybir.dt.np(F8)
    BF = mybir.dt.np(BF16)
    x = np.asarray(inputs["x"], np.float32)
    assert x.shape == (B, T, C)
    f64 = np.float64
    Wq = np.asarray(inputs["Wq"], f64); Wk = np.asarray(inputs["Wk"], f64)
    Wv = np.asarray(inputs["Wv"], f64); Wp = np.asarray(inputs["Wp"], f64)
    W1 = np.asarray(inputs["W1"], f64); W2 = np.asarray(inputs["W2"], np.float32)
    g1 = np.asarray(inputs["g1"], f64); be1 = np.asarray(inputs["be1"], f64)
    g2 = np.asarray(inputs["g2"], f64); be2 = np.asarray(inputs["be2"], f64)
    b1 = np.asarray(inputs["b1"], f64)
    shared = {
        "Wq": (WSC * g1[:, None] * Wq).astype(np.float32).astype(E4),
        "Wk": (WSC * g1[:, None] * Wk).astype(np.float32).astype(E4),
        "Wv": (WSC * g1[:, None] * Wv).astype(np.float32).astype(E4),
        "Wp": (WSC * Wp).astype(np.float32).astype(E4),
        "W1": (WSC * g2[:, None] * W1).astype(np.float32).astype(E4),
        "W1lo": ((WSC * g2[:, None] * W1).astype(np.float32)
                 - (WSC * g2[:, None] * W1).astype(np.float32).astype(E4)
                   .astype(np.float32)).astype(E4),
        "W2": (W2SC * W2).astype(E4),
        "W2lo": (W2SC * W2 - (W2SC * W2).astype(E4).astype(np.float32)).astype(E4),
        "qbias": (be1 @ Wq).astype(np.float32).reshape(NB, 128),
        "kbias": (be1 @ Wk).astype(np.float32).reshape(NB, 128),
        "vbias": (WSC * (be1 @ Wv)).astype(np.float32).reshape(1, C),
        "b1": (b1 + be2 @ W1).astype(np.float32).reshape(1, FF),
        "bp": np.asarray(inputs["bp"], np.float32).reshape(1, C),
        "b2": np.ascontiguousarray(
            np.asarray(inputs["b2"], np.float32).reshape(CCH, 128).T),
    }
    in_maps = []
    for c in range(N_CORES):
        b, par = c // 2, c % 2
        # permuted sequence: own (parity) blocks first, partner blocks after,
        # so the program is parity-independent; key positions ride as data
        gblocks = [2 * j + par for j in range(NB)] + \
                  [2 * j + (1 - par) for j in range(NB)]
        xperm = np.concatenate([x[b, g * 128:(g + 1) * 128, :] for g in gblocks], 0)
        # bp is folded into the residual rows (x2 = xown + bp + attn @ Wp)
        rows = xperm[:TOK] + np.asarray(inputs["bp"], np.float32)[None, :]
        qpos = np.stack([np.arange(g * 128, (g + 1) * 128, dtype=np.float32)
                         for g in gblocks[:NB]], 0)
        kposb = np.array([[g * 128 for g in gblocks]], dtype=np.float32)
        m = {"xfull": np.ascontiguousarray(xperm),
             "xown": np.ascontiguousarray(rows), "qpos": qpos, "kposb": kposb}
        m.update(shared)
        in_maps.append(m)
    return in_maps


def unshard_outputs(results):
    """list of per-core {'out': [TOK, C]} -> [B, T, C]"""
    out = np.zeros((B, T, C), np.float32)
    for c in range(N_CORES):
        b, par = c // 2, c % 2
        r = np.asarray(results[c]["out"])
        for j in range(NB):
            g = 2 * j + par
            out[b, g * 128:(g + 1) * 128, :] = r[j * 128:(j + 1) * 128, :]
    return out


_NC_CACHE = {}

def _get_nc():
    if "nc" not in _NC_CACHE:
        nc = bacc.Bacc("TRN2", target_bir_lowering=False, debug=False,
                       num_devices=N_CORES)
        build(nc, reps=1)
        nc.compile()
        _NC_CACHE["nc"] = nc
    return _NC_CACHE["nc"]


def kernel(**inputs):
    from concourse.bass_utils import run_bass_kernel_spmd
    nc = _get_nc()
    in_maps = shard_inputs(inputs)
    res = run_bass_kernel_spmd(nc, in_maps, core_ids=list(range(N_CORES)))
    return unshard_outputs(res.results)


# revision 70
# speedup vs baseline: 285.6664x; 1.0020x over previous
"""Self-contained Trainium2 kernel for the dense transformer block problem.

kernel(**inputs) takes the FULL inputs (as produced by the reference
setup_inputs), shards them across 8 NeuronCores (2 cores per batch element,
causal-balanced parity split of query blocks), runs a Bass/Tile SPMD kernel,
and reassembles the full [B, T, C] output.

Design (v2):
  - Wq/Wk/Wv/Wp are fp8e4m3 (host-quantized at 32x scale, LN1 gain folded in)
    and SBUF-resident; projections run as DoubleRow fp8 matmuls (2 contraction
    chunks per instruction).
  - LN1 -> transpose -> K/V flow is fused per 512-token group; Q is computed
    for the full sequence (same code path as K) and attention reads the own
    (parity) query blocks via strided views.
  - Scores run per head-PAIR: two row-tiled matmuls (contraction rows 0-63 /
    64-127 of the PE array) execute concurrently on hardware; one Exp
    activation covers both heads' score chunks.
  - V carries 32x values plus a ones column; AV accumulates [32*attn; rowsum]
    in PSUM; normalization folds the 1/32 into the per-token reciprocal
    multiply. attnT is fp8 in SBUF (no DRAM round trip), feeding a DoubleRow
    Wp projection.
  - x2 (attention residual) stays in SBUF; MLP is a single pass over bf16
    W1/W2 with bf16 ff1 activations; b2 and the final residual are folded
    into the output assembly.
"""
import sys
sys.path.insert(0, '/opt/trn_rl_repo')
import numpy as np
from contextlib import ExitStack

import concourse.bacc as bacc
import concourse.tile as tile
import concourse.mybir as mybir
from concourse.masks import make_identity

F32 = mybir.dt.float32
F32R = mybir.dt.float32r
BF16 = mybir.dt.bfloat16
F8 = mybir.dt.float8e4
AF = mybir.ActivationFunctionType
ALU = mybir.AluOpType
PM = mybir.MatmulPerfMode

B, T, C, H, DH = 4, 2048, 1024, 16, 64
N_CORES = 8
TOK = 1024          # own tokens per core
NB = TOK // 128     # 8 own query blocks
KB = T // 128       # 16 key blocks
CCH = C // 128      # 8 channel chunks
CP = CCH // 2       # 4 chunk pairs (DoubleRow)
FF = 4 * C          # 4096
FCH = FF // 128     # 32 ff chunks
EPS = 1e-5
WSC = 32.0          # fp8 weight scale

IN_NAMES = ["xfull", "xown", "qpos", "kposb", "Wq", "Wk", "Wv", "Wp", "bp",
            "W1", "W1lo", "b1", "W2", "W2lo", "b2", "qbias", "kbias", "vbias"]
W2SC = 64.0         # fp8 W2 scale (split hi/lo representation)


def build(nc, reps=1, loop_reps=None):
    """Trace the SPMD program into nc (a bacc.Bacc). Call nc.compile() after.

    Weight inputs arrive pre-folded on the host:
      Wq/Wk/Wv = fp8(32 * diag(g1) @ W);  qbias/kbias = be1 @ W
      Wp = fp8(32 * Wp);  vbias = 32 * be1 @ Wv
      W1 = bf16(diag(g2) @ W1);  b1 = b1 + be2 @ W1
      W2 = bf16(W2);  b2 arranged [128, CCH].
    """
    def din(name, shape, dt=F32):
        return nc.dram_tensor(name, shape, dt, kind="ExternalInput")

    xfull_d = din("xfull", [T, C])
    xown_d = din("xown", [TOK, C])
    qpos_d = din("qpos", [NB, 128])
    kposb_d = din("kposb", [1, KB])
    Wq_d = din("Wq", [C, C], F8); Wk_d = din("Wk", [C, C], F8)
    Wv_d = din("Wv", [C, C], F8); Wp_d = din("Wp", [C, C], F8)
    bp_d = din("bp", [1, C]); W1_d = din("W1", [C, FF], F8)
    W1lo_d = din("W1lo", [C, FF], F8); b1_d = din("b1", [1, FF])
    W2_d = din("W2", [FF, C], F8); W2lo_d = din("W2lo", [FF, C], F8)
    b2_d = din("b2", [128, CCH])
    qb_d = din("qbias", [NB, 128])   # be1 @ Wq, laid out [pair, within]
    kb_d = din("kbias", [NB, 128])   # be1 @ Wk
    vb_d = din("vbias", [1, C])      # 32 * be1 @ Wv
    out_d = nc.dram_tensor("out", [TOK, C], F32, kind="ExternalOutput")

    Wqv = Wq_d.ap().rearrange("(o p) m -> p o m", p=128)
    Wkv = Wk_d.ap().rearrange("(o p) m -> p o m", p=128)
    Wvv = Wv_d.ap().rearrange("(o p) m -> p o m", p=128)
    Wpv = Wp_d.ap().rearrange("(o p) m -> p o m", p=128)
    W1v = W1_d.ap().rearrange("(o p) m -> o p m", p=128)
    W1lov = W1lo_d.ap().rearrange("(o p) m -> o p m", p=128)
    W2v = W2_d.ap().rearrange("(o p) m -> p o m", p=128)  # [128, 32, 1024]
    W2lov = W2lo_d.ap().rearrange("(o p) m -> p o m", p=128)
    xf = xfull_d.ap()
    xo = xown_d.ap()

    env = locals()
    if loop_reps is not None:
        with tile.TileContext(nc) as tc:
            with tc.For_i(0, loop_reps, 1):
                with ExitStack() as top:
                    _build_body(nc, tc, top, env)
    else:
        for _rep in range(reps):
            with tile.TileContext(nc) as tc, ExitStack() as top:
                _build_body(nc, tc, top, env)
    return IN_NAMES


def _build_body(nc, tc, top, env):
    (xfull_d, xown_d, qpos_d, kposb_d, Wq_d, Wk_d, Wv_d, Wp_d, bp_d, W1_d,
     b1_d, W2_d, b2_d, qb_d, kb_d, vb_d, out_d, Wqv, Wkv, Wvv, Wpv, W1v, W1lov,
     W2v, W2lov, xf, xo) = (
        env[k] for k in ["xfull_d", "xown_d", "qpos_d", "kposb_d", "Wq_d",
                         "Wk_d", "Wv_d", "Wp_d", "bp_d", "W1_d", "b1_d", "W2_d",
                         "b2_d", "qb_d", "kb_d", "vb_d", "out_d", "Wqv", "Wkv",
                         "Wvv", "Wpv", "W1v", "W1lov", "W2v", "W2lov", "xf",
                         "xo"])

    const = top.enter_context(tc.tile_pool(name="const", bufs=1))
    identf = const.tile([128, 128], F32)
    make_identity(nc, identf[:])
    identb = const.tile([128, 128], BF16)
    nc.vector.tensor_copy(identb[:], identf[:])
    eps_t = const.tile([128, 1], F32)
    nc.vector.memset(eps_t[:], EPS)

    def ln_stats(pool, x_ap):
        n = x_ap.shape[-1] // 512
        xg = x_ap.rearrange("p (n f) -> p n f", f=512)
        stats = pool.tile([128, n, 6], F32, tag="ln_stats")
        mv = pool.tile([128, 2], F32, tag="ln_mv")
        for i in range(n):
            nc.vector.bn_stats(stats[:, i], xg[:, i])
        nc.vector.bn_aggr(mv[:], stats[:])
        rstd = pool.tile([128, 1], F32, tag="ln_rstd")
        nc.scalar.activation(rstd[:], mv[:, 1:2], AF.Sqrt, bias=eps_t[:])
        nc.vector.reciprocal(rstd[:], rstd[:])
        return mv[:, 0:1], rstd

    def ln_apply(pool, out_ap, x_ap, mean, rstd):
        # out = (x - mu) * rstd on ACT: Identity(x * rstd + (-mu * rstd))
        nmr = pool.tile([128, 1], F32, tag="ln_nmr")
        nc.vector.tensor_scalar(nmr[:], mean, rstd[:], -1.0,
                                op0=ALU.mult, op1=ALU.mult)
        nc.scalar.activation(out_ap, x_ap, AF.Identity,
                             bias=nmr[:], scale=rstd[:])

    # ---------------- resident pools ----------------
    # left side: pools that live to the end of the body (LIFO close order)
    es_wp = ExitStack()
    wpp = es_wp.enter_context(tc.tile_pool(name="wpp", bufs=1))
    Wp_sb = wpp.tile([128, CCH, C], F8)
    es_at = ExitStack()
    atp = es_at.enter_context(tc.tile_pool(name="attnT", bufs=1))
    attnT8 = atp.tile([128, CCH, TOK], F8)

    # right side: big transients, ordered by lifetime (longest first)
    es_kqv = ExitStack()
    kqvp = es_kqv.enter_context(tc.tile_pool(name="kqv", bufs=1, side="right"))
    KT = kqvp.tile([128, CCH, T], BF16)
    QTf = kqvp.tile([128, CCH, TOK], BF16)
    V_sb = kqvp.tile([128, KB, H, 65], BF16)
    ones_b = kqvp.tile([128, 1], BF16)
    nc.vector.memset(ones_b[:], 1.0)
    nc.vector.tensor_copy(V_sb[:, :, :, 64:65],
                          ones_b[:, 0:1, None, None].to_broadcast([128, KB, H, 1]))

    es_wqkv = ExitStack()
    wqkv = es_wqkv.enter_context(tc.tile_pool(name="wqkv", bufs=1, side="right"))
    Wq_sb = wqkv.tile([128, CCH, C], F8)
    Wk_sb = wqkv.tile([128, CCH, C], F8)
    Wv_sb = wqkv.tile([128, CCH, C], F8)
    nc.sync.dma_start(Wv_sb[:], Wvv)
    nc.sync.dma_start(Wk_sb[:], Wkv)
    nc.sync.dma_start(Wq_sb[:], Wqv)

    es_h = ExitStack()
    hp_ = es_h.enter_context(tc.tile_pool(name="hT8", bufs=1, side="right"))
    hT8 = hp_.tile([128, CCH, T], F8)

    # ============ Phase AB: LN1 + V/K/Q per 512-token group ============
    with tc.tile_pool(name="stAB", bufs=3) as stAB, \
         tc.tile_pool(name="stABc", bufs=1) as stABc, \
         tc.tile_pool(name="stAB_t_ps", bufs=3, space="PSUM") as psT, \
         tc.tile_pool(name="stAB_v_ps", bufs=1, space="PSUM") as psV, \
         tc.tile_pool(name="stAB_k_ps", bufs=2, space="PSUM") as psK:
        vb_b = stABc.tile([128, C], F32)
        nc.sync.dma_start(vb_b[:], vb_d.ap().to_broadcast([128, C]))
        kb_sb = stABc.tile([128, NB], F32)
        nc.sync.dma_start(kb_sb[:], kb_d.ap().rearrange("o p -> p o"))
        qb_sb = stABc.tile([128, NB], F32)
        nc.sync.dma_start(qb_sb[:], qb_d.ap().rearrange("o p -> p o"))
        def emit_ln(tb):
            x_t = stAB.tile([128, C], F32, tag="x_t")
            nc.sync.dma_start(x_t[:], xf[tb * 128:(tb + 1) * 128, :])
            mean, rstd = ln_stats(stAB, x_t[:])
            hrow = stAB.tile([128, C], BF16, tag="hrow")
            ln_apply(stAB, hrow[:], x_t[:], mean, rstd)
            return hrow

        def emit_tv(tb, hrow):
            for cc in range(CCH):
                pt = psT.tile([128, 128], BF16, tag="psT_t")
                nc.tensor.transpose(pt[:], hrow[:, cc * 128:(cc + 1) * 128],
                                    identb[:])
                eng = nc.scalar.copy if cc % 2 == 0 else nc.vector.tensor_copy
                eng(hT8[:, cc, tb * 128:(tb + 1) * 128], pt[:])
            # V for this token block: stationary hT8(tb), moving Wv
            pv = psV.tile([128, 2, 512], F32, tag="pv")
            for cp in range(CP):
                for grp in range(2):
                    nc.tensor.matmul(
                        pv[:, grp], hT8[:, 2 * cp:2 * cp + 2,
                                        tb * 128:(tb + 1) * 128],
                        Wv_sb[:, 2 * cp:2 * cp + 2, grp * 512:(grp + 1) * 512],
                        start=(cp == 0), stop=(cp == CP - 1),
                        perf_mode=PM.DoubleRow)
            for grp in range(2):
                nc.vector.tensor_tensor(
                    V_sb[:, tb, grp * 8:(grp + 1) * 8, 0:64],
                    pv[:, grp].rearrange("p (h d) -> p h d", d=64),
                    vb_b[:, grp * 512:(grp + 1) * 512].rearrange(
                        "p (h d) -> p h d", d=64),
                    ALU.add)

        def emit_kq(g):
            # K and Q (full sequence) for this 512-token group
            for pair in range(CCH):
                pk = psK.tile([128, 512], F32, tag="pk")
                for cp in range(CP):
                    nc.tensor.matmul(
                        pk[:], Wk_sb[:, 2 * cp:2 * cp + 2, pair * 128:(pair + 1) * 128],
                        hT8[:, 2 * cp:2 * cp + 2, g * 512:(g + 1) * 512],
                        start=(cp == 0), stop=(cp == CP - 1), perf_mode=PM.DoubleRow)
                nc.scalar.activation(KT[:, pair, g * 512:(g + 1) * 512], pk[:],
                                     AF.Identity, bias=kb_sb[:, pair:pair + 1],
                                     scale=1.0 / WSC)
                if g < TOK // 512:  # own (first) tokens only
                    pq = psK.tile([128, 512], F32, tag="pk")
                    for cp in range(CP):
                        nc.tensor.matmul(
                            pq[:], Wq_sb[:, 2 * cp:2 * cp + 2,
                                         pair * 128:(pair + 1) * 128],
                            hT8[:, 2 * cp:2 * cp + 2, g * 512:(g + 1) * 512],
                            start=(cp == 0), stop=(cp == CP - 1),
                            perf_mode=PM.DoubleRow)
                    nc.scalar.activation(QTf[:, pair, g * 512:(g + 1) * 512], pq[:],
                                         AF.Identity, bias=qb_sb[:, pair:pair + 1],
                                         scale=1.0 / WSC)

        # lag-1 software pipeline: tb's LN chain (Act/DVE) is emitted before
        # tb-1's PE consumers, so the in-order PE queue never waits on a
        # freshly-issued LN
        prev = None
        for tb in range(T // 128):
            hrow = emit_ln(tb)
            if prev is not None:
                emit_tv(prev[0], prev[1])
                if prev[0] % 4 == 3:
                    emit_kq(prev[0] // 4)
            prev = (tb, hrow)
        emit_tv(prev[0], prev[1])
        emit_kq(prev[0] // 4)
    es_h.close()    # free hT8
    es_wqkv.close() # free Wq/Wk/Wv

    # ---------- mask constants (key positions are per-core data) ----------
    es_mask = ExitStack()
    maskp = es_mask.enter_context(tc.tile_pool(name="maskp", bufs=1, side="right"))
    negm = maskp.tile([128, KB, 128], BF16)   # -1e9 where masked, else 0
    with tc.tile_pool(name="mtmp", bufs=1) as mtmp:
        pi_i = mtmp.tile([128, 1], mybir.dt.int32)
        nc.gpsimd.iota(pi_i[:], pattern=[[1, 1]], base=0, channel_multiplier=1)
        pi_f = mtmp.tile([128, 1], F32)
        nc.vector.tensor_copy(pi_f[:], pi_i[:])
        kpb = mtmp.tile([128, KB], F32)
        nc.sync.dma_start(kpb[:], kposb_d.ap().to_broadcast([128, KB]))
        kp_f = mtmp.tile([128, KB], F32)
        nc.vector.tensor_scalar(kp_f[:], kpb[:], pi_f[:], None, op0=ALU.add)
        qb = mtmp.tile([128, NB, 128], F32)
        for j in range(NB):
            nc.sync.dma_start(qb[:, j], qpos_d.ap()[j:j + 1, :].to_broadcast([128, 128]))
        for k in range(KB):
            jmin = k if k < NB else k - NB
            # negm[p_key, f_q] = -1e9 * (qpos_jmin[f] < keypos(block k)[p])
            nc.vector.tensor_scalar(
                negm[:, k], qb[:, jmin], kp_f[:, k:k + 1], -1e9,
                op0=ALU.is_lt, op1=ALU.mult)

    # ============ Phase C: attention (per head pair) ============
    nc.sync.dma_start(Wp_sb[:], Wpv)   # overlaps with attention
    with tc.tile_pool(name="stC", bufs=3) as stC, \
         tc.tile_pool(name="stC_att_ps", bufs=1, space="PSUM") as psCa, \
         tc.tile_pool(name="stC_s_ps", bufs=2, space="PSUM") as psCs, \
         tc.tile_pool(name="stC_t_ps", bufs=2, space="PSUM") as psCt:
        for hp in range(CCH):  # head pair = channel pair chunk
            for qp in range(2):  # query half: own cols [qp*512, qp*512+512)
                qbase = qp * 512
                klist = [k for k in range(KB)
                         if (k if k < NB else k - NB) * 128 < qbase + 512]
                ps_att = psCa.tile([128, 2, 512], F32, tag="ps_att")
                # software pipeline: emit scores/exp for k before AV of k-1,
                # so the in-order PE queue keeps computing scores while the
                # Act engine runs exp and AV(k-1) waits on it
                pend = None
                for ki, k in enumerate(klist):
                    jmin = k if k < NB else k - NB
                    q0 = jmin * 128
                    qlo = max(q0, qbase)
                    nq = qbase + 512 - qlo
                    weiT2 = stC.tile([128, 2, 512], BF16, tag="weiT")
                    ps_s = psCs.tile([128, 2, 512], F32, tag="ps_s")
                    diag = qlo == q0
                    for t in range(2):
                        nc.tensor.matmul(
                            ps_s[:, t, 0:nq],
                            KT[64 * t:64 * t + 64, hp, k * 128:(k + 1) * 128],
                            QTf[64 * t:64 * t + 64, hp, qlo:qlo + nq],
                            start=True, stop=not diag)
                        if diag:
                            # causal mask: accumulate -1e9 into the diagonal
                            # block via PE (identity @ negm)
                            nc.tensor.matmul(
                                ps_s[:, t, 0:128], identb[:], negm[:, k],
                                start=False, stop=True)
                    nc.scalar.activation(weiT2[:, :, 0:nq],
                                         ps_s[:, :, 0:nq], AF.Exp, scale=0.125)
                    if pend is not None:
                        pw, pqlo, pnq, pki = pend
                        for t in range(2):
                            nc.tensor.matmul(
                                ps_att[0:65, t, pqlo - qbase:512],
                                V_sb[:, klist[pki], 2 * hp + t, :],
                                pw[:, t, 0:pnq],
                                start=(pki == 0), stop=False)
                    pend = (weiT2, qlo, nq, ki)
                pw, pqlo, pnq, pki = pend
                for t in range(2):
                    nc.tensor.matmul(
                        ps_att[0:65, t, pqlo - qbase:512],
                        V_sb[:, klist[pki], 2 * hp + t, :],
                        pw[:, t, 0:pnq],
                        start=(pki == 0), stop=True)
                # normalize + transpose into attnT8 (copies on DVE: keep the
                # Act engine exp-only during attention to avoid table thrash)
                for t in range(2):
                    sb_at = stC.tile([128, 512], BF16, tag="sb_at")
                    nc.vector.tensor_copy(sb_at[0:65, :], ps_att[0:65, t, :])
                    for jj in range(4):
                        j = qp * 4 + jj
                        pt1 = psCt.tile([128, 128], BF16, tag="ptn")
                        nc.tensor.transpose(pt1[:], sb_at[:, jj * 128:(jj + 1) * 128],
                                            identb[:])
                        recip = stC.tile([128, 1], F32, tag="recip")
                        nc.vector.reciprocal(recip[:], pt1[:, 64:65])
                        attn_j = stC.tile([128, 64], BF16, tag="attn_j")
                        nc.vector.tensor_scalar(attn_j[:], pt1[:, 0:64], recip[:],
                                                1.0 / WSC, op0=ALU.mult, op1=ALU.mult)
                        pt2 = psCt.tile([128, 128], BF16, tag="ptn")
                        nc.tensor.transpose(pt2[0:64, :], attn_j[:], identb[:])
                        nc.vector.tensor_copy(
                            attnT8[64 * t:64 * t + 64, hp, j * 128:(j + 1) * 128],
                            pt2[0:64, :])
    es_mask.close()
    es_kqv.close()   # free KT, QTf, V

    # ============ Phase D: Wp proj + residual + LN2 ============
    es_x2 = ExitStack()
    x2p = es_x2.enter_context(tc.tile_pool(name="x2h2", bufs=1))
    x2_sb = x2p.tile([128, NB, C], F32)     # token-major
    # channel-major fp8 h2, split per 512-token half so the MLP can start
    # on the first half while phase D still produces the second
    h2Th = [x2p.tile([128, CCH, 512], F8, name=f"h2Th{i}") for i in range(2)]
    h2Tl = [x2p.tile([128, CCH, 512], F8, name=f"h2Tl{i}") for i in range(2)]
    es_ff = ExitStack()
    ffp = es_ff.enter_context(tc.tile_pool(name="ff1T", bufs=1))
    ff1Th = ffp.tile([128, FCH, TOK], F8)   # fp8 hi part of relu(ff1)
    ff1Tl = ffp.tile([128, FCH, TOK], F8)   # fp8 residual (lo) part
    with tc.tile_pool(name="stE_c", bufs=1) as stEc:
        b1p = stEc.tile([128, FCH], F32)
        nc.sync.dma_start(b1p[:], b1_d.ap().rearrange("x (o p) -> p (x o)", p=128))
        b2p = stEc.tile([128, CCH], F32)
        nc.sync.dma_start(b2p[:], b2_d.ap())
        # ---- Phase D + MLP-E1, interleaved: while phase D's second token
        # half runs its (Act/DVE-bound) LN chains, the first half's ff1
        # matmuls keep the PE busy ----
        with tc.tile_pool(name="stD", bufs=2) as stD, \
             tc.tile_pool(name="stD_ps", bufs=2, space="PSUM") as psD, \
             tc.tile_pool(name="stD_t_ps", bufs=2, space="PSUM") as psDt, \
             tc.tile_pool(name="stE1", bufs=2) as stE1, \
             tc.tile_pool(name="stE1_ps", bufs=2, space="PSUM") as psE1:

            def emit_d_proj(nt):
                pT_sb = stD.tile([128, CCH, 512], BF16, tag="pT_sb")
                for co in range(CCH):
                    pp = psD.tile([128, 512], F32, tag="pp")
                    for cp in range(CP):
                        nc.tensor.matmul(
                            pp[:], Wp_sb[:, 2 * cp:2 * cp + 2, co * 128:(co + 1) * 128],
                            attnT8[:, 2 * cp:2 * cp + 2, nt * 512:(nt + 1) * 512],
                            start=(cp == 0), stop=(cp == CP - 1),
                            perf_mode=PM.DoubleRow)
                    nc.scalar.activation(pT_sb[:, co], pp[:], AF.Identity,
                                         scale=1.0 / WSC)
                return pT_sb

            def emit_d_sub(nt, sub, pT_sb):
                tb = nt * 4 + sub
                xo_t = stD.tile([128, C], F32, tag="xo_t")
                nc.sync.dma_start(xo_t[:], xo[tb * 128:(tb + 1) * 128, :])
                for co in range(CCH):
                    ptd = psDt.tile([128, 128], BF16, tag="ptd")
                    nc.tensor.transpose(ptd[:], pT_sb[:, co, sub * 128:(sub + 1) * 128],
                                        identb[:])
                    nc.vector.tensor_tensor(x2_sb[:, tb, co * 128:(co + 1) * 128],
                                            ptd[:], xo_t[:, co * 128:(co + 1) * 128],
                                            ALU.add)
                # LN2
                mean, rstd = ln_stats(stD, x2_sb[:, tb, :])
                h2row = stD.tile([128, C], BF16, tag="h2row")
                ln_apply(stD, h2row[:], x2_sb[:, tb, :], mean, rstd)
                for cc in range(CCH):
                    pt = psDt.tile([128, 128], BF16, tag="ptd2")
                    nc.tensor.transpose(pt[:], h2row[:, cc * 128:(cc + 1) * 128],
                                        identb[:])
                    hi = h2Th[nt][:, cc, sub * 128:(sub + 1) * 128]
                    eng = nc.scalar.copy if cc % 2 == 0 else nc.vector.tensor_copy
                    eng(hi, pt[:])
                    nc.vector.tensor_tensor(
                        h2Tl[nt][:, cc, sub * 128:(sub + 1) * 128], pt[:], hi,
                        ALU.subtract)

            def emit_e1_fog(nt, fog):
                w1g = stE1.tile([128, CCH, 512], F8, tag="w1g")
                nc.sync.dma_start(
                    w1g[:], W1v.transpose([1, 0, 2])[:, :, fog * 512:(fog + 1) * 512])
                w1l = stE1.tile([128, CCH, 512], F8, tag="w1l")
                nc.sync.dma_start(
                    w1l[:], W1lov.transpose([1, 0, 2])[:, :, fog * 512:(fog + 1) * 512])
                for f4 in range(4):
                    fo = fog * 4 + f4
                    pf = psE1.tile([128, 512], F32, tag="pf")
                    slf = slice(f4 * 128, (f4 + 1) * 128)
                    for cp in range(CP):
                        sl2 = slice(2 * cp, 2 * cp + 2)
                        nc.tensor.matmul(pf[:], w1g[:, sl2, slf],
                                         h2Th[nt][:, sl2, :],
                                         start=(cp == 0), stop=False,
                                         perf_mode=PM.DoubleRow)
                        nc.tensor.matmul(pf[:], w1l[:, sl2, slf],
                                         h2Th[nt][:, sl2, :],
                                         start=False, stop=False,
                                         perf_mode=PM.DoubleRow)
                        nc.tensor.matmul(pf[:], w1g[:, sl2, slf],
                                         h2Tl[nt][:, sl2, :],
                                         start=False, stop=(cp == CP - 1),
                                         perf_mode=PM.DoubleRow)
                    ff1b = stE1.tile([128, 512], BF16, tag="ff1b")
                    nc.scalar.activation(ff1b[:], pf[:], AF.Relu,
                                         bias=b1p[:, fo:fo + 1],
                                         scale=1.0 / WSC)
                    hi = ff1Th[:, fo, nt * 512:(nt + 1) * 512]
                    nc.vector.tensor_copy(hi, ff1b[:])
                    nc.vector.tensor_tensor(
                        ff1Tl[:, fo, nt * 512:(nt + 1) * 512],
                        ff1b[:], hi, ALU.subtract)

            pT0 = emit_d_proj(0)
            pT1 = emit_d_proj(1)
            for sub in range(4):
                emit_d_sub(0, sub, pT0)
            for sub in range(4):
                emit_d_sub(1, sub, pT1)
                emit_e1_fog(0, 2 * sub)
                emit_e1_fog(0, 2 * sub + 1)
            for fog in range(8):
                emit_e1_fog(1, fog)
        with tc.tile_pool(name="stE2", bufs=2) as stE2, \
             tc.tile_pool(name="stE2_ps", bufs=2, space="PSUM") as psE2, \
             tc.tile_pool(name="stE2_t_ps", bufs=2, space="PSUM") as psEt:
            for co in range(CCH):
                w2c = stE2.tile([128, FCH, 128], F8, tag="w2c")
                nc.sync.dma_start(w2c[:], W2v[:, :, co * 128:(co + 1) * 128])
                w2l = stE2.tile([128, FCH, 128], F8, tag="w2l")
                nc.sync.dma_start(w2l[:], W2lov[:, :, co * 128:(co + 1) * 128])
                for nt in range(TOK // 512):
                    p2 = psE2.tile([128, 512], F32, tag="p2")
                    for fp in range(FCH // 2):
                        sl2 = slice(2 * fp, 2 * fp + 2)
                        slt = slice(nt * 512, (nt + 1) * 512)
                        nc.tensor.matmul(p2[:], w2c[:, sl2], ff1Th[:, sl2, slt],
                                         start=(fp == 0), stop=False,
                                         perf_mode=PM.DoubleRow)
                        nc.tensor.matmul(p2[:], w2l[:, sl2], ff1Th[:, sl2, slt],
                                         start=False, stop=False,
                                         perf_mode=PM.DoubleRow)
                        nc.tensor.matmul(p2[:], w2c[:, sl2], ff1Tl[:, sl2, slt],
                                         start=False, stop=(fp == FCH // 2 - 1),
                                         perf_mode=PM.DoubleRow)
                    ff2_c = stE2.tile([128, 512], BF16, tag="ff2_c")
                    nc.scalar.activation(ff2_c[:], p2[:], AF.Identity,
                                         bias=b2p[:, co:co + 1], scale=1.0 / W2SC)
                    for sub in range(4):
                        tb = nt * 4 + sub
                        ptf = psEt.tile([128, 128], BF16, tag="ptf")
                        nc.tensor.transpose(ptf[:], ff2_c[:, sub * 128:(sub + 1) * 128],
                                            identb[:])
                        out_c = stE2.tile([128, 128], F32, tag="out_c")
                        nc.vector.tensor_tensor(
                            out_c[:], ptf[:], x2_sb[:, tb, co * 128:(co + 1) * 128],
                            ALU.add)
                        nc.sync.dma_start(
                            out_d.ap()[tb * 128:(tb + 1) * 128,
                                       co * 128:(co + 1) * 128], out_c[:])
    es_ff.close()
    es_x2.close()
    es_at.close()   # free attnT8
    es_wp.close()   # free Wp


def make_nc():
    nc = bacc.Bacc("TRN2", target_bir_lowering=False, debug=False,
                   num_devices=N_CORES)
    build(nc)
    nc.compile()
    return nc


def shard_inputs(inputs):
    """Full inputs dict -> list of 8 per-core in_maps.

    Folds LN1 gain/bias into Wq/Wk/Wv (weights scaled by g1 per input channel,
    be1 contribution becomes an additive bias on q/k/v) and LN2's into W1/b1.
    Wq/Wk/Wv/Wp are fp8 at 32x scale; W1/W2 are bf16.
    """
    E4 = mybir.dt.np(F8)
    BF = mybir.dt.np(BF16)
    x = np.asarray(inputs["x"], np.float32)
    assert x.shape == (B, T, C)
    f64 = np.float64
    Wq = np.asarray(inputs["Wq"], f64); Wk = np.asarray(inputs["Wk"], f64)
    Wv = np.asarray(inputs["Wv"], f64); Wp = np.asarray(inputs["Wp"], f64)
    W1 = np.asarray(inputs["W1"], f64); W2 = np.asarray(inputs["W2"], np.float32)
    g1 = np.asarray(inputs["g1"], f64); be1 = np.asarray(inputs["be1"], f64)
    g2 = np.asarray(inputs["g2"], f64); be2 = np.asarray(inputs["be2"], f64)
    b1 = np.asarray(inputs["b1"], f64)
    shared = {
        "Wq": (WSC * g1[:, None] * Wq).astype(np.float32).astype(E4),
        "Wk": (WSC * g1[:, None] * Wk).astype(np.float32).astype(E4),
        "Wv": (WSC * g1[:, None] * Wv).astype(np.float32).astype(E4),
        "Wp": (WSC * Wp).astype(np.float32).astype(E4),
        "W1": (WSC * g2[:, None] * W1).astype(np.float32).astype(E4),
        "W1lo": ((WSC * g2[:, None] * W1).astype(np.float32)
                 - (WSC * g2[:, None] * W1).astype(np.float32).astype(E4)
                   .astype(np.float32)).astype(E4),
        "W2": (W2SC * W2).astype(E4),
        "W2lo": (W2SC * W2 - (W2SC * W2).astype(E4).astype(np.float32)).astype(E4),
        "qbias": (be1 @ Wq).astype(np.float32).reshape(NB, 128),
        "kbias": (be1 @ Wk).astype(np.float32).reshape(NB, 128),
        "vbias": (WSC * (be1 @ Wv)).astype(np.float32).reshape(1, C),
        "b1": (b1 + be2 @ W1).astype(np.float32).reshape(1, FF),
        "bp": np.asarray(inputs["bp"], np.float32).reshape(1, C),
        "b2": np.ascontiguousarray(
            np.asarray(inputs["b2"], np.float32).reshape(CCH, 128).T),
    }
    in_maps = []
    for c in range(N_CORES):
        b, par = c // 2, c % 2
        # permuted sequence: own (parity) blocks first, partner blocks after,
        # so the program is parity-independent; key positions ride as data
        gblocks = [2 * j + par for j in range(NB)] + \
                  [2 * j + (1 - par) for j in range(NB)]
        xperm = np.concatenate([x[b, g * 128:(g + 1) * 128, :] for g in gblocks], 0)
        # bp is folded into the residual rows (x2 = xown + bp + attn @ Wp)
        rows = xperm[:TOK] + np.asarray(inputs["bp"], np.float32)[None, :]
        qpos = np.stack([np.arange(g * 128, (g + 1) * 128, dtype=np.float32)
                         for g in gblocks[:NB]], 0)
        kposb = np.array([[g * 128 for g in gblocks]], dtype=np.float32)
        m = {"xfull": np.ascontiguousarray(xperm),
             "xown": np.ascontiguousarray(rows), "qpos": qpos, "kposb": kposb}
        m.update(shared)
        in_maps.append(m)
    return in_maps


def unshard_outputs(results):
    """list of per-core {'out': [TOK, C]} -> [B, T, C]"""
    out = np.zeros((B, T, C), np.float32)
    for c in range(N_CORES):
        b, par = c // 2, c % 2
        r = np.asarray(results[c]["out"])
        for j in range(NB):
            g = 2 * j + par
            out[b, g * 128:(g + 1) * 128, :] = r[j * 128:(j + 1) * 128, :]
    return out


_NC_CACHE = {}

def _get_nc():
    if "nc" not in _NC_CACHE:
        nc = bacc.Bacc("TRN2", target_bir_lowering=False, debug=False,
                       num_devices=N_CORES)
        build(nc, reps=1)
        nc.compile()
        _NC_CACHE["nc"] = nc
    return _NC_CACHE["nc"]


def kernel(**inputs):
    from concourse.bass_utils import run_bass_kernel_spmd
    nc = _get_nc()
    in_maps = shard_inputs(inputs)
    res = run_bass_kernel_spmd(nc, in_maps, core_ids=list(range(N_CORES)))
    return unshard_outputs(res.results)
